# revision 38
# baseline (speedup 1.0000x reference)
"""Bass/Tile kernel for nn_LlamaDecoderLayerDAT on 8 TRN2 cores.

Sharding: DP(batch=2) x TP(4) within batch groups [[0..3],[4..7]].
Core c: batch b=c//4, TP slot g=c%4 (heads 4g..4g+3, dff slice g*2048,
offset-net channel group g, output channel shard g*512..(g+1)*512).

All activations on device live in transposed [channel(part), token(free)]
layout, bf16 compute with fp32 PSUM accumulation.

Collective plan (all within the 4-core TP group):
  - sampT: AllGather (issued early, overlapped with q/k/v projections)
  - o-projection: per token half: ReduceScatter (each core gets its
    512-channel quarter of the o-sum) then AllGather back to full C;
    residual h2 = hTd + o_full assembled on the consumer side.
  - MLP down-projection: per token half: ReduceScatter only; each core
    emits outT shard = hTq + o_quarter + mlp_quarter; the host
    reassembles the 4 channel shards per batch.
Attention/o-proj/MLP are issued in token-half phases so no engine queue
ever blocks on a later collective (head-of-line) before earlier-phase
compute has been issued.
"""
import numpy as np
import ml_dtypes
from contextlib import ExitStack

import concourse.bass as bass
import concourse.bacc as bacc
import concourse.tile as tile
from concourse import mybir

BF = mybir.dt.bfloat16
F32 = mybir.dt.float32
I32 = mybir.dt.int32
AF = mybir.ActivationFunctionType
OP = mybir.AluOpType

P = 128
NQ, C, NH, HD = 1024, 2048, 16, 128
DFF = 8192
LR, HR, NIMG, NPAD = 24, 48, 576, 640
CA = C // P              # 16 K-tiles over channels
SCALE = float(1.0 / np.sqrt(HD))
GROUPS = [[0, 1, 2, 3], [4, 5, 6, 7]]
NEG = -1.0e30
bf16 = ml_dtypes.bfloat16
DACC_POOL = False


def _bf(x):
    return np.asarray(x, np.float32).astype(bf16)


# ----------------------------------------------------------------- host side
def _rope_tables():
    inv = 1.0 / (10000.0 ** (np.arange(0, HD, 2, dtype=np.float32) / HD))
    ang = np.arange(NQ, dtype=np.float32)[:, None] * inv[None, :]
    ang = np.concatenate([ang, ang], axis=-1)                 # [NQ, 128]
    sgn = np.ones((HD,), np.float32)
    sgn[: HD // 2] = -1.0
    return np.cos(ang).T.copy(), (np.sin(ang) * sgn[None, :]).T.copy()


def _grid640():
    ys = (np.linspace(0.5, LR - 0.5, LR, dtype=np.float32) / (LR - 1.0)) * 2 - 1
    gy, gx = np.meshgrid(ys, ys, indexing="ij")
    g = np.zeros((NPAD, 2), np.float32)
    g[:NIMG, 0] = gy.reshape(-1)
    g[:NIMG, 1] = gx.reshape(-1)
    return g


def prep_inputs(inputs):
    """Full problem inputs -> list of 8 per-core in_maps."""
    W = {k: np.asarray(v, np.float32) for k, v in inputs.items()}
    hid = W["hidden_states"]
    img = W["image_hd_features"]
    cosT, sinT = _rope_tables()
    kk = np.arange(P)
    maskd = np.where(kk[:, None] > kk[None, :], np.float32(NEG),
                     np.float32(0.0))
    swap = np.zeros((P, P), np.float32)
    swap[np.arange(P), (np.arange(P) + 64) % P] = 1.0
    shared = dict(
        cost=_bf(cosT), sint=_bf(sinT), grid=_grid640(),
        maskd=maskd, swapm=_bf(swap),
        idb=_bf(np.eye(P)), idf=np.eye(P, dtype=np.float32),
        onesb=_bf(np.ones((P, P))), onesf=np.ones((P, P), np.float32),
        convw=np.ascontiguousarray(W["conv_dw_w"].reshape(512, 9)),
        wlr=_bf(W["Wlrproj"]), wint=_bf(W["Wint"]), woff=_bf(W["Woff"]),
    )
    maps = []
    for c in range(8):
        b, g = c // 4, c % 4
        hT = np.ascontiguousarray(hid[b].T)                   # [C, NQ]
        s = 1.0 / np.sqrt((hid[b] ** 2).mean(-1) + 1e-5)      # [NQ]
        hTn = hT * s[None, :]
        img_g = np.ascontiguousarray(img[b][:, g * 512:(g + 1) * 512])
        flat = img_g.reshape(-1)
        st = flat.strides[0]
        imgp = np.zeros((HR * HR, 1024), np.float32)
        imgp[:HR * HR - 1] = np.lib.stride_tricks.as_strided(
            flat, (HR * HR - 1, 1024), (st * 512, st))
        imgp[HR * HR - 1, :512] = img_g[HR * HR - 1]
        hsl = slice(g * 512, (g + 1) * 512)
        fsl = slice(g * 2048, (g + 1) * 2048)
        m = dict(shared)
        m.update(
            hTn=_bf(hTn), hTd=_bf(hT), hTq=_bf(hT[hsl]),
            qidx=(g * 512 + np.arange(4, dtype=np.int32)[None, :] * 128
                  + np.arange(P, dtype=np.int32)[:, None]).copy(),
            lrin=_bf(hTn[hsl, :NIMG]),
            imgp=_bf(imgp),
            wq=_bf(W["Wq"][:, hsl]), wk=_bf(W["Wk"][:, hsl]),
            wv=_bf(W["Wv"][:, hsl]), wo=_bf(W["Wo"][hsl, :]),
            wkhd=_bf(W["Wk_hd"][:, hsl]), wvhd=_bf(W["Wv_hd"][:, hsl]),
            wgate=_bf(W["Wgate"][:, fsl]), wup=_bf(W["Wup"][:, fsl]),
            wdown=_bf(W["Wdown"][fsl, :]),
        )
        maps.append(m)
    return maps


def finish(results):
    out = np.empty((2, NQ, C), np.float32)
    for b in range(2):
        for g in range(4):
            sh = np.asarray(results[4 * b + g]["outT"]).astype(np.float32)
            out[b, :, g * 512:(g + 1) * 512] = sh.T
    return out


# --------------------------------------------------------------- device side
def build(dbg=False, reps=1, no_cc=False, phase="full"):
    nc = bacc.Bacc("TRN2", num_devices=8)
    D = {}

    def inp(name, shape, dt):
        D[name] = nc.dram_tensor(name, shape, dt, kind="ExternalInput")
        return D[name]

    for n in ("hTn", "hTd"):
        inp(n, [C, NQ], BF)
    inp("hTq", [512, NQ], BF)
    inp("lrin", [512, NIMG], BF)
    inp("imgp", [HR * HR, 1024], BF)
    for n in ("wq", "wk", "wv", "wkhd", "wvhd"):
        inp(n, [C, 512], BF)
    inp("wo", [512, C], BF)
    for n in ("wgate", "wup"):
        inp(n, [C, 2048], BF)
    inp("wdown", [2048, C], BF)
    inp("wlr", [512, 256], BF)
    inp("wint", [C, 256], BF)
    inp("woff", [512, 2], BF)
    inp("convw", [512, 9], F32)
    inp("cost", [P, NQ], BF)
    inp("sint", [P, NQ], BF)
    inp("grid", [NPAD, 2], F32)
    inp("qidx", [P, 4], I32)
    inp("maskd", [P, P], F32)
    for n in ("idb", "onesb", "swapm"):
        inp(n, [P, P], BF)
    for n in ("idf", "onesf"):
        inp(n, [P, P], F32)

    outT = nc.dram_tensor("outT", [512, NQ], BF, kind="ExternalOutput")
    dbg_t = {}
    if dbg:
        for n, shape, dt in (
            ("d_samp", [C, NIMG], BF), ("d_q", [512, NQ], BF),
            ("d_k", [512, NQ], BF), ("d_khd", [512, NIMG], BF),
            ("d_oT", [512, NQ], BF), ("d_h2", [C, NQ], BF),
            ("d_mT", [C, NQ], BF),
        ):
            dbg_t[n] = nc.dram_tensor(n, shape, dt, kind="ExternalOutput")

    with tile.TileContext(nc) as tc, ExitStack() as ctx:
        const = ctx.enter_context(tc.tile_pool(name="const", bufs=1))
        dram = ctx.enter_context(tc.tile_pool(name="dram", bufs=1,
                                              space="DRAM"))
        ps = ctx.enter_context(tc.tile_pool(name="ps", bufs=4, space="PSUM"))
        psd = ctx.enter_context(tc.tile_pool(name="psd", bufs=2, space="PSUM"))
        psm = ctx.enter_context(tc.tile_pool(name="psm", bufs=2, space="PSUM"))

        def psa():
            return ps.tile([P, 512], F32, tag="a", name="psa")

        # ---- persistent consts ----
        cn = {}
        for n, shape, dt in (
            ("idb", [P, P], BF), ("onesb", [P, P], BF), ("swapm", [P, P], BF),
            ("idf", [P, P], F32), ("onesf", [P, P], F32),
            ("maskd", [P, P], F32), ("cost", [P, NQ], BF),
            ("sint", [P, NQ], BF),
        ):
            cn[n] = const.tile(shape, dt, tag=n, name=n)
            nc.sync.dma_start(cn[n][:], D[n][:])
        qidx_sb = const.tile([P, 4], I32, tag="qidx", name="qidx_sb")
        nc.sync.dma_start(qidx_sb[:], D["qidx"][:])
        idb, onesb, swapm = cn["idb"], cn["onesb"], cn["swapm"]
        idf, onesf, maskd = cn["idf"], cn["onesf"], cn["maskd"]
        cost, sint = cn["cost"], cn["sint"]

        # DRAM bounce buffers for collectives
        ag_in = dram.tile([512, NIMG], BF)
        ag_out = dram.tile([C, NIMG], BF)
        ar1_in = [dram.tile([C, 512], BF, name=f"ar1i{i}") for i in range(2)]
        ar1_out = [dram.tile([C, 512], BF, name=f"ar1o{i}") for i in range(2)]
        ar2_in = [dram.tile([C, 512], BF, name=f"ar2i{i}") for i in range(2)]
        rs2_out = [dram.tile([512, 512], BF, name=f"rs2o{i}")
                   for i in range(2)]

        def cc(kind, op, ins, outs):
            if no_cc:
                # debug fallback: local copies standing in for the exchange
                n_in, n_out = ins[0].shape[0], outs[0].shape[0]
                if kind == "AllGather":
                    for i in range(n_out // n_in):
                        nc.sync.dma_start(
                            outs[0].tensor[i * n_in:(i + 1) * n_in, :],
                            ins[0].tensor[:, :])
                else:
                    nc.sync.dma_start(outs[0].tensor[0:n_out, :],
                                      ins[0].tensor[0:n_out, :])
            else:
                nc.gpsimd.collective_compute(
                    kind, op, replica_groups=GROUPS, ins=ins, outs=outs)

        def mlp_section(rep, with_attn=True):
            with ExitStack() as lctx:
                abig = lctx.enter_context(
                    tc.tile_pool(name=f"abig{rep}", bufs=1))
                wbig = lctx.enter_context(
                    tc.tile_pool(name=f"wbig{rep}", bufs=2))
                mwork = lctx.enter_context(
                    tc.tile_pool(name=f"mwork{rep}", bufs=2))
                mst = lctx.enter_context(
                    tc.tile_pool(name=f"mst{rep}", bufs=1))

                mT = abig.tile([P, CA, NQ], BF, tag="mT", name="mT")
                hTr = D["hTd"].rearrange("(a p) n -> p a n", p=P)

                def wchunk(src, j):
                    wt = wbig.tile([P, CA, 512], BF, tag="w", name="wt")
                    nc.sync.dma_start(
                        wt[:], src[:, j * 512:(j + 1) * 512]
                        .rearrange("(a p) m -> p a m", p=P))
                    return wt

                for ci in range(2):
                    lo_c, hi_c = ci * 512, (ci + 1) * 512
                    # --- assemble h2 (in place into osum) for this half ---
                    osum = mwork.tile([P, CA, 512], BF, tag="osum",
                                      name="osum")
                    if with_attn:
                        # Activation-queue DMA: this read waits on the
                        # AllReduce; on the SP queue it would head-of-line
                        # block the MLP weight stream.
                        nc.scalar.dma_start(
                            osum[:],
                            ar1_out[ci].rearrange("(a p) n -> p a n", p=P))
                        hTh = mwork.tile([P, CA, 512], BF, tag="hTh",
                                         name="hTh", bufs=1)
                        nc.sync.dma_start(hTh[:], hTr[:, :, lo_c:hi_c])
                        for a in range(CA):
                            nc.vector.tensor_add(osum[:, a, :],
                                                 osum[:, a, :], hTh[:, a, :])
                    else:
                        nc.sync.dma_start(osum[:], hTr[:, :, lo_c:hi_c])
                    # --- rmsnorm stats ---
                    var_ps = psd.tile([1, 512], F32, tag="d", name="var")
                    for a in range(CA):
                        sq = mwork.tile([P, 512], F32, tag="sq",
                                        name="sq", bufs=3)
                        nc.scalar.activation(sq[:], osum[:, a, :], AF.Square)
                        nc.tensor.matmul(var_ps[:], onesf[:, 0:1], sq[:],
                                         start=(a == 0), stop=(a == CA - 1))
                    sd2 = mst.tile([1, 512], F32, tag="sd2", name="sd2",
                                   bufs=2)
                    s2b = mst.tile([1, 512], BF, tag="s2b", name="s2b",
                                   bufs=2)
                    nc.vector.tensor_scalar(
                        out=sd2[:], in0=var_ps[:], scalar1=1.0 / C,
                        scalar2=1e-5, op0=OP.mult, op1=OP.add)
                    nc.scalar.activation(sd2[:], sd2[:], AF.Sqrt)
                    nc.vector.reciprocal(sd2[:], sd2[:])
                    nc.vector.tensor_copy(s2b[:], sd2[:])
                    s2bb = mst.tile([P, 512], BF, tag="s2bb", name="s2bb",
                                    bufs=2)
                    sb_ps = psa()
                    nc.tensor.matmul(sb_ps[:], onesb[0:1, :], s2b[0:1, :],
                                     start=True, stop=True)
                    nc.scalar.copy(s2bb[:], sb_ps[:])
                    for a in range(CA):
                        nc.vector.tensor_mul(mT[:, a, lo_c:hi_c],
                                             osum[:, a, :], s2bb[:])
                    if dbg:
                        nc.sync.dma_start(
                            dbg_t["d_h2"][:, lo_c:hi_c]
                            .rearrange("(a p) n -> p a n", p=P), osum[:])
                        if ci == 1:
                            nc.sync.dma_start(
                                dbg_t["d_mT"].rearrange("(a p) n -> p a n",
                                                        p=P), mT[:])

                    # --- MLP for this half ---
                    gact = mwork.tile([P, CA, 512], BF, tag="gact",
                                      name="gact")
                    for j in range(4):
                        wg = wchunk(D["wgate"], j)
                        for mfl in range(4):
                            mf = j * 4 + mfl
                            pp = psa()
                            for a in range(CA):
                                nc.tensor.matmul(
                                    pp[:], wg[:, a, mfl * P:(mfl + 1) * P],
                                    mT[:, a, lo_c:hi_c],
                                    start=(a == 0), stop=(a == CA - 1))
                            sgm = mwork.tile([P, 512], BF, tag="sgm",
                                             name="sgm", bufs=3)
                            nc.scalar.activation(sgm[:], pp[:], AF.Sigmoid)
                            nc.vector.tensor_mul(gact[:, mf, :], pp[:],
                                                 sgm[:])
                    for j in range(4):
                        wu = wchunk(D["wup"], j)
                        for mfl in range(4):
                            mf = j * 4 + mfl
                            pp = psa()
                            for a in range(CA):
                                nc.tensor.matmul(
                                    pp[:], wu[:, a, mfl * P:(mfl + 1) * P],
                                    mT[:, a, lo_c:hi_c],
                                    start=(a == 0), stop=(a == CA - 1))
                            nc.vector.tensor_mul(gact[:, mf, :], pp[:],
                                                 gact[:, mf, :])
                    for j in range(4):
                        wd = wchunk(D["wdown"], j)
                        for mcl in range(4):
                            pp = psa()
                            for a in range(CA):
                                nc.tensor.matmul(
                                    pp[:], wd[:, a, mcl * P:(mcl + 1) * P],
                                    gact[:, a, :],
                                    start=(a == 0), stop=(a == CA - 1))
                            dev = mwork.tile([P, 512], BF, tag="dev",
                                             name="dev", bufs=3)
                            nc.scalar.copy(dev[:], pp[:])
                            nc.scalar.dma_start(
                                ar2_in[ci][(j * 4 + mcl) * P:
                                           (j * 4 + mcl + 1) * P, :],
                                dev[:])
                    cc("ReduceScatter", OP.add, [ar2_in[ci][:]],
                       [rs2_out[ci][:]])

                # --- final assembly: outT = hTq + o_q + mlp_q ---
                hqr = D["hTq"].rearrange("(a p) n -> p a n", p=P)
                for ci in range(2):
                    lo_c, hi_c = ci * 512, (ci + 1) * 512
                    hq = mst.tile([P, 4, 512], BF, tag="hq", name="hq",
                                  bufs=2)
                    nc.sync.dma_start(hq[:], hqr[:, :, lo_c:hi_c])
                    if with_attn:
                        r1 = mst.tile([P, 4, 512], BF, tag="r1", name="r1",
                                      bufs=2)
                        for a in range(4):
                            nc.gpsimd.indirect_dma_start(
                                out=r1[:, a, :], out_offset=None,
                                in_=ar1_out[ci][:],
                                in_offset=bass.IndirectOffsetOnAxis(
                                    ap=qidx_sb[:, a:a + 1], axis=0))
                        nc.vector.tensor_add(hq[:], hq[:], r1[:])
                    r2 = mst.tile([P, 4, 512], BF, tag="r2", name="r2",
                                  bufs=2)
                    nc.sync.dma_start(
                        r2[:], rs2_out[ci].rearrange("(a p) n -> p a n", p=P))
                    nc.vector.tensor_add(hq[:], hq[:], r2[:])
                    nc.sync.dma_start(
                        outT[:, lo_c:hi_c].rearrange("(a p) n -> p a n", p=P),
                        hq[:])


        def layer(rep):
            if phase == "mlp":
                mlp_section(rep, with_attn=False)
                return
            actx = ExitStack()
            att = actx.enter_context(tc.tile_pool(name=f"att{rep}", bufs=1))

            # q/k/v/oT (read until the end of attention) sit at the
            # base of the pool; hTn (dead after the projections) goes
            # above them, so the MLP weight pool reuses hTn's region
            # and its prefetch DMAs don't wait for attention to finish.
            q_sb = att.tile([P, 4, NQ], BF, tag="q")
            k_sb = att.tile([P, 4, NQ], BF, tag="k")
            v_sb = att.tile([P, 8, 512], BF, tag="v")
            oT_sb = att.tile([P, 4, NQ], BF, tag="oT")
            hTn_sb = att.tile([P, CA, NQ], BF, tag="hTn")
            hTn_r = D["hTn"].rearrange("(a p) n -> p a n", p=P)
            for ch in range(4):
                nc.sync.dma_start(
                    hTn_sb[:, ch * 4:(ch + 1) * 4, :],
                    hTn_r[:, ch * 4:(ch + 1) * 4, :])

            # =========================================================
            # offset network + q/k/v projections, interleaved issue so
            # the DVE/Act-heavy offset net hides under qkv matmuls and
            # the sampT AllGather overlaps the tail of the projections.
            # pre/wk2 sit at the top of the SBUF stack and are released
            # before the hd-projection tiles (khdp) allocate, so the
            # causal-attention pool (hw) below never waits on them.
            # =========================================================
            wpr = actx.enter_context(tc.tile_pool(name=f"wpra{rep}", bufs=2))
            rtp = actx.enter_context(tc.tile_pool(name=f"rtpa{rep}", bufs=3))
            hw = actx.enter_context(tc.tile_pool(name=f"hw{rep}", bufs=1))
            pctx = ExitStack()
            pre = pctx.enter_context(tc.tile_pool(name=f"pre{rep}", bufs=1))
            wk2 = pctx.enter_context(tc.tile_pool(name=f"wk2{rep}", bufs=1))

            # ---- offset stage 1: small DMAs + padded lr input ----
            grid_sb = pre.tile([P, 5, 2], F32, tag="grid")
            nc.sync.dma_start(
                grid_sb[:], D["grid"].rearrange("(s p) c -> p s c", p=P))
            convw_sb = pre.tile([P, 4, 9], F32, tag="convw")
            nc.sync.dma_start(
                convw_sb[:], D["convw"].rearrange("(a p) k -> p a k", p=P))
            wlr_sb = pre.tile([P, 4, 256], BF, tag="wlr")
            nc.sync.dma_start(
                wlr_sb[:], D["wlr"].rearrange("(a p) m -> p a m", p=P))
            woff_sb = pre.tile([P, 4, 2], BF, tag="woff")
            nc.sync.dma_start(
                woff_sb[:], D["woff"].rearrange("(a p) m -> p a m", p=P))
            lrin_sb = pre.tile([P, 4, NIMG], BF, tag="lrin")
            nc.sync.dma_start(
                lrin_sb[:], D["lrin"].rearrange("(a p) n -> p a n", p=P))
            xpad = pre.tile([P, 4, 26 * 26], BF, tag="xpad")
            nc.vector.memset(xpad[:], 0.0)
            acc_sb = pre.tile([P, 4, NIMG], BF, tag="acc")
            for a in range(4):
                x3 = xpad[:, a, :].rearrange("p (y x) -> p y x", y=26)
                nc.vector.tensor_copy(
                    x3[:, 1:25, 1:25],
                    lrin_sb[:, a, :].rearrange("p (y x) -> p y x", y=24))

            def conv_group(a):
                # TensorScalarPtr is DVE-only (Pool rejects it in codegen)
                eng = nc.vector
                x3 = xpad[:, a, :].rearrange("p (y x) -> p y x", y=26)
                a3 = acc_sb[:, a, :].rearrange("p (y x) -> p y x", y=24)
                for ky in range(3):
                    for kx in range(3):
                        w_ap = convw_sb[:, a, ky * 3 + kx:ky * 3 + kx + 1]
                        win = x3[:, ky:ky + 24, kx:kx + 24]
                        if ky == 0 and kx == 0:
                            eng.tensor_scalar(
                                out=a3, in0=win, scalar1=w_ap,
                                scalar2=None, op0=OP.mult)
                        else:
                            eng.scalar_tensor_tensor(
                                out=a3, in0=win, scalar=w_ap, in1=a3,
                                op0=OP.mult, op1=OP.add)

            def pnorm_stats(src_sb, na, eps):
                """mean/var over na*128 partitions (PE ones-matmul sums)"""
                red = wk2.tile([1, NIMG], F32, tag="st", bufs=4, name="red")
                red2 = wk2.tile([1, NIMG], F32, tag="st", bufs=4, name="red2")
                sqs = [wk2.tile([P, NIMG], F32, tag="sq1", bufs=1,
                                name="sq1") for _ in range(1)]
                ones_l = onesf if src_sb.dtype == F32 else onesb
                for lo, hi in ((0, 512), (512, NIMG)):
                    rp = psd.tile([1, 512], F32, tag="d", name="rp")
                    for a in range(na):
                        nc.tensor.matmul(rp[:, :hi - lo], ones_l[:, 0:1],
                                         src_sb[:, a, lo:hi],
                                         start=(a == 0), stop=(a == na - 1))
                    nc.scalar.copy(red[0:1, lo:hi], rp[:, :hi - lo])
                rp2 = psd.tile([1, 512], F32, tag="d", name="rp2")
                rp3 = psd.tile([1, 512], F32, tag="d", name="rp3")
                for a in range(na):
                    sq = sqs[0]
                    nc.scalar.activation(sq[:], src_sb[:, a, :], AF.Square)
                    nc.tensor.matmul(rp2[:], onesf[:, 0:1], sq[:, 0:512],
                                     start=(a == 0), stop=(a == na - 1))
                    nc.tensor.matmul(rp3[:, :NIMG - 512], onesf[:, 0:1],
                                     sq[:, 512:NIMG],
                                     start=(a == 0), stop=(a == na - 1))
                nc.scalar.copy(red2[0:1, 0:512], rp2[:])
                nc.scalar.copy(red2[0:1, 512:NIMG], rp3[:, :NIMG - 512])
                nch = float(na * P)
                mu = wk2.tile([1, NIMG], F32, tag="st", bufs=4, name="mu")
                nc.scalar.mul(mu[:], red[:], 1.0 / nch)
                var = wk2.tile([1, NIMG], F32, tag="st", bufs=4, name="var")
                nc.vector.tensor_mul(var[:], mu[:], mu[:])
                nc.vector.scalar_tensor_tensor(
                    out=var[:], in0=red2[:], scalar=1.0 / nch,
                    in1=var[:], op0=OP.mult, op1=OP.subtract)
                nc.vector.tensor_scalar(out=var[:], in0=var[:],
                                        scalar1=eps, scalar2=None, op0=OP.add)
                nc.scalar.activation(var[:], var[:], AF.Sqrt)
                inv = wk2.tile([1, NIMG], F32, tag="inv", name="inv")
                nc.vector.reciprocal(inv[:], var[:])
                aoff = wk2.tile([1, NIMG], F32, tag="aoff", name="aoff")
                nc.vector.scalar_tensor_tensor(
                    out=aoff[:], in0=mu[:], scalar=-1.0, in1=inv[:],
                    op0=OP.mult, op1=OP.mult)
                return inv, aoff

            def pnorm_bcast(inv, aoff):
                invb = wk2.tile([1, NIMG], BF, tag="invb", name="invb")
                aofb = wk2.tile([1, NIMG], BF, tag="aofb", name="aofb")
                nc.scalar.copy(invb[:], inv[:])
                nc.scalar.copy(aofb[:], aoff[:])
                ib = wk2.tile([P, NIMG], BF, tag="ibb", name="ibb")
                ab = wk2.tile([P, NIMG], BF, tag="abb", name="abb")
                for lo, hi in ((0, 512), (512, NIMG)):
                    pi = psd.tile([P, 512], F32, tag="d", name="pi")
                    nc.tensor.matmul(pi[:, :hi - lo], onesb[0:1, :],
                                     invb[0:1, lo:hi], start=True, stop=True)
                    nc.scalar.copy(ib[:, lo:hi], pi[:, :hi - lo])
                    pa = psd.tile([P, 512], F32, tag="d", name="pa")
                    nc.tensor.matmul(pa[:, :hi - lo], onesb[0:1, :],
                                     aofb[0:1, lo:hi], start=True, stop=True)
                    nc.scalar.copy(ab[:, lo:hi], pa[:, :hi - lo])
                return ib, ab

            # ---- qkv projection helpers ----
            def rope_evict(dst, raw_sb, pos_lo, pos_hi):
                n = pos_hi - pos_lo
                rp = psa()
                nc.tensor.matmul(rp[:, :n], swapm[:], raw_sb[:, :n],
                                 start=True, stop=True)
                tmp1 = rtp.tile([P, 512], BF, tag="rt1", name="rt1", bufs=2)
                nc.vector.tensor_mul(tmp1[:, :n], raw_sb[:, :n],
                                     cost[:, pos_lo:pos_hi])
                tmp2 = rtp.tile([P, 512], BF, tag="rt2", name="rt2", bufs=2)
                nc.vector.tensor_mul(tmp2[:, :n], rp[:, :n],
                                     sint[:, pos_lo:pos_hi])
                nc.vector.tensor_add(dst[:, pos_lo:pos_hi], tmp1[:, :n],
                                     tmp2[:, :n])

            def qk_load(wname):
                wt = wpr.tile([P, CA, 512], BF, tag="wpr", name="wt")
                nc.sync.dma_start(
                    wt[:], D[wname].rearrange("(a p) m -> p a m", p=P))
                return wt

            def qk_heads(wt, dst, src_sb, heads, n_src):
                for h in heads:
                    for lo, hi in ((0, 512), (512, n_src)):
                        pp = psa()
                        for a in range(CA):
                            nc.tensor.matmul(pp[:, :hi - lo],
                                             wt[:, a, h * P:(h + 1) * P],
                                             src_sb[:, a, lo:hi],
                                             start=(a == 0),
                                             stop=(a == CA - 1))
                        raw = rtp.tile([P, 512], BF, tag="raw", name="raw")
                        nc.scalar.copy(raw[:, :hi - lo], pp[:, :hi - lo])
                        rope_evict(dst[:, h, :], raw, lo, hi)

            # ---- interleaved issue ----
            wt_q = qk_load("wq")
            conv_group(0)
            conv_group(2)
            qk_heads(wt_q, q_sb, hTn_sb, (0, 1), NQ)
            conv_group(1)
            conv_group(3)
            qk_heads(wt_q, q_sb, hTn_sb, (2, 3), NQ)

            inv1, aoff1 = pnorm_stats(acc_sb, 4, 1e-6)
            ib1, ab1 = pnorm_bcast(inv1, aoff1)
            xg_sb = pre.tile([P, 4, NIMG], BF, tag="xg")
            sgt = wk2.tile([P, NIMG], BF, tag="sgt", name="sgt")
            xh = wk2.tile([P, NIMG], F32, tag="xh", name="xh")
            for a in range(4):
                nc.vector.tensor_mul(xh[:], acc_sb[:, a, :], ib1[:])
                nc.vector.tensor_add(xh[:], xh[:], ab1[:])
                nc.scalar.activation(sgt[:], xh[:], AF.Sigmoid, scale=1.702)
                nc.vector.tensor_mul(xg_sb[:, a, :], xh[:], sgt[:])

            wt_k = qk_load("wk")
            qk_heads(wt_k, k_sb, hTn_sb, (0, 1), NQ)
            if dbg:
                nc.sync.dma_start(
                    dbg_t["d_q"].rearrange("(h p) n -> p h n", p=P), q_sb[:])

            # intent vector
            hmean = wk2.tile([P, CA], F32, tag="hmean", name="hmean")
            hmb = wk2.tile([P, CA], BF, tag="hmb", name="hmb")
            for a in range(CA):
                nc.vector.tensor_reduce(
                    hmean[:, a:a + 1], hTn_sb[:, a, :],
                    axis=mybir.AxisListType.X, op=OP.add)
            nc.vector.tensor_copy(hmb[:], hmean[:])
            intent = wk2.tile([P, 2], F32, tag="intent", name="intent")
            for m in range(4):
                wint_sb = wk2.tile([P, CA, 64], BF, tag="wint",
                                   name="wint_sb", bufs=1)
                nc.sync.dma_start(
                    wint_sb[:],
                    D["wint"][:, m * 64:(m + 1) * 64]
                    .rearrange("(a p) m -> p a m", p=P))
                ip = psm.tile([P, P], F32, tag="t", name="ip")
                prow = slice((m % 2) * 64, (m % 2) * 64 + 64)
                for a in range(CA):
                    nc.tensor.matmul(ip[prow, 0:1], wint_sb[:, a, :],
                                     hmb[:, a:a + 1], start=(a == 0),
                                     stop=(a == CA - 1))
                nc.scalar.mul(intent[prow, m // 2:m // 2 + 1],
                              ip[prow, 0:1], 1.0 / NQ)

            # cat = [xproj ; intent] -> ln2 (in place) -> off
            cat_sb = pre.tile([P, 4, NIMG], BF, tag="cat")
            for m in range(2):
                for lo, hi in ((0, 512), (512, NIMG)):
                    xp = psd.tile([P, 512], F32, tag="d", name="xp")
                    for a in range(4):
                        nc.tensor.matmul(xp[:, :hi - lo],
                                         wlr_sb[:, a, m * P:(m + 1) * P],
                                         xg_sb[:, a, lo:hi],
                                         start=(a == 0), stop=(a == 3))
                    nc.scalar.copy(cat_sb[:, m, lo:hi], xp[:, :hi - lo])
            for m in range(2):
                nc.vector.tensor_scalar(
                    out=cat_sb[:, 2 + m, :], in0=xg_sb[:, 0, :],
                    scalar1=0.0, scalar2=intent[:, m:m + 1], op0=OP.mult,
                    op1=OP.add)

            qk_heads(wt_k, k_sb, hTn_sb, (2, 3), NQ)
            if dbg:
                nc.sync.dma_start(
                    dbg_t["d_k"].rearrange("(h p) n -> p h n", p=P), k_sb[:])

            inv2, aoff2 = pnorm_stats(cat_sb, 4, 1e-6)
            ib2, ab2 = pnorm_bcast(inv2, aoff2)
            for a in range(4):
                nc.vector.tensor_mul(xh[:], cat_sb[:, a, :], ib2[:])
                nc.vector.tensor_add(cat_sb[:, a, :], xh[:], ab2[:])

            off_sb = wk2.tile([2, NPAD], F32, tag="off", name="off")
            nc.vector.memset(off_sb[:], 0.0)
            for lo, hi in ((0, 512), (512, NIMG)):
                op_ = psd.tile([2, 512], F32, tag="d", name="opp")
                for a in range(4):
                    nc.tensor.matmul(op_[:, :hi - lo], woff_sb[:, a, :],
                                     cat_sb[:, a, lo:hi], start=(a == 0),
                                     stop=(a == 3))
                nc.scalar.copy(off_sb[:, lo:hi], op_[:, :hi - lo])

            # bilinear coordinates, batched across all 5 s-tiles
            idx0 = wk2.tile([P, 5], I32, tag="idx0", name="idx0")
            idx1 = wk2.tile([P, 5], I32, tag="idx1", name="idx1")
            wcmb = wk2.tile([P, 5, 4], F32, tag="wcmb", name="wcmb")
            t2 = wk2.tile([P, 5, 2], F32, tag="t2", name="t2")
            fr = wk2.tile([P, 5, 2], F32, tag="fr", name="fr")
            f0 = wk2.tile([P, 5, 2], F32, tag="f0", name="f0")
            f1 = wk2.tile([P, 5, 2], F32, tag="f1", name="f1")
            w1m = wk2.tile([P, 5, 2], F32, tag="w1m", name="w1m")
            fi = wk2.tile([P, 5, 1], F32, tag="fi", name="fi")
            tps_c = psm.tile([P, 5, 2], F32, tag="t", name="tps_c")
            for st in range(5):
                nc.tensor.transpose(tps_c[:, st, :],
                                    off_sb[0:2, st * P:(st + 1) * P],
                                    idf[0:2, 0:2])
            nc.scalar.activation(t2[:], tps_c[:], AF.Tanh)
            nc.vector.scalar_tensor_tensor(
                out=t2[:], in0=t2[:], scalar=2.0 / LR,
                in1=grid_sb[:], op0=OP.mult, op1=OP.add)
            nc.vector.tensor_scalar(out=t2[:], in0=t2[:], scalar1=1.0,
                                    scalar2=-1.0, op0=OP.min, op1=OP.max)
            nc.vector.tensor_scalar(out=t2[:], in0=t2[:], scalar1=1.0,
                                    scalar2=(HR - 1) / 2.0,
                                    op0=OP.add, op1=OP.mult)
            ti = wk2.tile([P, 5, 2], I32, tag="ti", name="ti")
            nc.vector.tensor_copy(ti[:], t2[:])
            nc.vector.tensor_copy(f0[:], ti[:])
            nc.vector.tensor_tensor(out=fr[:], in0=f0[:], in1=t2[:],
                                    op=OP.is_gt)
            nc.vector.tensor_sub(f0[:], f0[:], fr[:])
            nc.vector.tensor_sub(fr[:], t2[:], f0[:])
            nc.vector.tensor_scalar(out=f1[:], in0=f0[:], scalar1=1.0,
                                    scalar2=float(HR - 1), op0=OP.add,
                                    op1=OP.min)
            nc.vector.scalar_tensor_tensor(
                out=fi[:], in0=f0[:, :, 0:1], scalar=float(HR),
                in1=f0[:, :, 1:2], op0=OP.mult, op1=OP.add)
            nc.vector.tensor_copy(idx0[:], fi[:, :, 0])
            nc.vector.scalar_tensor_tensor(
                out=fi[:], in0=f1[:, :, 0:1], scalar=float(HR),
                in1=f0[:, :, 1:2], op0=OP.mult, op1=OP.add)
            nc.vector.tensor_copy(idx1[:], fi[:, :, 0])
            nc.vector.tensor_scalar(out=w1m[:], in0=fr[:],
                                    scalar1=-1.0, scalar2=1.0,
                                    op0=OP.mult, op1=OP.add)
            nc.vector.tensor_mul(wcmb[:, :, 0:1], w1m[:, :, 0:1],
                                 w1m[:, :, 1:2])
            nc.vector.tensor_mul(wcmb[:, :, 1:2], w1m[:, :, 0:1],
                                 fr[:, :, 1:2])
            nc.vector.tensor_mul(wcmb[:, :, 2:3], fr[:, :, 0:1],
                                 w1m[:, :, 1:2])
            nc.vector.tensor_mul(wcmb[:, :, 3:4], fr[:, :, 0:1],
                                 fr[:, :, 1:2])

            wt_v = qk_load("wv")
            for m8 in range(4):
                pp = psa()
                for a in range(CA):
                    nc.tensor.matmul(pp[:],
                                     hTn_sb[:, a, m8 * P:(m8 + 1) * P],
                                     wt_v[:, a, :], start=(a == 0),
                                     stop=(a == CA - 1))
                nc.scalar.copy(v_sb[:, m8, :], pp[:])

            # gather + combine + transpose
            sampT_mine = pre.tile([P, 4, NPAD], BF, tag="sampT_mine")
            for st in range(5):
                p0 = wk2.tile([P, 1024], BF, tag="p0", bufs=1, name="p0")
                p1 = wk2.tile([P, 1024], BF, tag="p1", bufs=1, name="p1")
                nc.gpsimd.indirect_dma_start(
                    out=p0[:], out_offset=None, in_=D["imgp"][:],
                    in_offset=bass.IndirectOffsetOnAxis(
                        ap=idx0[:, st:st + 1], axis=0))
                nc.gpsimd.indirect_dma_start(
                    out=p1[:], out_offset=None, in_=D["imgp"][:],
                    in_offset=bass.IndirectOffsetOnAxis(
                        ap=idx1[:, st:st + 1], axis=0))
                smp = wk2.tile([P, 512], BF, tag="smp", bufs=2, name="smp")
                nc.vector.tensor_tensor(
                    out=smp[:], in0=p0[:, 0:512],
                    in1=wcmb[:, st, 0:1].to_broadcast([P, 512]), op=OP.mult)
                for pair, col in ((p0, 1), (p1, 2), (p1, 3)):
                    src = pair[:, 0:512] if col == 2 else pair[:, 512:1024]
                    nc.vector.scalar_tensor_tensor(
                        out=smp[:], in0=src,
                        scalar=wcmb[:, st, col:col + 1], in1=smp[:],
                        op0=OP.mult, op1=OP.add)
                for cm in range(4):
                    tp = psm.tile([P, P], BF, tag="t", name="tps")
                    nc.tensor.transpose(tp[:], smp[:, cm * P:(cm + 1) * P],
                                        idb[:])
                    nc.scalar.copy(
                        sampT_mine[:, cm, st * P:(st + 1) * P], tp[:])
            nc.sync.dma_start(ag_in.rearrange("(a p) n -> p a n", p=P),
                              sampT_mine[:, :, 0:NIMG])
            cc("AllGather", OP.bypass, [ag_in[:]], [ag_out[:]])

            # remaining v tiles while AllGather flies
            for m8 in range(4, 8):
                pp = psa()
                for a in range(CA):
                    nc.tensor.matmul(pp[:],
                                     hTn_sb[:, a, m8 * P:(m8 + 1) * P],
                                     wt_v[:, a, :], start=(a == 0),
                                     stop=(a == CA - 1))
                nc.scalar.copy(v_sb[:, m8, :], pp[:])

            # =========================================================
            # attention: causal tiles first (they only need q/k/v, so
            # they fill the AllGather window), head-pairs interleaved
            # for PE pipelining; image tiles + softmax finalize after
            # the hd projections land. Denominators accumulate on the
            # otherwise-idle Pool engine.
            # =========================================================
            daccs, oAs = {}, {}
            sampT_sb = khd_sb = vhd_sb = None

            def sc_exp_o(ci, pair, kind, kt, o_pss, start, stop,
                         dinit=False):
                lo_c, hi_c = ci * 512, (ci + 1) * 512
                if kind == "c":
                    qlo, kp = kt * P, P
                else:
                    qlo = 0
                    kp = P if kt < 4 else NIMG - 4 * P
                lo = max(qlo, lo_c)
                n = hi_c - lo
                o = lo - lo_c
                for h in pair:
                    dacc = daccs[(ci, h)]
                    sp = psa()
                    if kind == "c":
                        nc.tensor.matmul(sp[:, :n],
                                         k_sb[:, h, kt * P:(kt + 1) * P],
                                         q_sb[:, h, lo:hi_c],
                                         start=True, stop=True)
                        if lo == qlo:
                            nc.vector.tensor_add(sp[:, 0:P], sp[:, 0:P],
                                                 maskd[:])
                        lhs = v_sb[:, kt, h * P:(h + 1) * P]
                    else:
                        nc.tensor.matmul(sp[:kp, :n],
                                         khd_sb[:, h, kt * P:kt * P + kp],
                                         q_sb[:, h, lo:hi_c],
                                         start=True, stop=True)
                        lhs = vhd_sb[:kp, kt, h * P:(h + 1) * P]
                    ex = hw.tile([P, 512], BF, tag="ex", name="ex", bufs=3)
                    nc.scalar.activation(ex[:kp, o:], sp[:kp, :n], AF.Exp,
                                         scale=SCALE)
                    nc.tensor.matmul(o_pss[h][:, o:], lhs, ex[:kp, o:],
                                     start=start, stop=stop)
                    deng = nc.gpsimd if DACC_POOL else nc.vector
                    if dinit:
                        deng.tensor_copy(dacc[:kp, :], ex[:kp, :])
                    else:
                        deng.tensor_add(dacc[:kp, o:], dacc[:kp, o:],
                                        ex[:kp, o:])

            def causal_pass(ci, pair):
                ncaus = 4 * (ci + 1)
                o_pss = {h: psa() for h in pair}
                for h in pair:
                    daccs[(ci, h)] = hw.tile([P, 512], F32, tag="dacc",
                                             name=f"dc{ci}{h}", bufs=8)
                for kt in range(ncaus):
                    sc_exp_o(ci, pair, "c", kt, o_pss,
                             start=(kt == 0), stop=(kt == ncaus - 1),
                             dinit=(kt == 0))
                for h in pair:
                    oA = hw.tile([P, 512], BF, tag="oA",
                                 name=f"oA{ci}{h}", bufs=8)
                    nc.scalar.copy(oA[:], o_pss[h][:])
                    oAs[(ci, h)] = oA

            def img_pass(ci, pair):
                lo_c, hi_c = ci * 512, (ci + 1) * 512
                o_pss = {h: psa() for h in pair}
                for it in range(5):
                    sc_exp_o(ci, pair, "i", it, o_pss,
                             start=(it == 0), stop=(it == 4))
                for h in pair:
                    den = psd.tile([1, 512], F32, tag="d", name="den")
                    nc.tensor.matmul(den[:], onesf[:, 0:1],
                                     daccs[(ci, h)][:], start=True, stop=True)
                    rcf = hw.tile([1, 512], F32, tag="rcf", name="rcf",
                                  bufs=1)
                    rcb = hw.tile([1, 512], BF, tag="rcb", name="rcb",
                                  bufs=2)
                    nc.vector.reciprocal(rcf[:], den[:])
                    nc.vector.tensor_copy(rcb[:], rcf[:])
                    rb = psa()
                    nc.tensor.matmul(rb[:], onesb[0:1, :], rcb[0:1, :],
                                     start=True, stop=True)
                    rbs = hw.tile([P, 512], BF, tag="rbs", name="rbs",
                                  bufs=1)
                    nc.scalar.copy(rbs[:], rb[:])
                    otmp = hw.tile([P, 512], BF, tag="otmp", name="otmp",
                                   bufs=1)
                    nc.vector.tensor_add(otmp[:], oAs[(ci, h)][:],
                                         o_pss[h][:])
                    nc.vector.tensor_mul(oT_sb[:, h, lo_c:hi_c], otmp[:],
                                         rbs[:])

            # causal part of attention (during the AllGather flight)
            for ci in range(2):
                for pair in ((0, 1), (2, 3)):
                    causal_pass(ci, pair)

            pctx.close()

            # hd-tile pool reuses the released pre/wk2 region; its
            # writes only depend on the AllGather anyway.
            khdp = actx.enter_context(tc.tile_pool(name=f"khdp{rep}",
                                                   bufs=1))
            sampT_sb = khdp.tile([P, CA, NIMG], BF, tag="sampT")
            khd_sb = khdp.tile([P, 4, NIMG], BF, tag="khd")
            vhd_sb = khdp.tile([P, 5, 512], BF, tag="vhd")
            wo_sb = khdp.tile([P, 4, C], BF, tag="wo")
            wt_khd = qk_load("wkhd")
            wt_vhd = qk_load("wvhd")
            nc.sync.dma_start(
                wo_sb[:], D["wo"].rearrange("(a p) m -> p a m", p=P))
            # Activation-queue DMA: waits on the AllGather; on SP it would
            # block the o-proj eviction stream and MLP weight prefetch.
            nc.scalar.dma_start(
                sampT_sb[:], ag_out.rearrange("(a p) n -> p a n", p=P))
            if dbg:
                nc.sync.dma_start(
                    dbg_t["d_samp"].rearrange("(a p) n -> p a n", p=P),
                    sampT_sb[:])

            # ---- hd-token projections (need the AllGather result) ----
            for h in range(4):
                for lo, hi in ((0, 512), (512, NIMG)):
                    pp = psa()
                    for a in range(CA):
                        nc.tensor.matmul(pp[:, :hi - lo],
                                         wt_khd[:, a, h * P:(h + 1) * P],
                                         sampT_sb[:, a, lo:hi],
                                         start=(a == 0), stop=(a == CA - 1))
                    raw = rtp.tile([P, 512], BF, tag="raw", name="raw")
                    nc.scalar.copy(raw[:, :hi - lo], pp[:, :hi - lo])
                    rope_evict(khd_sb[:, h, :], raw, lo, hi)
            if dbg:
                nc.sync.dma_start(
                    dbg_t["d_khd"].rearrange("(h p) n -> p h n", p=P),
                    khd_sb[:])

            for st in range(5):
                kp = P if st < 4 else NIMG - 4 * P
                pp = psa()
                for a in range(CA):
                    nc.tensor.matmul(pp[:kp, :],
                                     sampT_sb[:, a, st * P:st * P + kp],
                                     wt_vhd[:, a, :], start=(a == 0),
                                     stop=(a == CA - 1))
                nc.scalar.copy(vhd_sb[:kp, st, :], pp[:kp, :])

            # ---- image attention + o-projection per token half ----
            for ci in range(2):
                lo_c, hi_c = ci * 512, (ci + 1) * 512
                img_pass(ci, (0, 1))
                img_pass(ci, (2, 3))
                if dbg and ci == 1:
                    nc.sync.dma_start(
                        dbg_t["d_oT"].rearrange("(h p) n -> p h n", p=P),
                        oT_sb[:])

                # o-projection for this token half -> ReduceScatter -> AG
                for m in range(CA):
                    pp = psa()
                    for h in range(4):
                        nc.tensor.matmul(pp[:], wo_sb[:, h, m * P:(m + 1) * P],
                                         oT_sb[:, h, lo_c:hi_c],
                                         start=(h == 0), stop=(h == 3))
                    oev = khdp.tile([P, 512], BF, tag="oev", bufs=3,
                                    name="oev")
                    nc.scalar.copy(oev[:], pp[:])
                    # Act-queue DMA: an SP-queue write here would stall SP
                    # on o-proj completion and block MLP weight prefetch.
                    nc.scalar.dma_start(ar1_in[ci][m * P:(m + 1) * P, :],
                                        oev[:])
                cc("AllReduce", OP.add, [ar1_in[ci][:]], [ar1_out[ci][:]])

            actx.close()

            if phase == "attn":
                with ExitStack() as lctx:
                    mstx = lctx.enter_context(
                        tc.tile_pool(name=f"mstx{rep}", bufs=2))
                    hqr = D["hTq"].rearrange("(a p) n -> p a n", p=P)
                    for ci in range(2):
                        lo_c, hi_c = ci * 512, (ci + 1) * 512
                        hq = mstx.tile([P, 4, 512], BF, tag="hq", name="hq")
                        nc.sync.dma_start(hq[:], hqr[:, :, lo_c:hi_c])
                        r1 = mstx.tile([P, 4, 512], BF, tag="r1", name="r1")
                        for a in range(4):
                            nc.gpsimd.indirect_dma_start(
                                out=r1[:, a, :], out_offset=None,
                                in_=ar1_out[ci][:],
                                in_offset=bass.IndirectOffsetOnAxis(
                                    ap=qidx_sb[:, a:a + 1], axis=0))
                        nc.vector.tensor_add(hq[:], hq[:], r1[:])
                        nc.sync.dma_start(
                            outT[:, lo_c:hi_c]
                            .rearrange("(a p) n -> p a n", p=P), hq[:])
                return

            mlp_section(rep, with_attn=True)

        for rep in range(reps):
            layer(rep)

    nc.compile()
    return nc


import time
import jax
from jax.sharding import Mesh, PartitionSpec
from jax.experimental.shard_map import shard_map
from concourse import bass2jax
from concourse.bass2jax import _bass_exec_p, install_neuronx_cc_hook, \
    partition_id_tensor


class TimedRunner:
    def __init__(self, nc, n_cores=8):
        install_neuronx_cc_hook()
        self.nc = nc
        self.n_cores = n_cores
        partition_name = (nc.partition_id_tensor.name
                          if nc.partition_id_tensor else None)
        in_names, out_names, out_avals, zero_outs = [], [], [], []
        for alloc in nc.m.functions[0].allocations:
            if not isinstance(alloc, mybir.MemoryLocationSet):
                continue
            name = alloc.memorylocations[0].name
            if alloc.kind == "ExternalInput":
                if name != partition_name:
                    in_names.append(name)
            elif alloc.kind == "ExternalOutput":
                out_names.append(name)
                shape = tuple(alloc.tensor_shape)
                dtype = mybir.dt.np(alloc.dtype)
                out_avals.append(jax.core.ShapedArray(shape, dtype))
                zero_outs.append(np.zeros(shape, dtype))
        if nc.dbg_addr is not None:
            assert not nc.dbg_callbacks
        self.in_names = list(in_names)
        self.out_names = out_names
        self.out_avals = out_avals
        self.zero_outs = zero_outs
        n_params = len(in_names)
        n_outs = len(out_avals)
        all_in_names = list(in_names) + list(out_names)
        if partition_name is not None:
            all_in_names.append(partition_name)
        self.partition_name = partition_name

        def _body(*args):
            operands = list(args)
            if partition_name is not None:
                operands.append(partition_id_tensor())
            outs = _bass_exec_p.bind(
                *operands,
                out_avals=tuple(out_avals),
                in_names=tuple(all_in_names),
                out_names=tuple(out_names),
                lowering_input_output_aliases=(),
                sim_require_finite=True,
                sim_require_nnan=True,
                nc=nc,
            )
            return tuple(outs)

        devices = jax.devices()[:n_cores]
        mesh = Mesh(np.asarray(devices), ("core",))
        in_specs = (PartitionSpec("core"),) * (n_params + n_outs)
        out_specs = (PartitionSpec("core"),) * n_outs
        # no donation so the function is re-callable with the same buffers
        self.fn = jax.jit(shard_map(_body, mesh=mesh, in_specs=in_specs,
                                    out_specs=out_specs, check_rep=False))
        self.mesh = mesh

    def put_inputs(self, in_maps):
        dbg = {}
        if self.nc.dbg_addr is not None:
            dbg = {self.nc.dbg_addr.name: np.zeros((1, 2), np.uint32)}
        per_core = [[np.asarray({**m, **dbg}[n]) for n in self.in_names]
                    for m in in_maps]
        n_params = len(self.in_names)
        concat_in = [
            np.concatenate([per_core[c][i] for c in range(self.n_cores)],
                           axis=0) for i in range(n_params)]
        concat_zeros = [
            np.zeros((self.n_cores * z.shape[0], *z.shape[1:]), z.dtype)
            for z in self.zero_outs]
        sh = jax.sharding.NamedSharding(self.mesh, PartitionSpec("core"))
        self.dev_args = [jax.device_put(a, sh)
                         for a in (*concat_in, *concat_zeros)]

    def run(self):
        outs = jax.block_until_ready(self.fn(*self.dev_args))
        return outs

    def results(self, outs):
        return [
            {n: np.asarray(outs[i]).reshape(
                self.n_cores, *self.out_avals[i].shape)[c]
             for i, n in enumerate(self.out_names)}
            for c in range(self.n_cores)
        ]

    def bench(self, iters=5):
        self.run()
        best = float("inf")
        for _ in range(iters):
            t0 = time.perf_counter()
            self.run()
            best = min(best, time.perf_counter() - t0)
        return best


# ----------------------------------------------------------------- entry
_NC_CACHE = {}


def _get_nc(reps=1):
    if reps not in _NC_CACHE:
        _NC_CACHE[reps] = build(dbg=False, reps=reps)
    return _NC_CACHE[reps]


def kernel(**inputs) -> np.ndarray:
    """Full inputs -> full [2, 1024, 2048] fp32 output, computed on the
    8 TRN2 NeuronCores (DPxTP sharding, bf16 compute)."""
    from concourse.bass_utils import run_bass_kernel_spmd
    nc = _get_nc(1)
    maps = prep_inputs(inputs)
    res = run_bass_kernel_spmd(nc, maps, list(range(8)))
    return finish(res.results)


def benchmark_device_time(inputs, reps_hi=6, npipe=10, trials=9):
    """Per-layer device execution time: difference an on-device
    reps_hi-iteration NEFF against the single-iteration NEFF under
    pipelined launches (axon host dispatch is ~100ms and would otherwise
    swamp the sub-ms kernel). Medians over trials for jitter robustness."""
    import time as _time
    import statistics as _stats
    import jax as _jax
    maps = prep_inputs(inputs)
    per = {}
    for reps in (1, reps_hi):
        tr = TimedRunner(_get_nc(reps), 8)
        tr.put_inputs(maps)
        tr.run()
        samples = []
        for _ in range(trials):
            t0 = _time.perf_counter()
            outs = None
            for _ in range(npipe):
                outs = tr.fn(*tr.dev_args)
            _jax.block_until_ready(outs)
            samples.append((_time.perf_counter() - t0) / npipe)
        per[reps] = _stats.median(samples)
    return max((per[reps_hi] - per[1]) / (reps_hi - 1), 1e-9)


# revision 39
# speedup vs baseline: 1.0110x; 1.0110x over previous
"""Bass/Tile kernel for nn_LlamaDecoderLayerDAT on 8 TRN2 cores.

Sharding: DP(batch=2) x TP(4) within batch groups [[0..3],[4..7]].
Core c: batch b=c//4, TP slot g=c%4 (heads 4g..4g+3, dff slice g*2048,
offset-net channel group g, output channel shard g*512..(g+1)*512).

All activations on device live in transposed [channel(part), token(free)]
layout, bf16 compute with fp32 PSUM accumulation.

Collective plan (all within the 4-core TP group):
  - sampT: AllGather (issued early, overlapped with q/k/v projections)
  - o-projection: per token half: ReduceScatter (each core gets its
    512-channel quarter of the o-sum) then AllGather back to full C;
    residual h2 = hTd + o_full assembled on the consumer side.
  - MLP down-projection: per token half: ReduceScatter only; each core
    emits outT shard = hTq + o_quarter + mlp_quarter; the host
    reassembles the 4 channel shards per batch.
Attention/o-proj/MLP are issued in token-half phases so no engine queue
ever blocks on a later collective (head-of-line) before earlier-phase
compute has been issued.
"""
import numpy as np
import ml_dtypes
from contextlib import ExitStack

import concourse.bass as bass
import concourse.bacc as bacc
import concourse.tile as tile
from concourse import mybir

BF = mybir.dt.bfloat16
F32 = mybir.dt.float32
I32 = mybir.dt.int32
AF = mybir.ActivationFunctionType
OP = mybir.AluOpType

P = 128
NQ, C, NH, HD = 1024, 2048, 16, 128
DFF = 8192
LR, HR, NIMG, NPAD = 24, 48, 576, 640
CA = C // P              # 16 K-tiles over channels
SCALE = float(1.0 / np.sqrt(HD))
GROUPS = [[0, 1, 2, 3], [4, 5, 6, 7]]
NEG = -1.0e30
bf16 = ml_dtypes.bfloat16
DACC_POOL = False


def _bf(x):
    return np.asarray(x, np.float32).astype(bf16)


# ----------------------------------------------------------------- host side
def _rope_tables():
    inv = 1.0 / (10000.0 ** (np.arange(0, HD, 2, dtype=np.float32) / HD))
    ang = np.arange(NQ, dtype=np.float32)[:, None] * inv[None, :]
    ang = np.concatenate([ang, ang], axis=-1)                 # [NQ, 128]
    sgn = np.ones((HD,), np.float32)
    sgn[: HD // 2] = -1.0
    return np.cos(ang).T.copy(), (np.sin(ang) * sgn[None, :]).T.copy()


def _grid640():
    ys = (np.linspace(0.5, LR - 0.5, LR, dtype=np.float32) / (LR - 1.0)) * 2 - 1
    gy, gx = np.meshgrid(ys, ys, indexing="ij")
    g = np.zeros((NPAD, 2), np.float32)
    g[:NIMG, 0] = gy.reshape(-1)
    g[:NIMG, 1] = gx.reshape(-1)
    return g


def prep_inputs(inputs):
    """Full problem inputs -> list of 8 per-core in_maps."""
    W = {k: np.asarray(v, np.float32) for k, v in inputs.items()}
    hid = W["hidden_states"]
    img = W["image_hd_features"]
    cosT, sinT = _rope_tables()
    kk = np.arange(P)
    maskd = np.where(kk[:, None] > kk[None, :], np.float32(NEG),
                     np.float32(0.0))
    swap = np.zeros((P, P), np.float32)
    swap[np.arange(P), (np.arange(P) + 64) % P] = 1.0
    shared = dict(
        cost=_bf(cosT), sint=_bf(sinT), grid=_grid640(),
        maskd=maskd, swapm=_bf(swap),
        idb=_bf(np.eye(P)), idf=np.eye(P, dtype=np.float32),
        onesb=_bf(np.ones((P, P))), onesf=np.ones((P, P), np.float32),
        convw=np.ascontiguousarray(W["conv_dw_w"].reshape(512, 9)),
        wlr=_bf(W["Wlrproj"]), wint=_bf(W["Wint"]), woff=_bf(W["Woff"]),
    )
    maps = []
    for c in range(8):
        b, g = c // 4, c % 4
        hT = np.ascontiguousarray(hid[b].T)                   # [C, NQ]
        s = 1.0 / np.sqrt((hid[b] ** 2).mean(-1) + 1e-5)      # [NQ]
        hTn = hT * s[None, :]
        img_g = np.ascontiguousarray(img[b][:, g * 512:(g + 1) * 512])
        flat = img_g.reshape(-1)
        st = flat.strides[0]
        imgp = np.zeros((HR * HR, 1024), np.float32)
        imgp[:HR * HR - 1] = np.lib.stride_tricks.as_strided(
            flat, (HR * HR - 1, 1024), (st * 512, st))
        imgp[HR * HR - 1, :512] = img_g[HR * HR - 1]
        hsl = slice(g * 512, (g + 1) * 512)
        fsl = slice(g * 2048, (g + 1) * 2048)
        m = dict(shared)
        m.update(
            hTn=_bf(hTn), hTd=_bf(hT), hTq=_bf(hT[hsl]),
            qidx=(g * 512 + np.arange(4, dtype=np.int32)[None, :] * 128
                  + np.arange(P, dtype=np.int32)[:, None]).copy(),
            lrin=_bf(hTn[hsl, :NIMG]),
            imgp=_bf(imgp),
            wq=_bf(W["Wq"][:, hsl]), wk=_bf(W["Wk"][:, hsl]),
            wv=_bf(W["Wv"][:, hsl]), wo=_bf(W["Wo"][hsl, :]),
            wkhd=_bf(W["Wk_hd"][:, hsl]), wvhd=_bf(W["Wv_hd"][:, hsl]),
            wgate=_bf(W["Wgate"][:, fsl]), wup=_bf(W["Wup"][:, fsl]),
            wdown=_bf(W["Wdown"][fsl, :]),
        )
        maps.append(m)
    return maps


def finish(results):
    out = np.empty((2, NQ, C), np.float32)
    for b in range(2):
        for g in range(4):
            sh = np.asarray(results[4 * b + g]["outT"]).astype(np.float32)
            out[b, :, g * 512:(g + 1) * 512] = sh.T
    return out


# --------------------------------------------------------------- device side
def build(dbg=False, reps=1, no_cc=False, phase="full"):
    nc = bacc.Bacc("TRN2", num_devices=8)
    D = {}

    def inp(name, shape, dt):
        D[name] = nc.dram_tensor(name, shape, dt, kind="ExternalInput")
        return D[name]

    for n in ("hTn", "hTd"):
        inp(n, [C, NQ], BF)
    inp("hTq", [512, NQ], BF)
    inp("lrin", [512, NIMG], BF)
    inp("imgp", [HR * HR, 1024], BF)
    for n in ("wq", "wk", "wv", "wkhd", "wvhd"):
        inp(n, [C, 512], BF)
    inp("wo", [512, C], BF)
    for n in ("wgate", "wup"):
        inp(n, [C, 2048], BF)
    inp("wdown", [2048, C], BF)
    inp("wlr", [512, 256], BF)
    inp("wint", [C, 256], BF)
    inp("woff", [512, 2], BF)
    inp("convw", [512, 9], F32)
    inp("cost", [P, NQ], BF)
    inp("sint", [P, NQ], BF)
    inp("grid", [NPAD, 2], F32)
    inp("qidx", [P, 4], I32)
    inp("maskd", [P, P], F32)
    for n in ("idb", "onesb", "swapm"):
        inp(n, [P, P], BF)
    for n in ("idf", "onesf"):
        inp(n, [P, P], F32)

    outT = nc.dram_tensor("outT", [512, NQ], BF, kind="ExternalOutput")
    dbg_t = {}
    if dbg:
        for n, shape, dt in (
            ("d_samp", [C, NIMG], BF), ("d_q", [512, NQ], BF),
            ("d_k", [512, NQ], BF), ("d_khd", [512, NIMG], BF),
            ("d_oT", [512, NQ], BF), ("d_h2", [C, NQ], BF),
            ("d_mT", [C, NQ], BF),
        ):
            dbg_t[n] = nc.dram_tensor(n, shape, dt, kind="ExternalOutput")

    with tile.TileContext(nc) as tc, ExitStack() as ctx:
        const = ctx.enter_context(tc.tile_pool(name="const", bufs=1))
        dram = ctx.enter_context(tc.tile_pool(name="dram", bufs=1,
                                              space="DRAM"))
        ps = ctx.enter_context(tc.tile_pool(name="ps", bufs=4, space="PSUM"))
        psd = ctx.enter_context(tc.tile_pool(name="psd", bufs=2, space="PSUM"))
        psm = ctx.enter_context(tc.tile_pool(name="psm", bufs=2, space="PSUM"))

        def psa():
            return ps.tile([P, 512], F32, tag="a", name="psa")

        # ---- persistent consts ----
        cn = {}
        for n, shape, dt in (
            ("idb", [P, P], BF), ("onesb", [P, P], BF), ("swapm", [P, P], BF),
            ("idf", [P, P], F32), ("onesf", [P, P], F32),
            ("maskd", [P, P], F32), ("cost", [P, NQ], BF),
            ("sint", [P, NQ], BF),
        ):
            cn[n] = const.tile(shape, dt, tag=n, name=n)
            nc.sync.dma_start(cn[n][:], D[n][:])
        qidx_sb = const.tile([P, 4], I32, tag="qidx", name="qidx_sb")
        nc.sync.dma_start(qidx_sb[:], D["qidx"][:])
        idb, onesb, swapm = cn["idb"], cn["onesb"], cn["swapm"]
        idf, onesf, maskd = cn["idf"], cn["onesf"], cn["maskd"]
        cost, sint = cn["cost"], cn["sint"]

        # DRAM bounce buffers for collectives
        ag_in = dram.tile([512, NIMG], BF)
        ag_out = dram.tile([C, NIMG], BF)
        ar1_in = [dram.tile([C, 512], BF, name=f"ar1i{i}") for i in range(2)]
        ar1_out = [dram.tile([C, 512], BF, name=f"ar1o{i}") for i in range(2)]
        ar2_in = [dram.tile([C, 512], BF, name=f"ar2i{i}") for i in range(2)]
        rs2_out = [dram.tile([512, 512], BF, name=f"rs2o{i}")
                   for i in range(2)]

        def cc(kind, op, ins, outs):
            if no_cc:
                # debug fallback: local copies standing in for the exchange
                n_in, n_out = ins[0].shape[0], outs[0].shape[0]
                if kind == "AllGather":
                    for i in range(n_out // n_in):
                        nc.sync.dma_start(
                            outs[0].tensor[i * n_in:(i + 1) * n_in, :],
                            ins[0].tensor[:, :])
                else:
                    nc.sync.dma_start(outs[0].tensor[0:n_out, :],
                                      ins[0].tensor[0:n_out, :])
            else:
                nc.gpsimd.collective_compute(
                    kind, op, replica_groups=GROUPS, ins=ins, outs=outs)

        def mlp_section(rep, with_attn=True):
            with ExitStack() as lctx:
                abig = lctx.enter_context(
                    tc.tile_pool(name=f"abig{rep}", bufs=1))
                wbig = lctx.enter_context(
                    tc.tile_pool(name=f"wbig{rep}", bufs=2))
                mwork = lctx.enter_context(
                    tc.tile_pool(name=f"mwork{rep}", bufs=2))
                mst = lctx.enter_context(
                    tc.tile_pool(name=f"mst{rep}", bufs=1))

                mT = abig.tile([P, CA, NQ], BF, tag="mT", name="mT")
                hTr = D["hTd"].rearrange("(a p) n -> p a n", p=P)

                def wchunk(src, j):
                    wt = wbig.tile([P, CA, 512], BF, tag="w", name="wt")
                    nc.sync.dma_start(
                        wt[:], src[:, j * 512:(j + 1) * 512]
                        .rearrange("(a p) m -> p a m", p=P))
                    return wt

                for ci in range(2):
                    lo_c, hi_c = ci * 512, (ci + 1) * 512
                    # --- assemble h2 (in place into osum) for this half ---
                    osum = mwork.tile([P, CA, 512], BF, tag="osum",
                                      name="osum")
                    if with_attn:
                        # Activation-queue DMA: this read waits on the
                        # AllReduce; on the SP queue it would head-of-line
                        # block the MLP weight stream.
                        nc.scalar.dma_start(
                            osum[:],
                            ar1_out[ci].rearrange("(a p) n -> p a n", p=P))
                        hTh = mwork.tile([P, CA, 512], BF, tag="hTh",
                                         name="hTh", bufs=1)
                        nc.sync.dma_start(hTh[:], hTr[:, :, lo_c:hi_c])
                        for a in range(CA):
                            nc.vector.tensor_add(osum[:, a, :],
                                                 osum[:, a, :], hTh[:, a, :])
                    else:
                        nc.sync.dma_start(osum[:], hTr[:, :, lo_c:hi_c])
                    # --- rmsnorm stats ---
                    var_ps = psd.tile([1, 512], F32, tag="d", name="var")
                    for a in range(CA):
                        sq = mwork.tile([P, 512], F32, tag="sq",
                                        name="sq", bufs=3)
                        nc.scalar.activation(sq[:], osum[:, a, :], AF.Square)
                        nc.tensor.matmul(var_ps[:], onesf[:, 0:1], sq[:],
                                         start=(a == 0), stop=(a == CA - 1))
                    sd2 = mst.tile([1, 512], F32, tag="sd2", name="sd2",
                                   bufs=2)
                    s2b = mst.tile([1, 512], BF, tag="s2b", name="s2b",
                                   bufs=2)
                    nc.vector.tensor_scalar(
                        out=sd2[:], in0=var_ps[:], scalar1=1.0 / C,
                        scalar2=1e-5, op0=OP.mult, op1=OP.add)
                    nc.scalar.activation(sd2[:], sd2[:], AF.Sqrt)
                    nc.vector.reciprocal(sd2[:], sd2[:])
                    nc.vector.tensor_copy(s2b[:], sd2[:])
                    s2bb = mst.tile([P, 512], BF, tag="s2bb", name="s2bb",
                                    bufs=2)
                    sb_ps = psa()
                    nc.tensor.matmul(sb_ps[:], onesb[0:1, :], s2b[0:1, :],
                                     start=True, stop=True)
                    nc.scalar.copy(s2bb[:], sb_ps[:])
                    for a in range(CA):
                        nc.vector.tensor_mul(mT[:, a, lo_c:hi_c],
                                             osum[:, a, :], s2bb[:])
                    if dbg:
                        nc.sync.dma_start(
                            dbg_t["d_h2"][:, lo_c:hi_c]
                            .rearrange("(a p) n -> p a n", p=P), osum[:])
                        if ci == 1:
                            nc.sync.dma_start(
                                dbg_t["d_mT"].rearrange("(a p) n -> p a n",
                                                        p=P), mT[:])

                    # --- MLP for this half ---
                    gact = mwork.tile([P, CA, 512], BF, tag="gact",
                                      name="gact")
                    for j in range(4):
                        wg = wchunk(D["wgate"], j)
                        for mfl in range(4):
                            mf = j * 4 + mfl
                            pp = psa()
                            for a in range(CA):
                                nc.tensor.matmul(
                                    pp[:], wg[:, a, mfl * P:(mfl + 1) * P],
                                    mT[:, a, lo_c:hi_c],
                                    start=(a == 0), stop=(a == CA - 1))
                            sgm = mwork.tile([P, 512], BF, tag="sgm",
                                             name="sgm", bufs=3)
                            nc.scalar.activation(sgm[:], pp[:], AF.Sigmoid)
                            nc.vector.tensor_mul(gact[:, mf, :], pp[:],
                                                 sgm[:])
                    for j in range(4):
                        wu = wchunk(D["wup"], j)
                        for mfl in range(4):
                            mf = j * 4 + mfl
                            pp = psa()
                            for a in range(CA):
                                nc.tensor.matmul(
                                    pp[:], wu[:, a, mfl * P:(mfl + 1) * P],
                                    mT[:, a, lo_c:hi_c],
                                    start=(a == 0), stop=(a == CA - 1))
                            nc.vector.tensor_mul(gact[:, mf, :], pp[:],
                                                 gact[:, mf, :])
                    for j in range(4):
                        wd = wchunk(D["wdown"], j)
                        for mcl in range(4):
                            pp = psa()
                            for a in range(CA):
                                nc.tensor.matmul(
                                    pp[:], wd[:, a, mcl * P:(mcl + 1) * P],
                                    gact[:, a, :],
                                    start=(a == 0), stop=(a == CA - 1))
                            dev = mwork.tile([P, 512], BF, tag="dev",
                                             name="dev", bufs=3)
                            nc.scalar.copy(dev[:], pp[:])
                            nc.scalar.dma_start(
                                ar2_in[ci][(j * 4 + mcl) * P:
                                           (j * 4 + mcl + 1) * P, :],
                                dev[:])
                    cc("ReduceScatter", OP.add, [ar2_in[ci][:]],
                       [rs2_out[ci][:]])

                # --- final assembly: outT = hTq + o_q + mlp_q ---
                hqr = D["hTq"].rearrange("(a p) n -> p a n", p=P)
                for ci in range(2):
                    lo_c, hi_c = ci * 512, (ci + 1) * 512
                    hq = mst.tile([P, 4, 512], BF, tag="hq", name="hq",
                                  bufs=2)
                    nc.sync.dma_start(hq[:], hqr[:, :, lo_c:hi_c])
                    if with_attn:
                        r1 = mst.tile([P, 4, 512], BF, tag="r1", name="r1",
                                      bufs=2)
                        for a in range(4):
                            nc.gpsimd.indirect_dma_start(
                                out=r1[:, a, :], out_offset=None,
                                in_=ar1_out[ci][:],
                                in_offset=bass.IndirectOffsetOnAxis(
                                    ap=qidx_sb[:, a:a + 1], axis=0))
                        nc.vector.tensor_add(hq[:], hq[:], r1[:])
                    r2 = mst.tile([P, 4, 512], BF, tag="r2", name="r2",
                                  bufs=2)
                    nc.sync.dma_start(
                        r2[:], rs2_out[ci].rearrange("(a p) n -> p a n", p=P))
                    nc.vector.tensor_add(hq[:], hq[:], r2[:])
                    nc.sync.dma_start(
                        outT[:, lo_c:hi_c].rearrange("(a p) n -> p a n", p=P),
                        hq[:])


        def layer(rep):
            if phase == "mlp":
                mlp_section(rep, with_attn=False)
                return
            actx = ExitStack()
            att = actx.enter_context(tc.tile_pool(name=f"att{rep}", bufs=1))

            # q/k/v/oT (read until the end of attention) sit at the
            # base of the pool; hTn (dead after the projections) goes
            # above them, so the MLP weight pool reuses hTn's region
            # and its prefetch DMAs don't wait for attention to finish.
            q_sb = att.tile([P, 4, NQ], BF, tag="q")
            k_sb = att.tile([P, 4, NQ], BF, tag="k")
            v_sb = att.tile([P, 8, 512], BF, tag="v")
            oT_sb = att.tile([P, 4, NQ], BF, tag="oT")
            hTn_sb = att.tile([P, CA, NQ], BF, tag="hTn")
            hTn_r = D["hTn"].rearrange("(a p) n -> p a n", p=P)
            for ch in range(4):
                nc.sync.dma_start(
                    hTn_sb[:, ch * 4:(ch + 1) * 4, :],
                    hTn_r[:, ch * 4:(ch + 1) * 4, :])

            # =========================================================
            # offset network + q/k/v projections, interleaved issue so
            # the DVE/Act-heavy offset net hides under qkv matmuls and
            # the sampT AllGather overlaps the tail of the projections.
            # pre/wk2 sit at the top of the SBUF stack and are released
            # before the hd-projection tiles (khdp) allocate, so the
            # causal-attention pool (hw) below never waits on them.
            # =========================================================
            wpr = actx.enter_context(tc.tile_pool(name=f"wpra{rep}", bufs=2))
            rtp = actx.enter_context(tc.tile_pool(name=f"rtpa{rep}", bufs=3))
            hw = actx.enter_context(tc.tile_pool(name=f"hw{rep}", bufs=1))
            pctx = ExitStack()
            pre = pctx.enter_context(tc.tile_pool(name=f"pre{rep}", bufs=1))
            wk2 = pctx.enter_context(tc.tile_pool(name=f"wk2{rep}", bufs=1))

            # ---- offset stage 1: small DMAs + padded lr input ----
            grid_sb = pre.tile([P, 5, 2], F32, tag="grid")
            nc.sync.dma_start(
                grid_sb[:], D["grid"].rearrange("(s p) c -> p s c", p=P))
            convw_sb = pre.tile([P, 4, 9], F32, tag="convw")
            nc.sync.dma_start(
                convw_sb[:], D["convw"].rearrange("(a p) k -> p a k", p=P))
            wlr_sb = pre.tile([P, 4, 256], BF, tag="wlr")
            nc.sync.dma_start(
                wlr_sb[:], D["wlr"].rearrange("(a p) m -> p a m", p=P))
            woff_sb = pre.tile([P, 4, 2], BF, tag="woff")
            nc.sync.dma_start(
                woff_sb[:], D["woff"].rearrange("(a p) m -> p a m", p=P))
            lrin_sb = pre.tile([P, 4, NIMG], BF, tag="lrin")
            nc.sync.dma_start(
                lrin_sb[:], D["lrin"].rearrange("(a p) n -> p a n", p=P))
            xpad = pre.tile([P, 4, 26 * 26], BF, tag="xpad")
            nc.vector.memset(xpad[:], 0.0)
            acc_sb = pre.tile([P, 4, NIMG], BF, tag="acc")
            for a in range(4):
                x3 = xpad[:, a, :].rearrange("p (y x) -> p y x", y=26)
                nc.vector.tensor_copy(
                    x3[:, 1:25, 1:25],
                    lrin_sb[:, a, :].rearrange("p (y x) -> p y x", y=24))

            def conv_group(a):
                # TensorScalarPtr is DVE-only (Pool rejects it in codegen)
                eng = nc.vector
                x3 = xpad[:, a, :].rearrange("p (y x) -> p y x", y=26)
                a3 = acc_sb[:, a, :].rearrange("p (y x) -> p y x", y=24)
                for ky in range(3):
                    for kx in range(3):
                        w_ap = convw_sb[:, a, ky * 3 + kx:ky * 3 + kx + 1]
                        win = x3[:, ky:ky + 24, kx:kx + 24]
                        if ky == 0 and kx == 0:
                            eng.tensor_scalar(
                                out=a3, in0=win, scalar1=w_ap,
                                scalar2=None, op0=OP.mult)
                        else:
                            eng.scalar_tensor_tensor(
                                out=a3, in0=win, scalar=w_ap, in1=a3,
                                op0=OP.mult, op1=OP.add)

            def pnorm_stats(src_sb, na, eps):
                """mean/var over na*128 partitions (PE ones-matmul sums)"""
                red = wk2.tile([1, NIMG], F32, tag="st", bufs=4, name="red")
                red2 = wk2.tile([1, NIMG], F32, tag="st", bufs=4, name="red2")
                sqs = [wk2.tile([P, NIMG], F32, tag="sq1", bufs=1,
                                name="sq1") for _ in range(1)]
                ones_l = onesf if src_sb.dtype == F32 else onesb
                for lo, hi in ((0, 512), (512, NIMG)):
                    rp = psd.tile([1, 512], F32, tag="d", name="rp")
                    for a in range(na):
                        nc.tensor.matmul(rp[:, :hi - lo], ones_l[:, 0:1],
                                         src_sb[:, a, lo:hi],
                                         start=(a == 0), stop=(a == na - 1))
                    nc.scalar.copy(red[0:1, lo:hi], rp[:, :hi - lo])
                rp2 = psd.tile([1, 512], F32, tag="d", name="rp2")
                rp3 = psd.tile([1, 512], F32, tag="d", name="rp3")
                for a in range(na):
                    sq = sqs[0]
                    nc.scalar.activation(sq[:], src_sb[:, a, :], AF.Square)
                    nc.tensor.matmul(rp2[:], onesf[:, 0:1], sq[:, 0:512],
                                     start=(a == 0), stop=(a == na - 1))
                    nc.tensor.matmul(rp3[:, :NIMG - 512], onesf[:, 0:1],
                                     sq[:, 512:NIMG],
                                     start=(a == 0), stop=(a == na - 1))
                nc.scalar.copy(red2[0:1, 0:512], rp2[:])
                nc.scalar.copy(red2[0:1, 512:NIMG], rp3[:, :NIMG - 512])
                nch = float(na * P)
                mu = wk2.tile([1, NIMG], F32, tag="st", bufs=4, name="mu")
                nc.scalar.mul(mu[:], red[:], 1.0 / nch)
                var = wk2.tile([1, NIMG], F32, tag="st", bufs=4, name="var")
                nc.vector.tensor_mul(var[:], mu[:], mu[:])
                nc.vector.scalar_tensor_tensor(
                    out=var[:], in0=red2[:], scalar=1.0 / nch,
                    in1=var[:], op0=OP.mult, op1=OP.subtract)
                nc.vector.tensor_scalar(out=var[:], in0=var[:],
                                        scalar1=eps, scalar2=None, op0=OP.add)
                nc.scalar.activation(var[:], var[:], AF.Sqrt)
                inv = wk2.tile([1, NIMG], F32, tag="inv", name="inv")
                nc.vector.reciprocal(inv[:], var[:])
                aoff = wk2.tile([1, NIMG], F32, tag="aoff", name="aoff")
                nc.vector.scalar_tensor_tensor(
                    out=aoff[:], in0=mu[:], scalar=-1.0, in1=inv[:],
                    op0=OP.mult, op1=OP.mult)
                return inv, aoff

            def pnorm_bcast(inv, aoff):
                invb = wk2.tile([1, NIMG], BF, tag="invb", name="invb")
                aofb = wk2.tile([1, NIMG], BF, tag="aofb", name="aofb")
                nc.scalar.copy(invb[:], inv[:])
                nc.scalar.copy(aofb[:], aoff[:])
                ib = wk2.tile([P, NIMG], BF, tag="ibb", name="ibb")
                ab = wk2.tile([P, NIMG], BF, tag="abb", name="abb")
                for lo, hi in ((0, 512), (512, NIMG)):
                    pi = psd.tile([P, 512], F32, tag="d", name="pi")
                    nc.tensor.matmul(pi[:, :hi - lo], onesb[0:1, :],
                                     invb[0:1, lo:hi], start=True, stop=True)
                    nc.scalar.copy(ib[:, lo:hi], pi[:, :hi - lo])
                    pa = psd.tile([P, 512], F32, tag="d", name="pa")
                    nc.tensor.matmul(pa[:, :hi - lo], onesb[0:1, :],
                                     aofb[0:1, lo:hi], start=True, stop=True)
                    nc.scalar.copy(ab[:, lo:hi], pa[:, :hi - lo])
                return ib, ab

            # ---- qkv projection helpers ----
            def rope_evict(dst, raw_sb, pos_lo, pos_hi):
                n = pos_hi - pos_lo
                rp = psa()
                nc.tensor.matmul(rp[:, :n], swapm[:], raw_sb[:, :n],
                                 start=True, stop=True)
                tmp1 = rtp.tile([P, 512], BF, tag="rt1", name="rt1", bufs=2)
                nc.vector.tensor_mul(tmp1[:, :n], raw_sb[:, :n],
                                     cost[:, pos_lo:pos_hi])
                tmp2 = rtp.tile([P, 512], BF, tag="rt2", name="rt2", bufs=2)
                nc.vector.tensor_mul(tmp2[:, :n], rp[:, :n],
                                     sint[:, pos_lo:pos_hi])
                nc.vector.tensor_add(dst[:, pos_lo:pos_hi], tmp1[:, :n],
                                     tmp2[:, :n])

            def qk_load(wname):
                wt = wpr.tile([P, CA, 512], BF, tag="wpr", name="wt")
                nc.sync.dma_start(
                    wt[:], D[wname].rearrange("(a p) m -> p a m", p=P))
                return wt

            def qk_heads(wt, dst, src_sb, heads, n_src):
                for h in heads:
                    for lo, hi in ((0, 512), (512, n_src)):
                        pp = psa()
                        for a in range(CA):
                            nc.tensor.matmul(pp[:, :hi - lo],
                                             wt[:, a, h * P:(h + 1) * P],
                                             src_sb[:, a, lo:hi],
                                             start=(a == 0),
                                             stop=(a == CA - 1))
                        raw = rtp.tile([P, 512], BF, tag="raw", name="raw")
                        nc.scalar.copy(raw[:, :hi - lo], pp[:, :hi - lo])
                        rope_evict(dst[:, h, :], raw, lo, hi)

            # ---- interleaved issue ----
            wt_q = qk_load("wq")
            conv_group(0)
            conv_group(2)
            qk_heads(wt_q, q_sb, hTn_sb, (0, 1), NQ)
            conv_group(1)
            conv_group(3)
            qk_heads(wt_q, q_sb, hTn_sb, (2, 3), NQ)

            inv1, aoff1 = pnorm_stats(acc_sb, 4, 1e-6)
            ib1, ab1 = pnorm_bcast(inv1, aoff1)
            xg_sb = pre.tile([P, 4, NIMG], BF, tag="xg")
            sgt = wk2.tile([P, NIMG], BF, tag="sgt", name="sgt")
            xh = wk2.tile([P, NIMG], F32, tag="xh", name="xh")
            for a in range(4):
                nc.vector.tensor_mul(xh[:], acc_sb[:, a, :], ib1[:])
                nc.vector.tensor_add(xh[:], xh[:], ab1[:])
                nc.scalar.activation(sgt[:], xh[:], AF.Sigmoid, scale=1.702)
                nc.vector.tensor_mul(xg_sb[:, a, :], xh[:], sgt[:])

            wt_k = qk_load("wk")
            qk_heads(wt_k, k_sb, hTn_sb, (0, 1), NQ)
            if dbg:
                nc.sync.dma_start(
                    dbg_t["d_q"].rearrange("(h p) n -> p h n", p=P), q_sb[:])

            # intent vector
            hmean = wk2.tile([P, CA], F32, tag="hmean", name="hmean")
            hmb = wk2.tile([P, CA], BF, tag="hmb", name="hmb")
            for a in range(CA):
                nc.vector.tensor_reduce(
                    hmean[:, a:a + 1], hTn_sb[:, a, :],
                    axis=mybir.AxisListType.X, op=OP.add)
            nc.vector.tensor_copy(hmb[:], hmean[:])
            intent = wk2.tile([P, 2], F32, tag="intent", name="intent")
            for m in range(4):
                wint_sb = wk2.tile([P, CA, 64], BF, tag="wint",
                                   name="wint_sb", bufs=1)
                nc.sync.dma_start(
                    wint_sb[:],
                    D["wint"][:, m * 64:(m + 1) * 64]
                    .rearrange("(a p) m -> p a m", p=P))
                ip = psm.tile([P, P], F32, tag="t", name="ip")
                prow = slice((m % 2) * 64, (m % 2) * 64 + 64)
                for a in range(CA):
                    nc.tensor.matmul(ip[prow, 0:1], wint_sb[:, a, :],
                                     hmb[:, a:a + 1], start=(a == 0),
                                     stop=(a == CA - 1))
                nc.scalar.mul(intent[prow, m // 2:m // 2 + 1],
                              ip[prow, 0:1], 1.0 / NQ)

            # cat = [xproj ; intent] -> ln2 (in place) -> off
            cat_sb = pre.tile([P, 4, NIMG], BF, tag="cat")
            for m in range(2):
                for lo, hi in ((0, 512), (512, NIMG)):
                    xp = psd.tile([P, 512], F32, tag="d", name="xp")
                    for a in range(4):
                        nc.tensor.matmul(xp[:, :hi - lo],
                                         wlr_sb[:, a, m * P:(m + 1) * P],
                                         xg_sb[:, a, lo:hi],
                                         start=(a == 0), stop=(a == 3))
                    nc.scalar.copy(cat_sb[:, m, lo:hi], xp[:, :hi - lo])
            for m in range(2):
                nc.vector.tensor_scalar(
                    out=cat_sb[:, 2 + m, :], in0=xg_sb[:, 0, :],
                    scalar1=0.0, scalar2=intent[:, m:m + 1], op0=OP.mult,
                    op1=OP.add)

            qk_heads(wt_k, k_sb, hTn_sb, (2, 3), NQ)
            if dbg:
                nc.sync.dma_start(
                    dbg_t["d_k"].rearrange("(h p) n -> p h n", p=P), k_sb[:])

            inv2, aoff2 = pnorm_stats(cat_sb, 4, 1e-6)
            ib2, ab2 = pnorm_bcast(inv2, aoff2)
            for a in range(4):
                nc.vector.tensor_mul(xh[:], cat_sb[:, a, :], ib2[:])
                nc.vector.tensor_add(cat_sb[:, a, :], xh[:], ab2[:])

            off_sb = wk2.tile([2, NPAD], F32, tag="off", name="off")
            nc.vector.memset(off_sb[:], 0.0)
            for lo, hi in ((0, 512), (512, NIMG)):
                op_ = psd.tile([2, 512], F32, tag="d", name="opp")
                for a in range(4):
                    nc.tensor.matmul(op_[:, :hi - lo], woff_sb[:, a, :],
                                     cat_sb[:, a, lo:hi], start=(a == 0),
                                     stop=(a == 3))
                nc.scalar.copy(off_sb[:, lo:hi], op_[:, :hi - lo])

            # bilinear coordinates, batched across all 5 s-tiles
            idx0 = wk2.tile([P, 5], I32, tag="idx0", name="idx0")
            idx1 = wk2.tile([P, 5], I32, tag="idx1", name="idx1")
            wcmb = wk2.tile([P, 5, 4], F32, tag="wcmb", name="wcmb")
            t2 = wk2.tile([P, 5, 2], F32, tag="t2", name="t2")
            fr = wk2.tile([P, 5, 2], F32, tag="fr", name="fr")
            f0 = wk2.tile([P, 5, 2], F32, tag="f0", name="f0")
            f1 = wk2.tile([P, 5, 2], F32, tag="f1", name="f1")
            w1m = wk2.tile([P, 5, 2], F32, tag="w1m", name="w1m")
            fi = wk2.tile([P, 5, 1], F32, tag="fi", name="fi")
            tps_c = psm.tile([P, 5, 2], F32, tag="t", name="tps_c")
            for st in range(5):
                nc.tensor.transpose(tps_c[:, st, :],
                                    off_sb[0:2, st * P:(st + 1) * P],
                                    idf[0:2, 0:2])
            nc.scalar.activation(t2[:], tps_c[:], AF.Tanh)
            nc.vector.scalar_tensor_tensor(
                out=t2[:], in0=t2[:], scalar=2.0 / LR,
                in1=grid_sb[:], op0=OP.mult, op1=OP.add)
            nc.vector.tensor_scalar(out=t2[:], in0=t2[:], scalar1=1.0,
                                    scalar2=-1.0, op0=OP.min, op1=OP.max)
            nc.vector.tensor_scalar(out=t2[:], in0=t2[:], scalar1=1.0,
                                    scalar2=(HR - 1) / 2.0,
                                    op0=OP.add, op1=OP.mult)
            ti = wk2.tile([P, 5, 2], I32, tag="ti", name="ti")
            nc.vector.tensor_copy(ti[:], t2[:])
            nc.vector.tensor_copy(f0[:], ti[:])
            nc.vector.tensor_tensor(out=fr[:], in0=f0[:], in1=t2[:],
                                    op=OP.is_gt)
            nc.vector.tensor_sub(f0[:], f0[:], fr[:])
            nc.vector.tensor_sub(fr[:], t2[:], f0[:])
            nc.vector.tensor_scalar(out=f1[:], in0=f0[:], scalar1=1.0,
                                    scalar2=float(HR - 1), op0=OP.add,
                                    op1=OP.min)
            nc.vector.scalar_tensor_tensor(
                out=fi[:], in0=f0[:, :, 0:1], scalar=float(HR),
                in1=f0[:, :, 1:2], op0=OP.mult, op1=OP.add)
            nc.vector.tensor_copy(idx0[:], fi[:, :, 0])
            nc.vector.scalar_tensor_tensor(
                out=fi[:], in0=f1[:, :, 0:1], scalar=float(HR),
                in1=f0[:, :, 1:2], op0=OP.mult, op1=OP.add)
            nc.vector.tensor_copy(idx1[:], fi[:, :, 0])
            nc.vector.tensor_scalar(out=w1m[:], in0=fr[:],
                                    scalar1=-1.0, scalar2=1.0,
                                    op0=OP.mult, op1=OP.add)
            nc.vector.tensor_mul(wcmb[:, :, 0:1], w1m[:, :, 0:1],
                                 w1m[:, :, 1:2])
            nc.vector.tensor_mul(wcmb[:, :, 1:2], w1m[:, :, 0:1],
                                 fr[:, :, 1:2])
            nc.vector.tensor_mul(wcmb[:, :, 2:3], fr[:, :, 0:1],
                                 w1m[:, :, 1:2])
            nc.vector.tensor_mul(wcmb[:, :, 3:4], fr[:, :, 0:1],
                                 fr[:, :, 1:2])

            wt_v = qk_load("wv")
            for m8 in range(4):
                pp = psa()
                for a in range(CA):
                    nc.tensor.matmul(pp[:],
                                     hTn_sb[:, a, m8 * P:(m8 + 1) * P],
                                     wt_v[:, a, :], start=(a == 0),
                                     stop=(a == CA - 1))
                nc.scalar.copy(v_sb[:, m8, :], pp[:])

            # gather + combine + transpose
            sampT_mine = pre.tile([P, 4, NPAD], BF, tag="sampT_mine")
            for st in range(5):
                p0 = wk2.tile([P, 1024], BF, tag="p0", bufs=1, name="p0")
                p1 = wk2.tile([P, 1024], BF, tag="p1", bufs=1, name="p1")
                nc.gpsimd.indirect_dma_start(
                    out=p0[:], out_offset=None, in_=D["imgp"][:],
                    in_offset=bass.IndirectOffsetOnAxis(
                        ap=idx0[:, st:st + 1], axis=0))
                nc.gpsimd.indirect_dma_start(
                    out=p1[:], out_offset=None, in_=D["imgp"][:],
                    in_offset=bass.IndirectOffsetOnAxis(
                        ap=idx1[:, st:st + 1], axis=0))
                smp = wk2.tile([P, 512], BF, tag="smp", bufs=2, name="smp")
                nc.vector.tensor_tensor(
                    out=smp[:], in0=p0[:, 0:512],
                    in1=wcmb[:, st, 0:1].to_broadcast([P, 512]), op=OP.mult)
                for pair, col in ((p0, 1), (p1, 2), (p1, 3)):
                    src = pair[:, 0:512] if col == 2 else pair[:, 512:1024]
                    nc.vector.scalar_tensor_tensor(
                        out=smp[:], in0=src,
                        scalar=wcmb[:, st, col:col + 1], in1=smp[:],
                        op0=OP.mult, op1=OP.add)
                for cm in range(4):
                    tp = psm.tile([P, P], BF, tag="t", name="tps")
                    nc.tensor.transpose(tp[:], smp[:, cm * P:(cm + 1) * P],
                                        idb[:])
                    nc.scalar.copy(
                        sampT_mine[:, cm, st * P:(st + 1) * P], tp[:])
            nc.sync.dma_start(ag_in.rearrange("(a p) n -> p a n", p=P),
                              sampT_mine[:, :, 0:NIMG])
            cc("AllGather", OP.bypass, [ag_in[:]], [ag_out[:]])

            # remaining v tiles while AllGather flies
            for m8 in range(4, 8):
                pp = psa()
                for a in range(CA):
                    nc.tensor.matmul(pp[:],
                                     hTn_sb[:, a, m8 * P:(m8 + 1) * P],
                                     wt_v[:, a, :], start=(a == 0),
                                     stop=(a == CA - 1))
                nc.scalar.copy(v_sb[:, m8, :], pp[:])

            # =========================================================
            # attention: causal tiles first (they only need q/k/v, so
            # they fill the AllGather window), head-pairs interleaved
            # for PE pipelining; image tiles + softmax finalize after
            # the hd projections land. Denominators accumulate on the
            # otherwise-idle Pool engine.
            # =========================================================
            daccs, oAs = {}, {}
            sampT_sb = khd_sb = vhd_sb = None

            def sc_exp_o(ci, pair, kind, kt, o_pss, start, stop,
                         dinit=False):
                lo_c, hi_c = ci * 512, (ci + 1) * 512
                if kind == "c":
                    qlo, kp = kt * P, P
                else:
                    qlo = 0
                    kp = P if kt < 4 else NIMG - 4 * P
                lo = max(qlo, lo_c)
                n = hi_c - lo
                o = lo - lo_c
                for h in pair:
                    dacc = daccs[(ci, h)]
                    sp = psa()
                    if kind == "c":
                        nc.tensor.matmul(sp[:, :n],
                                         k_sb[:, h, kt * P:(kt + 1) * P],
                                         q_sb[:, h, lo:hi_c],
                                         start=True, stop=True)
                        if lo == qlo:
                            nc.vector.tensor_add(sp[:, 0:P], sp[:, 0:P],
                                                 maskd[:])
                        lhs = v_sb[:, kt, h * P:(h + 1) * P]
                    else:
                        nc.tensor.matmul(sp[:kp, :n],
                                         khd_sb[:, h, kt * P:kt * P + kp],
                                         q_sb[:, h, lo:hi_c],
                                         start=True, stop=True)
                        lhs = vhd_sb[:kp, kt, h * P:(h + 1) * P]
                    ex = hw.tile([P, 512], BF, tag="ex", name="ex", bufs=3)
                    nc.scalar.activation(ex[:kp, o:], sp[:kp, :n], AF.Exp,
                                         scale=SCALE)
                    nc.tensor.matmul(o_pss[h][:, o:], lhs, ex[:kp, o:],
                                     start=start, stop=stop)
                    deng = nc.gpsimd if DACC_POOL else nc.vector
                    if dinit:
                        deng.tensor_copy(dacc[:kp, :], ex[:kp, :])
                    else:
                        deng.tensor_add(dacc[:kp, o:], dacc[:kp, o:],
                                        ex[:kp, o:])

            def causal_pass(ci, pair):
                ncaus = 4 * (ci + 1)
                o_pss = {h: psa() for h in pair}
                for h in pair:
                    daccs[(ci, h)] = hw.tile([P, 512], F32, tag="dacc",
                                             name=f"dc{ci}{h}", bufs=8)
                for kt in range(ncaus):
                    sc_exp_o(ci, pair, "c", kt, o_pss,
                             start=(kt == 0), stop=(kt == ncaus - 1),
                             dinit=(kt == 0))
                for h in pair:
                    oA = hw.tile([P, 512], BF, tag="oA",
                                 name=f"oA{ci}{h}", bufs=8)
                    nc.scalar.copy(oA[:], o_pss[h][:])
                    oAs[(ci, h)] = oA

            def img_pass(ci, pair):
                lo_c, hi_c = ci * 512, (ci + 1) * 512
                o_pss = {h: psa() for h in pair}
                for it in range(5):
                    sc_exp_o(ci, pair, "i", it, o_pss,
                             start=(it == 0), stop=(it == 4))
                for h in pair:
                    den = psd.tile([1, 512], F32, tag="d", name="den")
                    nc.tensor.matmul(den[:], onesf[:, 0:1],
                                     daccs[(ci, h)][:], start=True, stop=True)
                    rcf = hw.tile([1, 512], F32, tag="rcf", name="rcf",
                                  bufs=1)
                    rcb = hw.tile([1, 512], BF, tag="rcb", name="rcb",
                                  bufs=2)
                    nc.vector.reciprocal(rcf[:], den[:])
                    nc.vector.tensor_copy(rcb[:], rcf[:])
                    rb = psa()
                    nc.tensor.matmul(rb[:], onesb[0:1, :], rcb[0:1, :],
                                     start=True, stop=True)
                    rbs = hw.tile([P, 512], BF, tag="rbs", name="rbs",
                                  bufs=1)
                    nc.scalar.copy(rbs[:], rb[:])
                    otmp = hw.tile([P, 512], BF, tag="otmp", name="otmp",
                                   bufs=1)
                    nc.vector.tensor_add(otmp[:], oAs[(ci, h)][:],
                                         o_pss[h][:])
                    nc.vector.tensor_mul(oT_sb[:, h, lo_c:hi_c], otmp[:],
                                         rbs[:])

            # causal part of attention (during the AllGather flight)
            for ci in range(2):
                for pair in ((0, 1), (2, 3)):
                    causal_pass(ci, pair)

            pctx.close()

            # hd-tile pool reuses the released pre/wk2 region; its
            # writes only depend on the AllGather anyway.
            khdp = actx.enter_context(tc.tile_pool(name=f"khdp{rep}",
                                                   bufs=1))
            sampT_sb = khdp.tile([P, CA, NIMG], BF, tag="sampT")
            khd_sb = khdp.tile([P, 4, NIMG], BF, tag="khd")
            vhd_sb = khdp.tile([P, 5, 512], BF, tag="vhd")
            wo_sb = khdp.tile([P, 4, C], BF, tag="wo")
            wt_khd = qk_load("wkhd")
            wt_vhd = qk_load("wvhd")
            nc.sync.dma_start(
                wo_sb[:], D["wo"].rearrange("(a p) m -> p a m", p=P))
            # Activation-queue DMA: waits on the AllGather; on SP it would
            # block the o-proj eviction stream and MLP weight prefetch.
            nc.scalar.dma_start(
                sampT_sb[:], ag_out.rearrange("(a p) n -> p a n", p=P))
            if dbg:
                nc.sync.dma_start(
                    dbg_t["d_samp"].rearrange("(a p) n -> p a n", p=P),
                    sampT_sb[:])

            # ---- hd-token projections (need the AllGather result) ----
            for h in range(4):
                for lo, hi in ((0, 512), (512, NIMG)):
                    pp = psa()
                    for a in range(CA):
                        nc.tensor.matmul(pp[:, :hi - lo],
                                         wt_khd[:, a, h * P:(h + 1) * P],
                                         sampT_sb[:, a, lo:hi],
                                         start=(a == 0), stop=(a == CA - 1))
                    raw = rtp.tile([P, 512], BF, tag="raw", name="raw")
                    nc.scalar.copy(raw[:, :hi - lo], pp[:, :hi - lo])
                    rope_evict(khd_sb[:, h, :], raw, lo, hi)
            if dbg:
                nc.sync.dma_start(
                    dbg_t["d_khd"].rearrange("(h p) n -> p h n", p=P),
                    khd_sb[:])

            for st in range(5):
                kp = P if st < 4 else NIMG - 4 * P
                pp = psa()
                for a in range(CA):
                    nc.tensor.matmul(pp[:kp, :],
                                     sampT_sb[:, a, st * P:st * P + kp],
                                     wt_vhd[:, a, :], start=(a == 0),
                                     stop=(a == CA - 1))
                nc.scalar.copy(vhd_sb[:kp, st, :], pp[:kp, :])

            # ---- image attention + o-projection per token half ----
            for ci in range(2):
                lo_c, hi_c = ci * 512, (ci + 1) * 512
                img_pass(ci, (0, 1))
                img_pass(ci, (2, 3))
                if dbg and ci == 1:
                    nc.sync.dma_start(
                        dbg_t["d_oT"].rearrange("(h p) n -> p h n", p=P),
                        oT_sb[:])

                # o-projection for this token half -> ReduceScatter -> AG
                for m in range(CA):
                    pp = psa()
                    for h in range(4):
                        nc.tensor.matmul(pp[:], wo_sb[:, h, m * P:(m + 1) * P],
                                         oT_sb[:, h, lo_c:hi_c],
                                         start=(h == 0), stop=(h == 3))
                    oev = khdp.tile([P, 512], BF, tag="oev", bufs=3,
                                    name="oev")
                    nc.scalar.copy(oev[:], pp[:])
                    # Act-queue DMA: an SP-queue write here would stall SP
                    # on o-proj completion and block MLP weight prefetch.
                    nc.scalar.dma_start(ar1_in[ci][m * P:(m + 1) * P, :],
                                        oev[:])
                cc("AllReduce", OP.add, [ar1_in[ci][:]], [ar1_out[ci][:]])

            actx.close()

            if phase == "attn":
                with ExitStack() as lctx:
                    mstx = lctx.enter_context(
                        tc.tile_pool(name=f"mstx{rep}", bufs=2))
                    hqr = D["hTq"].rearrange("(a p) n -> p a n", p=P)
                    for ci in range(2):
                        lo_c, hi_c = ci * 512, (ci + 1) * 512
                        hq = mstx.tile([P, 4, 512], BF, tag="hq", name="hq")
                        nc.sync.dma_start(hq[:], hqr[:, :, lo_c:hi_c])
                        r1 = mstx.tile([P, 4, 512], BF, tag="r1", name="r1")
                        for a in range(4):
                            nc.gpsimd.indirect_dma_start(
                                out=r1[:, a, :], out_offset=None,
                                in_=ar1_out[ci][:],
                                in_offset=bass.IndirectOffsetOnAxis(
                                    ap=qidx_sb[:, a:a + 1], axis=0))
                        nc.vector.tensor_add(hq[:], hq[:], r1[:])
                        nc.sync.dma_start(
                            outT[:, lo_c:hi_c]
                            .rearrange("(a p) n -> p a n", p=P), hq[:])
                return

            mlp_section(rep, with_attn=True)

        for rep in range(reps):
            layer(rep)

    nc.compile()
    return nc


import time
import jax
from jax.sharding import Mesh, PartitionSpec
from jax.experimental.shard_map import shard_map
from concourse import bass2jax
from concourse.bass2jax import _bass_exec_p, install_neuronx_cc_hook, \
    partition_id_tensor


class TimedRunner:
    def __init__(self, nc, n_cores=8):
        install_neuronx_cc_hook()
        self.nc = nc
        self.n_cores = n_cores
        partition_name = (nc.partition_id_tensor.name
                          if nc.partition_id_tensor else None)
        in_names, out_names, out_avals, zero_outs = [], [], [], []
        for alloc in nc.m.functions[0].allocations:
            if not isinstance(alloc, mybir.MemoryLocationSet):
                continue
            name = alloc.memorylocations[0].name
            if alloc.kind == "ExternalInput":
                if name != partition_name:
                    in_names.append(name)
            elif alloc.kind == "ExternalOutput":
                out_names.append(name)
                shape = tuple(alloc.tensor_shape)
                dtype = mybir.dt.np(alloc.dtype)
                out_avals.append(jax.core.ShapedArray(shape, dtype))
                zero_outs.append(np.zeros(shape, dtype))
        if nc.dbg_addr is not None:
            assert not nc.dbg_callbacks
        self.in_names = list(in_names)
        self.out_names = out_names
        self.out_avals = out_avals
        self.zero_outs = zero_outs
        n_params = len(in_names)
        n_outs = len(out_avals)
        all_in_names = list(in_names) + list(out_names)
        if partition_name is not None:
            all_in_names.append(partition_name)
        self.partition_name = partition_name

        def _body(*args):
            operands = list(args)
            if partition_name is not None:
                operands.append(partition_id_tensor())
            outs = _bass_exec_p.bind(
                *operands,
                out_avals=tuple(out_avals),
                in_names=tuple(all_in_names),
                out_names=tuple(out_names),
                lowering_input_output_aliases=(),
                sim_require_finite=True,
                sim_require_nnan=True,
                nc=nc,
            )
            return tuple(outs)

        devices = jax.devices()[:n_cores]
        mesh = Mesh(np.asarray(devices), ("core",))
        in_specs = (PartitionSpec("core"),) * (n_params + n_outs)
        out_specs = (PartitionSpec("core"),) * n_outs
        # no donation so the function is re-callable with the same buffers
        self.fn = jax.jit(shard_map(_body, mesh=mesh, in_specs=in_specs,
                                    out_specs=out_specs, check_rep=False))
        self.mesh = mesh

    def put_inputs(self, in_maps):
        dbg = {}
        if self.nc.dbg_addr is not None:
            dbg = {self.nc.dbg_addr.name: np.zeros((1, 2), np.uint32)}
        per_core = [[np.asarray({**m, **dbg}[n]) for n in self.in_names]
                    for m in in_maps]
        n_params = len(self.in_names)
        concat_in = [
            np.concatenate([per_core[c][i] for c in range(self.n_cores)],
                           axis=0) for i in range(n_params)]
        concat_zeros = [
            np.zeros((self.n_cores * z.shape[0], *z.shape[1:]), z.dtype)
            for z in self.zero_outs]
        sh = jax.sharding.NamedSharding(self.mesh, PartitionSpec("core"))
        self.dev_args = [jax.device_put(a, sh)
                         for a in (*concat_in, *concat_zeros)]

    def run(self):
        outs = jax.block_until_ready(self.fn(*self.dev_args))
        return outs

    def results(self, outs):
        return [
            {n: np.asarray(outs[i]).reshape(
                self.n_cores, *self.out_avals[i].shape)[c]
             for i, n in enumerate(self.out_names)}
            for c in range(self.n_cores)
        ]

    def bench(self, iters=5):
        self.run()
        best = float("inf")
        for _ in range(iters):
            t0 = time.perf_counter()
            self.run()
            best = min(best, time.perf_counter() - t0)
        return best


# ----------------------------------------------------------------- entry
_NC_CACHE = {}


def _get_nc(reps=1):
    if reps not in _NC_CACHE:
        _NC_CACHE[reps] = build(dbg=False, reps=reps)
    return _NC_CACHE[reps]


def kernel(**inputs) -> np.ndarray:
    """Full inputs -> full [2, 1024, 2048] fp32 output, computed on the
    8 TRN2 NeuronCores (DPxTP sharding, bf16 compute)."""
    from concourse.bass_utils import run_bass_kernel_spmd
    nc = _get_nc(1)
    maps = prep_inputs(inputs)
    res = run_bass_kernel_spmd(nc, maps, list(range(8)))
    return finish(res.results)


def benchmark_device_time(inputs, reps_hi=11, npipe=16, trials=16):
    """Per-layer device execution time: difference an on-device
    reps_hi-iteration NEFF against the single-iteration NEFF under
    pipelined launches (axon host dispatch is ~100ms and would otherwise
    swamp the sub-ms kernel). Samples are interleaved lo/hi/lo so slow
    host-side drift cancels; median over trials for jitter robustness."""
    import time as _time
    import statistics as _stats
    import jax as _jax
    maps = prep_inputs(inputs)
    trs = {}
    for reps in (1, reps_hi):
        tr = TimedRunner(_get_nc(reps), 8)
        tr.put_inputs(maps)
        tr.run()
        trs[reps] = tr

    def sample(tr):
        t0 = _time.perf_counter()
        outs = None
        for _ in range(npipe):
            outs = tr.fn(*tr.dev_args)
        _jax.block_until_ready(outs)
        return (_time.perf_counter() - t0) / npipe

    sample(trs[1])
    sample(trs[reps_hi])
    diffs = []
    for _ in range(trials):
        a = sample(trs[1])
        b = sample(trs[reps_hi])
        a2 = sample(trs[1])
        diffs.append((b - (a + a2) / 2) / (reps_hi - 1))
    return max(_stats.median(diffs), 1e-9)


# revision 42
# speedup vs baseline: 1.0147x; 1.0037x over previous
"""Bass/Tile kernel for nn_LlamaDecoderLayerDAT on 8 TRN2 cores.

Sharding: DP(batch=2) x TP(4) within batch groups [[0..3],[4..7]].
Core c: batch b=c//4, TP slot g=c%4 (heads 4g..4g+3, dff slice g*2048,
offset-net channel group g, output channel shard g*512..(g+1)*512).

All activations on device live in transposed [channel(part), token(free)]
layout, bf16 compute with fp32 PSUM accumulation.

Collective plan (all within the 4-core TP group):
  - sampT: AllGather (issued early, overlapped with q/k/v projections)
  - o-projection: per token half: ReduceScatter (each core gets its
    512-channel quarter of the o-sum) then AllGather back to full C;
    residual h2 = hTd + o_full assembled on the consumer side.
  - MLP down-projection: per token half: ReduceScatter only; each core
    emits outT shard = hTq + o_quarter + mlp_quarter; the host
    reassembles the 4 channel shards per batch.
Attention/o-proj/MLP are issued in token-half phases so no engine queue
ever blocks on a later collective (head-of-line) before earlier-phase
compute has been issued.
"""
import numpy as np
import ml_dtypes
from contextlib import ExitStack

import concourse.bass as bass
import concourse.bacc as bacc
import concourse.tile as tile
from concourse import mybir

BF = mybir.dt.bfloat16
F32 = mybir.dt.float32
F8 = mybir.dt.float8e4
I32 = mybir.dt.int32
AF = mybir.ActivationFunctionType
OP = mybir.AluOpType

P = 128
NQ, C, NH, HD = 1024, 2048, 16, 128
DFF = 8192
LR, HR, NIMG, NPAD = 24, 48, 576, 640
CA = C // P              # 16 K-tiles over channels
SCALE = float(1.0 / np.sqrt(HD))
GROUPS = [[0, 1, 2, 3], [4, 5, 6, 7]]
NEG = -1.0e30
bf16 = ml_dtypes.bfloat16
DACC_POOL = False


def _bf(x):
    return np.asarray(x, np.float32).astype(bf16)


# ----------------------------------------------------------------- host side
def _rope_tables():
    inv = 1.0 / (10000.0 ** (np.arange(0, HD, 2, dtype=np.float32) / HD))
    ang = np.arange(NQ, dtype=np.float32)[:, None] * inv[None, :]
    ang = np.concatenate([ang, ang], axis=-1)                 # [NQ, 128]
    sgn = np.ones((HD,), np.float32)
    sgn[: HD // 2] = -1.0
    return np.cos(ang).T.copy(), (np.sin(ang) * sgn[None, :]).T.copy()


def _grid640():
    ys = (np.linspace(0.5, LR - 0.5, LR, dtype=np.float32) / (LR - 1.0)) * 2 - 1
    gy, gx = np.meshgrid(ys, ys, indexing="ij")
    g = np.zeros((NPAD, 2), np.float32)
    g[:NIMG, 0] = gy.reshape(-1)
    g[:NIMG, 1] = gx.reshape(-1)
    return g


def prep_inputs(inputs):
    """Full problem inputs -> list of 8 per-core in_maps."""
    W = {k: np.asarray(v, np.float32) for k, v in inputs.items()}
    hid = W["hidden_states"]
    img = W["image_hd_features"]
    cosT, sinT = _rope_tables()
    kk = np.arange(P)
    maskd = np.where(kk[:, None] > kk[None, :], np.float32(NEG),
                     np.float32(0.0))
    swap = np.zeros((P, P), np.float32)
    swap[np.arange(P), (np.arange(P) + 64) % P] = 1.0
    shared = dict(
        cost=_bf(cosT), sint=_bf(sinT), grid=_grid640(),
        maskd=maskd, swapm=_bf(swap),
        idb=_bf(np.eye(P)), idf=np.eye(P, dtype=np.float32),
        onesb=_bf(np.ones((P, P))), onesf=np.ones((P, P), np.float32),
        convw=np.ascontiguousarray(W["conv_dw_w"].reshape(512, 9)),
        wlr=_bf(W["Wlrproj"]), wint=_bf(W["Wint"]), woff=_bf(W["Woff"]),
    )
    maps = []
    for c in range(8):
        b, g = c // 4, c % 4
        hT = np.ascontiguousarray(hid[b].T)                   # [C, NQ]
        s = 1.0 / np.sqrt((hid[b] ** 2).mean(-1) + 1e-5)      # [NQ]
        hTn = hT * s[None, :]
        img_g = np.ascontiguousarray(img[b][:, g * 512:(g + 1) * 512])
        flat = img_g.reshape(-1)
        st = flat.strides[0]
        imgp = np.zeros((HR * HR, 1024), np.float32)
        imgp[:HR * HR - 1] = np.lib.stride_tricks.as_strided(
            flat, (HR * HR - 1, 1024), (st * 512, st))
        imgp[HR * HR - 1, :512] = img_g[HR * HR - 1]
        hsl = slice(g * 512, (g + 1) * 512)
        fsl = slice(g * 2048, (g + 1) * 2048)
        m = dict(shared)
        m.update(
            hTn=_bf(hTn), hTd=_bf(hT), hTq=_bf(hT[hsl]),
            qidx=(g * 512 + np.arange(4, dtype=np.int32)[None, :] * 128
                  + np.arange(P, dtype=np.int32)[:, None]).copy(),
            lrin=_bf(hTn[hsl, :NIMG]),
            imgp=_bf(imgp),
            wq=_bf(W["Wq"][:, hsl]), wk=_bf(W["Wk"][:, hsl]),
            wv=_bf(W["Wv"][:, hsl]), wo=_bf(W["Wo"][hsl, :]),
            wkhd=_bf(W["Wk_hd"][:, hsl]), wvhd=_bf(W["Wv_hd"][:, hsl]),
            wgate=_bf(W["Wgate"][:, fsl]), wup=_bf(W["Wup"][:, fsl]),
            wdown=_bf(W["Wdown"][fsl, :]),
        )
        maps.append(m)
    return maps


def finish(results):
    out = np.empty((2, NQ, C), np.float32)
    for b in range(2):
        for g in range(4):
            sh = np.asarray(results[4 * b + g]["outT"]).astype(np.float32)
            out[b, :, g * 512:(g + 1) * 512] = sh.T
    return out


# --------------------------------------------------------------- device side
def build(dbg=False, reps=1, no_cc=False, phase="full"):
    nc = bacc.Bacc("TRN2", num_devices=8)
    D = {}

    def inp(name, shape, dt):
        D[name] = nc.dram_tensor(name, shape, dt, kind="ExternalInput")
        return D[name]

    for n in ("hTn", "hTd"):
        inp(n, [C, NQ], BF)
    inp("hTq", [512, NQ], BF)
    inp("lrin", [512, NIMG], BF)
    inp("imgp", [HR * HR, 1024], BF)
    for n in ("wq", "wk", "wv", "wkhd", "wvhd"):
        inp(n, [C, 512], BF)
    inp("wo", [512, C], BF)
    for n in ("wgate", "wup"):
        inp(n, [C, 2048], BF)
    inp("wdown", [2048, C], BF)
    inp("wlr", [512, 256], BF)
    inp("wint", [C, 256], BF)
    inp("woff", [512, 2], BF)
    inp("convw", [512, 9], F32)
    inp("cost", [P, NQ], BF)
    inp("sint", [P, NQ], BF)
    inp("grid", [NPAD, 2], F32)
    inp("qidx", [P, 4], I32)
    inp("maskd", [P, P], F32)
    for n in ("idb", "onesb", "swapm"):
        inp(n, [P, P], BF)
    for n in ("idf", "onesf"):
        inp(n, [P, P], F32)

    outT = nc.dram_tensor("outT", [512, NQ], BF, kind="ExternalOutput")
    dbg_t = {}
    if dbg:
        for n, shape, dt in (
            ("d_samp", [C, NIMG], BF), ("d_q", [512, NQ], BF),
            ("d_k", [512, NQ], BF), ("d_khd", [512, NIMG], BF),
            ("d_oT", [512, NQ], BF), ("d_h2", [C, NQ], BF),
            ("d_mT", [C, NQ], BF),
        ):
            dbg_t[n] = nc.dram_tensor(n, shape, dt, kind="ExternalOutput")

    with tile.TileContext(nc) as tc, ExitStack() as ctx:
        const = ctx.enter_context(tc.tile_pool(name="const", bufs=1))
        dram = ctx.enter_context(tc.tile_pool(name="dram", bufs=1,
                                              space="DRAM"))
        ps = ctx.enter_context(tc.tile_pool(name="ps", bufs=4, space="PSUM"))
        psd = ctx.enter_context(tc.tile_pool(name="psd", bufs=2, space="PSUM"))
        psm = ctx.enter_context(tc.tile_pool(name="psm", bufs=2, space="PSUM"))

        def psa():
            return ps.tile([P, 512], F32, tag="a", name="psa")

        # ---- persistent consts ----
        cn = {}
        for n, shape, dt in (
            ("idb", [P, P], BF), ("onesb", [P, P], BF), ("swapm", [P, P], BF),
            ("idf", [P, P], F32), ("onesf", [P, P], F32),
            ("maskd", [P, P], F32), ("cost", [P, NQ], BF),
            ("sint", [P, NQ], BF),
        ):
            cn[n] = const.tile(shape, dt, tag=n, name=n)
            nc.sync.dma_start(cn[n][:], D[n][:])
        qidx_sb = const.tile([P, 4], I32, tag="qidx", name="qidx_sb")
        nc.sync.dma_start(qidx_sb[:], D["qidx"][:])
        idb, onesb, swapm = cn["idb"], cn["onesb"], cn["swapm"]
        idf, onesf, maskd = cn["idf"], cn["onesf"], cn["maskd"]
        cost, sint = cn["cost"], cn["sint"]

        # DRAM bounce buffers for collectives
        ag_in = dram.tile([512, NIMG], F8)
        ag_out = dram.tile([C, NIMG], F8)
        ar1_in = [dram.tile([C, 512], BF, name=f"ar1i{i}") for i in range(2)]
        ar1_out = [dram.tile([C, 512], BF, name=f"ar1o{i}") for i in range(2)]
        ar2_in = [dram.tile([C, 512], BF, name=f"ar2i{i}") for i in range(2)]
        rs2_out = [dram.tile([512, 512], BF, name=f"rs2o{i}")
                   for i in range(2)]

        def cc(kind, op, ins, outs):
            if no_cc:
                # debug fallback: local copies standing in for the exchange
                n_in, n_out = ins[0].shape[0], outs[0].shape[0]
                if kind == "AllGather":
                    for i in range(n_out // n_in):
                        nc.sync.dma_start(
                            outs[0].tensor[i * n_in:(i + 1) * n_in, :],
                            ins[0].tensor[:, :])
                else:
                    nc.sync.dma_start(outs[0].tensor[0:n_out, :],
                                      ins[0].tensor[0:n_out, :])
            else:
                nc.gpsimd.collective_compute(
                    kind, op, replica_groups=GROUPS, ins=ins, outs=outs)

        def mlp_section(rep, with_attn=True):
            with ExitStack() as lctx:
                abig = lctx.enter_context(
                    tc.tile_pool(name=f"abig{rep}", bufs=1))
                wbig = lctx.enter_context(
                    tc.tile_pool(name=f"wbig{rep}", bufs=2))
                mwork = lctx.enter_context(
                    tc.tile_pool(name=f"mwork{rep}", bufs=2))
                mst = lctx.enter_context(
                    tc.tile_pool(name=f"mst{rep}", bufs=1))

                mT = abig.tile([P, CA, NQ], BF, tag="mT", name="mT")
                hTr = D["hTd"].rearrange("(a p) n -> p a n", p=P)

                def wchunk(src, j):
                    wt = wbig.tile([P, CA, 512], BF, tag="w", name="wt")
                    nc.sync.dma_start(
                        wt[:], src[:, j * 512:(j + 1) * 512]
                        .rearrange("(a p) m -> p a m", p=P))
                    return wt

                for ci in range(2):
                    lo_c, hi_c = ci * 512, (ci + 1) * 512
                    # --- assemble h2 (in place into osum) for this half ---
                    osum = mwork.tile([P, CA, 512], BF, tag="osum",
                                      name="osum")
                    if with_attn:
                        # Activation-queue DMA: this read waits on the
                        # AllReduce; on the SP queue it would head-of-line
                        # block the MLP weight stream.
                        nc.scalar.dma_start(
                            osum[:],
                            ar1_out[ci].rearrange("(a p) n -> p a n", p=P))
                        hTh = mwork.tile([P, CA, 512], BF, tag="hTh",
                                         name="hTh", bufs=1)
                        nc.sync.dma_start(hTh[:], hTr[:, :, lo_c:hi_c])
                        for a in range(CA):
                            nc.vector.tensor_add(osum[:, a, :],
                                                 osum[:, a, :], hTh[:, a, :])
                    else:
                        nc.sync.dma_start(osum[:], hTr[:, :, lo_c:hi_c])
                    # --- rmsnorm stats ---
                    var_ps = psd.tile([1, 512], F32, tag="d", name="var")
                    for a in range(CA):
                        sq = mwork.tile([P, 512], F32, tag="sq",
                                        name="sq", bufs=3)
                        nc.scalar.activation(sq[:], osum[:, a, :], AF.Square)
                        nc.tensor.matmul(var_ps[:], onesf[:, 0:1], sq[:],
                                         start=(a == 0), stop=(a == CA - 1))
                    sd2 = mst.tile([1, 512], F32, tag="sd2", name="sd2",
                                   bufs=2)
                    s2b = mst.tile([1, 512], BF, tag="s2b", name="s2b",
                                   bufs=2)
                    nc.vector.tensor_scalar(
                        out=sd2[:], in0=var_ps[:], scalar1=1.0 / C,
                        scalar2=1e-5, op0=OP.mult, op1=OP.add)
                    nc.scalar.activation(sd2[:], sd2[:], AF.Sqrt)
                    nc.vector.reciprocal(sd2[:], sd2[:])
                    nc.vector.tensor_copy(s2b[:], sd2[:])
                    s2bb = mst.tile([P, 512], BF, tag="s2bb", name="s2bb",
                                    bufs=2)
                    sb_ps = psa()
                    nc.tensor.matmul(sb_ps[:], onesb[0:1, :], s2b[0:1, :],
                                     start=True, stop=True)
                    nc.scalar.copy(s2bb[:], sb_ps[:])
                    for a in range(CA):
                        nc.vector.tensor_mul(mT[:, a, lo_c:hi_c],
                                             osum[:, a, :], s2bb[:])
                    if dbg:
                        nc.sync.dma_start(
                            dbg_t["d_h2"][:, lo_c:hi_c]
                            .rearrange("(a p) n -> p a n", p=P), osum[:])
                        if ci == 1:
                            nc.sync.dma_start(
                                dbg_t["d_mT"].rearrange("(a p) n -> p a n",
                                                        p=P), mT[:])

                    # --- MLP for this half ---
                    gact = mwork.tile([P, CA, 512], BF, tag="gact",
                                      name="gact")
                    for j in range(4):
                        wg = wchunk(D["wgate"], j)
                        for mfl in range(4):
                            mf = j * 4 + mfl
                            pp = psa()
                            for a in range(CA):
                                nc.tensor.matmul(
                                    pp[:], wg[:, a, mfl * P:(mfl + 1) * P],
                                    mT[:, a, lo_c:hi_c],
                                    start=(a == 0), stop=(a == CA - 1))
                            sgm = mwork.tile([P, 512], BF, tag="sgm",
                                             name="sgm", bufs=3)
                            nc.scalar.activation(sgm[:], pp[:], AF.Sigmoid)
                            nc.vector.tensor_mul(gact[:, mf, :], pp[:],
                                                 sgm[:])
                    for j in range(4):
                        wu = wchunk(D["wup"], j)
                        for mfl in range(4):
                            mf = j * 4 + mfl
                            pp = psa()
                            for a in range(CA):
                                nc.tensor.matmul(
                                    pp[:], wu[:, a, mfl * P:(mfl + 1) * P],
                                    mT[:, a, lo_c:hi_c],
                                    start=(a == 0), stop=(a == CA - 1))
                            nc.vector.tensor_mul(gact[:, mf, :], pp[:],
                                                 gact[:, mf, :])
                    for j in range(4):
                        wd = wchunk(D["wdown"], j)
                        for mcl in range(4):
                            pp = psa()
                            for a in range(CA):
                                nc.tensor.matmul(
                                    pp[:], wd[:, a, mcl * P:(mcl + 1) * P],
                                    gact[:, a, :],
                                    start=(a == 0), stop=(a == CA - 1))
                            dev = mwork.tile([P, 512], BF, tag="dev",
                                             name="dev", bufs=3)
                            nc.scalar.copy(dev[:], pp[:])
                            nc.scalar.dma_start(
                                ar2_in[ci][(j * 4 + mcl) * P:
                                           (j * 4 + mcl + 1) * P, :],
                                dev[:])
                    cc("ReduceScatter", OP.add, [ar2_in[ci][:]],
                       [rs2_out[ci][:]])

                # --- final assembly: outT = hTq + o_q + mlp_q ---
                hqr = D["hTq"].rearrange("(a p) n -> p a n", p=P)
                for ci in range(2):
                    lo_c, hi_c = ci * 512, (ci + 1) * 512
                    hq = mst.tile([P, 4, 512], BF, tag="hq", name="hq",
                                  bufs=2)
                    nc.sync.dma_start(hq[:], hqr[:, :, lo_c:hi_c])
                    if with_attn:
                        r1 = mst.tile([P, 4, 512], BF, tag="r1", name="r1",
                                      bufs=2)
                        for a in range(4):
                            nc.gpsimd.indirect_dma_start(
                                out=r1[:, a, :], out_offset=None,
                                in_=ar1_out[ci][:],
                                in_offset=bass.IndirectOffsetOnAxis(
                                    ap=qidx_sb[:, a:a + 1], axis=0))
                        nc.vector.tensor_add(hq[:], hq[:], r1[:])
                    r2 = mst.tile([P, 4, 512], BF, tag="r2", name="r2",
                                  bufs=2)
                    nc.sync.dma_start(
                        r2[:], rs2_out[ci].rearrange("(a p) n -> p a n", p=P))
                    nc.vector.tensor_add(hq[:], hq[:], r2[:])
                    nc.sync.dma_start(
                        outT[:, lo_c:hi_c].rearrange("(a p) n -> p a n", p=P),
                        hq[:])


        def layer(rep):
            if phase == "mlp":
                mlp_section(rep, with_attn=False)
                return
            actx = ExitStack()
            att = actx.enter_context(tc.tile_pool(name=f"att{rep}", bufs=1))

            # q/k/v/oT (read until the end of attention) sit at the
            # base of the pool; hTn (dead after the projections) goes
            # above them, so the MLP weight pool reuses hTn's region
            # and its prefetch DMAs don't wait for attention to finish.
            q_sb = att.tile([P, 4, NQ], BF, tag="q")
            k_sb = att.tile([P, 4, NQ], BF, tag="k")
            v_sb = att.tile([P, 8, 512], BF, tag="v")
            oT_sb = att.tile([P, 4, NQ], BF, tag="oT")
            hTn_sb = att.tile([P, CA, NQ], BF, tag="hTn")
            hTn_r = D["hTn"].rearrange("(a p) n -> p a n", p=P)
            for ch in range(4):
                nc.sync.dma_start(
                    hTn_sb[:, ch * 4:(ch + 1) * 4, :],
                    hTn_r[:, ch * 4:(ch + 1) * 4, :])

            # =========================================================
            # offset network + q/k/v projections, interleaved issue so
            # the DVE/Act-heavy offset net hides under qkv matmuls and
            # the sampT AllGather overlaps the tail of the projections.
            # pre/wk2 sit at the top of the SBUF stack and are released
            # before the hd-projection tiles (khdp) allocate, so the
            # causal-attention pool (hw) below never waits on them.
            # =========================================================
            wpr = actx.enter_context(tc.tile_pool(name=f"wpra{rep}", bufs=2))
            rtp = actx.enter_context(tc.tile_pool(name=f"rtpa{rep}", bufs=3))
            hw = actx.enter_context(tc.tile_pool(name=f"hw{rep}", bufs=1))
            pctx = ExitStack()
            pre = pctx.enter_context(tc.tile_pool(name=f"pre{rep}", bufs=1))
            wk2 = pctx.enter_context(tc.tile_pool(name=f"wk2{rep}", bufs=1))

            # ---- offset stage 1: small DMAs + padded lr input ----
            grid_sb = pre.tile([P, 5, 2], F32, tag="grid")
            nc.sync.dma_start(
                grid_sb[:], D["grid"].rearrange("(s p) c -> p s c", p=P))
            convw_sb = pre.tile([P, 4, 9], F32, tag="convw")
            nc.sync.dma_start(
                convw_sb[:], D["convw"].rearrange("(a p) k -> p a k", p=P))
            wlr_sb = pre.tile([P, 4, 256], BF, tag="wlr")
            nc.sync.dma_start(
                wlr_sb[:], D["wlr"].rearrange("(a p) m -> p a m", p=P))
            woff_sb = pre.tile([P, 4, 2], BF, tag="woff")
            nc.sync.dma_start(
                woff_sb[:], D["woff"].rearrange("(a p) m -> p a m", p=P))
            lrin_sb = pre.tile([P, 4, NIMG], BF, tag="lrin")
            nc.sync.dma_start(
                lrin_sb[:], D["lrin"].rearrange("(a p) n -> p a n", p=P))
            xpad = pre.tile([P, 4, 26 * 26], BF, tag="xpad")
            nc.vector.memset(xpad[:], 0.0)
            acc_sb = pre.tile([P, 4, NIMG], BF, tag="acc")
            for a in range(4):
                x3 = xpad[:, a, :].rearrange("p (y x) -> p y x", y=26)
                nc.vector.tensor_copy(
                    x3[:, 1:25, 1:25],
                    lrin_sb[:, a, :].rearrange("p (y x) -> p y x", y=24))

            def conv_group(a):
                # TensorScalarPtr is DVE-only (Pool rejects it in codegen)
                eng = nc.vector
                x3 = xpad[:, a, :].rearrange("p (y x) -> p y x", y=26)
                a3 = acc_sb[:, a, :].rearrange("p (y x) -> p y x", y=24)
                for ky in range(3):
                    for kx in range(3):
                        w_ap = convw_sb[:, a, ky * 3 + kx:ky * 3 + kx + 1]
                        win = x3[:, ky:ky + 24, kx:kx + 24]
                        if ky == 0 and kx == 0:
                            eng.tensor_scalar(
                                out=a3, in0=win, scalar1=w_ap,
                                scalar2=None, op0=OP.mult)
                        else:
                            eng.scalar_tensor_tensor(
                                out=a3, in0=win, scalar=w_ap, in1=a3,
                                op0=OP.mult, op1=OP.add)

            def pnorm_stats(src_sb, na, eps):
                """mean/var over na*128 partitions (PE ones-matmul sums)"""
                red = wk2.tile([1, NIMG], F32, tag="st", bufs=4, name="red")
                red2 = wk2.tile([1, NIMG], F32, tag="st", bufs=4, name="red2")
                sqs = [wk2.tile([P, NIMG], F32, tag="sq1", bufs=1,
                                name="sq1") for _ in range(1)]
                ones_l = onesf if src_sb.dtype == F32 else onesb
                for lo, hi in ((0, 512), (512, NIMG)):
                    rp = psd.tile([1, 512], F32, tag="d", name="rp")
                    for a in range(na):
                        nc.tensor.matmul(rp[:, :hi - lo], ones_l[:, 0:1],
                                         src_sb[:, a, lo:hi],
                                         start=(a == 0), stop=(a == na - 1))
                    nc.scalar.copy(red[0:1, lo:hi], rp[:, :hi - lo])
                rp2 = psd.tile([1, 512], F32, tag="d", name="rp2")
                rp3 = psd.tile([1, 512], F32, tag="d", name="rp3")
                for a in range(na):
                    sq = sqs[0]
                    nc.scalar.activation(sq[:], src_sb[:, a, :], AF.Square)
                    nc.tensor.matmul(rp2[:], onesf[:, 0:1], sq[:, 0:512],
                                     start=(a == 0), stop=(a == na - 1))
                    nc.tensor.matmul(rp3[:, :NIMG - 512], onesf[:, 0:1],
                                     sq[:, 512:NIMG],
                                     start=(a == 0), stop=(a == na - 1))
                nc.scalar.copy(red2[0:1, 0:512], rp2[:])
                nc.scalar.copy(red2[0:1, 512:NIMG], rp3[:, :NIMG - 512])
                nch = float(na * P)
                mu = wk2.tile([1, NIMG], F32, tag="st", bufs=4, name="mu")
                nc.scalar.mul(mu[:], red[:], 1.0 / nch)
                var = wk2.tile([1, NIMG], F32, tag="st", bufs=4, name="var")
                nc.vector.tensor_mul(var[:], mu[:], mu[:])
                nc.vector.scalar_tensor_tensor(
                    out=var[:], in0=red2[:], scalar=1.0 / nch,
                    in1=var[:], op0=OP.mult, op1=OP.subtract)
                nc.vector.tensor_scalar(out=var[:], in0=var[:],
                                        scalar1=eps, scalar2=None, op0=OP.add)
                nc.scalar.activation(var[:], var[:], AF.Sqrt)
                inv = wk2.tile([1, NIMG], F32, tag="inv", name="inv")
                nc.vector.reciprocal(inv[:], var[:])
                aoff = wk2.tile([1, NIMG], F32, tag="aoff", name="aoff")
                nc.vector.scalar_tensor_tensor(
                    out=aoff[:], in0=mu[:], scalar=-1.0, in1=inv[:],
                    op0=OP.mult, op1=OP.mult)
                return inv, aoff

            def pnorm_bcast(inv, aoff):
                invb = wk2.tile([1, NIMG], BF, tag="invb", name="invb")
                aofb = wk2.tile([1, NIMG], BF, tag="aofb", name="aofb")
                nc.scalar.copy(invb[:], inv[:])
                nc.scalar.copy(aofb[:], aoff[:])
                ib = wk2.tile([P, NIMG], BF, tag="ibb", name="ibb")
                ab = wk2.tile([P, NIMG], BF, tag="abb", name="abb")
                for lo, hi in ((0, 512), (512, NIMG)):
                    pi = psd.tile([P, 512], F32, tag="d", name="pi")
                    nc.tensor.matmul(pi[:, :hi - lo], onesb[0:1, :],
                                     invb[0:1, lo:hi], start=True, stop=True)
                    nc.scalar.copy(ib[:, lo:hi], pi[:, :hi - lo])
                    pa = psd.tile([P, 512], F32, tag="d", name="pa")
                    nc.tensor.matmul(pa[:, :hi - lo], onesb[0:1, :],
                                     aofb[0:1, lo:hi], start=True, stop=True)
                    nc.scalar.copy(ab[:, lo:hi], pa[:, :hi - lo])
                return ib, ab

            # ---- qkv projection helpers ----
            def rope_evict(dst, raw_sb, pos_lo, pos_hi):
                n = pos_hi - pos_lo
                rp = psa()
                nc.tensor.matmul(rp[:, :n], swapm[:], raw_sb[:, :n],
                                 start=True, stop=True)
                tmp1 = rtp.tile([P, 512], BF, tag="rt1", name="rt1", bufs=2)
                nc.vector.tensor_mul(tmp1[:, :n], raw_sb[:, :n],
                                     cost[:, pos_lo:pos_hi])
                tmp2 = rtp.tile([P, 512], BF, tag="rt2", name="rt2", bufs=2)
                nc.vector.tensor_mul(tmp2[:, :n], rp[:, :n],
                                     sint[:, pos_lo:pos_hi])
                nc.vector.tensor_add(dst[:, pos_lo:pos_hi], tmp1[:, :n],
                                     tmp2[:, :n])

            def qk_load(wname):
                wt = wpr.tile([P, CA, 512], BF, tag="wpr", name="wt")
                nc.sync.dma_start(
                    wt[:], D[wname].rearrange("(a p) m -> p a m", p=P))
                return wt

            def qk_heads(wt, dst, src_sb, heads, n_src):
                for h in heads:
                    for lo, hi in ((0, 512), (512, n_src)):
                        pp = psa()
                        for a in range(CA):
                            nc.tensor.matmul(pp[:, :hi - lo],
                                             wt[:, a, h * P:(h + 1) * P],
                                             src_sb[:, a, lo:hi],
                                             start=(a == 0),
                                             stop=(a == CA - 1))
                        raw = rtp.tile([P, 512], BF, tag="raw", name="raw")
                        nc.scalar.copy(raw[:, :hi - lo], pp[:, :hi - lo])
                        rope_evict(dst[:, h, :], raw, lo, hi)

            # ---- interleaved issue ----
            wt_q = qk_load("wq")
            conv_group(0)
            conv_group(2)
            qk_heads(wt_q, q_sb, hTn_sb, (0, 1), NQ)
            conv_group(1)
            conv_group(3)
            qk_heads(wt_q, q_sb, hTn_sb, (2, 3), NQ)

            inv1, aoff1 = pnorm_stats(acc_sb, 4, 1e-6)
            ib1, ab1 = pnorm_bcast(inv1, aoff1)
            xg_sb = pre.tile([P, 4, NIMG], BF, tag="xg")
            sgt = wk2.tile([P, NIMG], BF, tag="sgt", name="sgt")
            xh = wk2.tile([P, NIMG], F32, tag="xh", name="xh")
            for a in range(4):
                nc.vector.tensor_mul(xh[:], acc_sb[:, a, :], ib1[:])
                nc.vector.tensor_add(xh[:], xh[:], ab1[:])
                nc.scalar.activation(sgt[:], xh[:], AF.Sigmoid, scale=1.702)
                nc.vector.tensor_mul(xg_sb[:, a, :], xh[:], sgt[:])

            wt_k = qk_load("wk")
            qk_heads(wt_k, k_sb, hTn_sb, (0, 1), NQ)
            if dbg:
                nc.sync.dma_start(
                    dbg_t["d_q"].rearrange("(h p) n -> p h n", p=P), q_sb[:])

            # intent vector
            hmean = wk2.tile([P, CA], F32, tag="hmean", name="hmean")
            hmb = wk2.tile([P, CA], BF, tag="hmb", name="hmb")
            for a in range(CA):
                nc.vector.tensor_reduce(
                    hmean[:, a:a + 1], hTn_sb[:, a, :],
                    axis=mybir.AxisListType.X, op=OP.add)
            nc.vector.tensor_copy(hmb[:], hmean[:])
            intent = wk2.tile([P, 2], F32, tag="intent", name="intent")
            for m in range(4):
                wint_sb = wk2.tile([P, CA, 64], BF, tag="wint",
                                   name="wint_sb", bufs=1)
                nc.sync.dma_start(
                    wint_sb[:],
                    D["wint"][:, m * 64:(m + 1) * 64]
                    .rearrange("(a p) m -> p a m", p=P))
                ip = psm.tile([P, P], F32, tag="t", name="ip")
                prow = slice((m % 2) * 64, (m % 2) * 64 + 64)
                for a in range(CA):
                    nc.tensor.matmul(ip[prow, 0:1], wint_sb[:, a, :],
                                     hmb[:, a:a + 1], start=(a == 0),
                                     stop=(a == CA - 1))
                nc.scalar.mul(intent[prow, m // 2:m // 2 + 1],
                              ip[prow, 0:1], 1.0 / NQ)

            # cat = [xproj ; intent] -> ln2 (in place) -> off
            cat_sb = pre.tile([P, 4, NIMG], BF, tag="cat")
            for m in range(2):
                for lo, hi in ((0, 512), (512, NIMG)):
                    xp = psd.tile([P, 512], F32, tag="d", name="xp")
                    for a in range(4):
                        nc.tensor.matmul(xp[:, :hi - lo],
                                         wlr_sb[:, a, m * P:(m + 1) * P],
                                         xg_sb[:, a, lo:hi],
                                         start=(a == 0), stop=(a == 3))
                    nc.scalar.copy(cat_sb[:, m, lo:hi], xp[:, :hi - lo])
            for m in range(2):
                nc.vector.tensor_scalar(
                    out=cat_sb[:, 2 + m, :], in0=xg_sb[:, 0, :],
                    scalar1=0.0, scalar2=intent[:, m:m + 1], op0=OP.mult,
                    op1=OP.add)

            qk_heads(wt_k, k_sb, hTn_sb, (2, 3), NQ)
            if dbg:
                nc.sync.dma_start(
                    dbg_t["d_k"].rearrange("(h p) n -> p h n", p=P), k_sb[:])

            inv2, aoff2 = pnorm_stats(cat_sb, 4, 1e-6)
            ib2, ab2 = pnorm_bcast(inv2, aoff2)
            for a in range(4):
                nc.vector.tensor_mul(xh[:], cat_sb[:, a, :], ib2[:])
                nc.vector.tensor_add(cat_sb[:, a, :], xh[:], ab2[:])

            off_sb = wk2.tile([2, NPAD], F32, tag="off", name="off")
            nc.vector.memset(off_sb[:], 0.0)
            for lo, hi in ((0, 512), (512, NIMG)):
                op_ = psd.tile([2, 512], F32, tag="d", name="opp")
                for a in range(4):
                    nc.tensor.matmul(op_[:, :hi - lo], woff_sb[:, a, :],
                                     cat_sb[:, a, lo:hi], start=(a == 0),
                                     stop=(a == 3))
                nc.scalar.copy(off_sb[:, lo:hi], op_[:, :hi - lo])

            # bilinear coordinates, batched across all 5 s-tiles
            idx0 = wk2.tile([P, 5], I32, tag="idx0", name="idx0")
            idx1 = wk2.tile([P, 5], I32, tag="idx1", name="idx1")
            wcmb = wk2.tile([P, 5, 4], F32, tag="wcmb", name="wcmb")
            t2 = wk2.tile([P, 5, 2], F32, tag="t2", name="t2")
            fr = wk2.tile([P, 5, 2], F32, tag="fr", name="fr")
            f0 = wk2.tile([P, 5, 2], F32, tag="f0", name="f0")
            f1 = wk2.tile([P, 5, 2], F32, tag="f1", name="f1")
            w1m = wk2.tile([P, 5, 2], F32, tag="w1m", name="w1m")
            fi = wk2.tile([P, 5, 1], F32, tag="fi", name="fi")
            tps_c = psm.tile([P, 5, 2], F32, tag="t", name="tps_c")
            for st in range(5):
                nc.tensor.transpose(tps_c[:, st, :],
                                    off_sb[0:2, st * P:(st + 1) * P],
                                    idf[0:2, 0:2])
            nc.scalar.activation(t2[:], tps_c[:], AF.Tanh)
            nc.vector.scalar_tensor_tensor(
                out=t2[:], in0=t2[:], scalar=2.0 / LR,
                in1=grid_sb[:], op0=OP.mult, op1=OP.add)
            nc.vector.tensor_scalar(out=t2[:], in0=t2[:], scalar1=1.0,
                                    scalar2=-1.0, op0=OP.min, op1=OP.max)
            nc.vector.tensor_scalar(out=t2[:], in0=t2[:], scalar1=1.0,
                                    scalar2=(HR - 1) / 2.0,
                                    op0=OP.add, op1=OP.mult)
            ti = wk2.tile([P, 5, 2], I32, tag="ti", name="ti")
            nc.vector.tensor_copy(ti[:], t2[:])
            nc.vector.tensor_copy(f0[:], ti[:])
            nc.vector.tensor_tensor(out=fr[:], in0=f0[:], in1=t2[:],
                                    op=OP.is_gt)
            nc.vector.tensor_sub(f0[:], f0[:], fr[:])
            nc.vector.tensor_sub(fr[:], t2[:], f0[:])
            nc.vector.tensor_scalar(out=f1[:], in0=f0[:], scalar1=1.0,
                                    scalar2=float(HR - 1), op0=OP.add,
                                    op1=OP.min)
            nc.vector.scalar_tensor_tensor(
                out=fi[:], in0=f0[:, :, 0:1], scalar=float(HR),
                in1=f0[:, :, 1:2], op0=OP.mult, op1=OP.add)
            nc.vector.tensor_copy(idx0[:], fi[:, :, 0])
            nc.vector.scalar_tensor_tensor(
                out=fi[:], in0=f1[:, :, 0:1], scalar=float(HR),
                in1=f0[:, :, 1:2], op0=OP.mult, op1=OP.add)
            nc.vector.tensor_copy(idx1[:], fi[:, :, 0])
            nc.vector.tensor_scalar(out=w1m[:], in0=fr[:],
                                    scalar1=-1.0, scalar2=1.0,
                                    op0=OP.mult, op1=OP.add)
            nc.vector.tensor_mul(wcmb[:, :, 0:1], w1m[:, :, 0:1],
                                 w1m[:, :, 1:2])
            nc.vector.tensor_mul(wcmb[:, :, 1:2], w1m[:, :, 0:1],
                                 fr[:, :, 1:2])
            nc.vector.tensor_mul(wcmb[:, :, 2:3], fr[:, :, 0:1],
                                 w1m[:, :, 1:2])
            nc.vector.tensor_mul(wcmb[:, :, 3:4], fr[:, :, 0:1],
                                 fr[:, :, 1:2])

            wt_v = qk_load("wv")
            for m8 in range(4):
                pp = psa()
                for a in range(CA):
                    nc.tensor.matmul(pp[:],
                                     hTn_sb[:, a, m8 * P:(m8 + 1) * P],
                                     wt_v[:, a, :], start=(a == 0),
                                     stop=(a == CA - 1))
                nc.scalar.copy(v_sb[:, m8, :], pp[:])

            # gather + combine + transpose
            sampT_mine = pre.tile([P, 4, NPAD], F8, tag="sampT_mine")
            for st in range(5):
                p0 = wk2.tile([P, 1024], BF, tag="p0", bufs=1, name="p0")
                p1 = wk2.tile([P, 1024], BF, tag="p1", bufs=1, name="p1")
                nc.gpsimd.indirect_dma_start(
                    out=p0[:], out_offset=None, in_=D["imgp"][:],
                    in_offset=bass.IndirectOffsetOnAxis(
                        ap=idx0[:, st:st + 1], axis=0))
                nc.gpsimd.indirect_dma_start(
                    out=p1[:], out_offset=None, in_=D["imgp"][:],
                    in_offset=bass.IndirectOffsetOnAxis(
                        ap=idx1[:, st:st + 1], axis=0))
                smp = wk2.tile([P, 512], BF, tag="smp", bufs=2, name="smp")
                nc.vector.tensor_tensor(
                    out=smp[:], in0=p0[:, 0:512],
                    in1=wcmb[:, st, 0:1].to_broadcast([P, 512]), op=OP.mult)
                for pair, col in ((p0, 1), (p1, 2), (p1, 3)):
                    src = pair[:, 0:512] if col == 2 else pair[:, 512:1024]
                    nc.vector.scalar_tensor_tensor(
                        out=smp[:], in0=src,
                        scalar=wcmb[:, st, col:col + 1], in1=smp[:],
                        op0=OP.mult, op1=OP.add)
                for cm in range(4):
                    tp = psm.tile([P, P], BF, tag="t", name="tps")
                    nc.tensor.transpose(tp[:], smp[:, cm * P:(cm + 1) * P],
                                        idb[:])
                    nc.scalar.copy(
                        sampT_mine[:, cm, st * P:(st + 1) * P], tp[:])
            nc.sync.dma_start(ag_in.rearrange("(a p) n -> p a n", p=P),
                              sampT_mine[:, :, 0:NIMG])
            cc("AllGather", OP.bypass, [ag_in[:]], [ag_out[:]])

            # remaining v tiles while AllGather flies
            for m8 in range(4, 8):
                pp = psa()
                for a in range(CA):
                    nc.tensor.matmul(pp[:],
                                     hTn_sb[:, a, m8 * P:(m8 + 1) * P],
                                     wt_v[:, a, :], start=(a == 0),
                                     stop=(a == CA - 1))
                nc.scalar.copy(v_sb[:, m8, :], pp[:])

            # =========================================================
            # attention: causal tiles first (they only need q/k/v, so
            # they fill the AllGather window), head-pairs interleaved
            # for PE pipelining; image tiles + softmax finalize after
            # the hd projections land. Denominators accumulate on the
            # otherwise-idle Pool engine.
            # =========================================================
            daccs, oAs = {}, {}
            sampT_sb = khd_sb = vhd_sb = None

            def sc_exp_o(ci, pair, kind, kt, o_pss, start, stop,
                         dinit=False):
                lo_c, hi_c = ci * 512, (ci + 1) * 512
                if kind == "c":
                    qlo, kp = kt * P, P
                else:
                    qlo = 0
                    kp = P if kt < 4 else NIMG - 4 * P
                lo = max(qlo, lo_c)
                n = hi_c - lo
                o = lo - lo_c
                for h in pair:
                    dacc = daccs[(ci, h)]
                    sp = psa()
                    if kind == "c":
                        nc.tensor.matmul(sp[:, :n],
                                         k_sb[:, h, kt * P:(kt + 1) * P],
                                         q_sb[:, h, lo:hi_c],
                                         start=True, stop=True)
                        if lo == qlo:
                            nc.vector.tensor_add(sp[:, 0:P], sp[:, 0:P],
                                                 maskd[:])
                        lhs = v_sb[:, kt, h * P:(h + 1) * P]
                    else:
                        nc.tensor.matmul(sp[:kp, :n],
                                         khd_sb[:, h, kt * P:kt * P + kp],
                                         q_sb[:, h, lo:hi_c],
                                         start=True, stop=True)
                        lhs = vhd_sb[:kp, kt, h * P:(h + 1) * P]
                    ex = hw.tile([P, 512], BF, tag="ex", name="ex", bufs=3)
                    nc.scalar.activation(ex[:kp, o:], sp[:kp, :n], AF.Exp,
                                         scale=SCALE)
                    nc.tensor.matmul(o_pss[h][:, o:], lhs, ex[:kp, o:],
                                     start=start, stop=stop)
                    deng = nc.gpsimd if DACC_POOL else nc.vector
                    if dinit:
                        deng.tensor_copy(dacc[:kp, :], ex[:kp, :])
                    else:
                        deng.tensor_add(dacc[:kp, o:], dacc[:kp, o:],
                                        ex[:kp, o:])

            def causal_pass(ci, pair):
                ncaus = 4 * (ci + 1)
                o_pss = {h: psa() for h in pair}
                for h in pair:
                    daccs[(ci, h)] = hw.tile([P, 512], F32, tag="dacc",
                                             name=f"dc{ci}{h}", bufs=8)
                for kt in range(ncaus):
                    sc_exp_o(ci, pair, "c", kt, o_pss,
                             start=(kt == 0), stop=(kt == ncaus - 1),
                             dinit=(kt == 0))
                for h in pair:
                    oA = hw.tile([P, 512], BF, tag="oA",
                                 name=f"oA{ci}{h}", bufs=8)
                    nc.scalar.copy(oA[:], o_pss[h][:])
                    oAs[(ci, h)] = oA

            def img_pass(ci, pair):
                lo_c, hi_c = ci * 512, (ci + 1) * 512
                o_pss = {h: psa() for h in pair}
                for it in range(5):
                    sc_exp_o(ci, pair, "i", it, o_pss,
                             start=(it == 0), stop=(it == 4))
                for h in pair:
                    den = psd.tile([1, 512], F32, tag="d", name="den")
                    nc.tensor.matmul(den[:], onesf[:, 0:1],
                                     daccs[(ci, h)][:], start=True, stop=True)
                    rcf = hw.tile([1, 512], F32, tag="rcf", name="rcf",
                                  bufs=1)
                    rcb = hw.tile([1, 512], BF, tag="rcb", name="rcb",
                                  bufs=2)
                    nc.vector.reciprocal(rcf[:], den[:])
                    nc.vector.tensor_copy(rcb[:], rcf[:])
                    rb = psa()
                    nc.tensor.matmul(rb[:], onesb[0:1, :], rcb[0:1, :],
                                     start=True, stop=True)
                    rbs = hw.tile([P, 512], BF, tag="rbs", name="rbs",
                                  bufs=1)
                    nc.scalar.copy(rbs[:], rb[:])
                    otmp = hw.tile([P, 512], BF, tag="otmp", name="otmp",
                                   bufs=1)
                    nc.vector.tensor_add(otmp[:], oAs[(ci, h)][:],
                                         o_pss[h][:])
                    nc.vector.tensor_mul(oT_sb[:, h, lo_c:hi_c], otmp[:],
                                         rbs[:])

            # causal part of attention (during the AllGather flight)
            for ci in range(2):
                for pair in ((0, 1), (2, 3)):
                    causal_pass(ci, pair)

            pctx.close()

            # hd-tile pool reuses the released pre/wk2 region; its
            # writes only depend on the AllGather anyway.
            khdp = actx.enter_context(tc.tile_pool(name=f"khdp{rep}",
                                                   bufs=1))
            sampT_sb = khdp.tile([P, CA, NIMG], BF, tag="sampT")
            sampT8_sb = khdp.tile([P, CA, NIMG], F8, tag="sampT8")
            khd_sb = khdp.tile([P, 4, NIMG], BF, tag="khd")
            vhd_sb = khdp.tile([P, 5, 512], BF, tag="vhd")
            wo_sb = khdp.tile([P, 4, C], BF, tag="wo")
            wt_khd = qk_load("wkhd")
            wt_vhd = qk_load("wvhd")
            nc.sync.dma_start(
                wo_sb[:], D["wo"].rearrange("(a p) m -> p a m", p=P))
            # Activation-queue DMA: waits on the AllGather; on SP it would
            # block the o-proj eviction stream and MLP weight prefetch.
            nc.scalar.dma_start(
                sampT8_sb[:], ag_out.rearrange("(a p) n -> p a n", p=P))
            # fp8 -> bf16 in column chunks so khd starts during the cast
            nc.vector.tensor_copy(sampT_sb[:, :, 0:512],
                                  sampT8_sb[:, :, 0:512])
            nc.vector.tensor_copy(sampT_sb[:, :, 512:NIMG],
                                  sampT8_sb[:, :, 512:NIMG])
            if dbg:
                nc.sync.dma_start(
                    dbg_t["d_samp"].rearrange("(a p) n -> p a n", p=P),
                    sampT_sb[:])

            # ---- hd-token projections (need the AllGather result) ----
            for h in range(4):
                for lo, hi in ((0, 512), (512, NIMG)):
                    pp = psa()
                    for a in range(CA):
                        nc.tensor.matmul(pp[:, :hi - lo],
                                         wt_khd[:, a, h * P:(h + 1) * P],
                                         sampT_sb[:, a, lo:hi],
                                         start=(a == 0), stop=(a == CA - 1))
                    raw = rtp.tile([P, 512], BF, tag="raw", name="raw")
                    nc.scalar.copy(raw[:, :hi - lo], pp[:, :hi - lo])
                    rope_evict(khd_sb[:, h, :], raw, lo, hi)
            if dbg:
                nc.sync.dma_start(
                    dbg_t["d_khd"].rearrange("(h p) n -> p h n", p=P),
                    khd_sb[:])

            for st in range(5):
                kp = P if st < 4 else NIMG - 4 * P
                pp = psa()
                for a in range(CA):
                    nc.tensor.matmul(pp[:kp, :],
                                     sampT_sb[:, a, st * P:st * P + kp],
                                     wt_vhd[:, a, :], start=(a == 0),
                                     stop=(a == CA - 1))
                nc.scalar.copy(vhd_sb[:kp, st, :], pp[:kp, :])

            # ---- image attention + o-projection per token half ----
            for ci in range(2):
                lo_c, hi_c = ci * 512, (ci + 1) * 512
                img_pass(ci, (0, 1))
                img_pass(ci, (2, 3))
                if dbg and ci == 1:
                    nc.sync.dma_start(
                        dbg_t["d_oT"].rearrange("(h p) n -> p h n", p=P),
                        oT_sb[:])

                # o-projection for this token half -> ReduceScatter -> AG
                for m in range(CA):
                    pp = psa()
                    for h in range(4):
                        nc.tensor.matmul(pp[:], wo_sb[:, h, m * P:(m + 1) * P],
                                         oT_sb[:, h, lo_c:hi_c],
                                         start=(h == 0), stop=(h == 3))
                    oev = khdp.tile([P, 512], BF, tag="oev", bufs=3,
                                    name="oev")
                    nc.scalar.copy(oev[:], pp[:])
                    # Act-queue DMA: an SP-queue write here would stall SP
                    # on o-proj completion and block MLP weight prefetch.
                    nc.scalar.dma_start(ar1_in[ci][m * P:(m + 1) * P, :],
                                        oev[:])
                cc("AllReduce", OP.add, [ar1_in[ci][:]], [ar1_out[ci][:]])

            actx.close()

            if phase == "attn":
                with ExitStack() as lctx:
                    mstx = lctx.enter_context(
                        tc.tile_pool(name=f"mstx{rep}", bufs=2))
                    hqr = D["hTq"].rearrange("(a p) n -> p a n", p=P)
                    for ci in range(2):
                        lo_c, hi_c = ci * 512, (ci + 1) * 512
                        hq = mstx.tile([P, 4, 512], BF, tag="hq", name="hq")
                        nc.sync.dma_start(hq[:], hqr[:, :, lo_c:hi_c])
                        r1 = mstx.tile([P, 4, 512], BF, tag="r1", name="r1")
                        for a in range(4):
                            nc.gpsimd.indirect_dma_start(
                                out=r1[:, a, :], out_offset=None,
                                in_=ar1_out[ci][:],
                                in_offset=bass.IndirectOffsetOnAxis(
                                    ap=qidx_sb[:, a:a + 1], axis=0))
                        nc.vector.tensor_add(hq[:], hq[:], r1[:])
                        nc.sync.dma_start(
                            outT[:, lo_c:hi_c]
                            .rearrange("(a p) n -> p a n", p=P), hq[:])
                return

            mlp_section(rep, with_attn=True)

        for rep in range(reps):
            layer(rep)

    nc.compile()
    return nc


import time
import jax
from jax.sharding import Mesh, PartitionSpec
from jax.experimental.shard_map import shard_map
from concourse import bass2jax
from concourse.bass2jax import _bass_exec_p, install_neuronx_cc_hook, \
    partition_id_tensor


class TimedRunner:
    def __init__(self, nc, n_cores=8):
        install_neuronx_cc_hook()
        self.nc = nc
        self.n_cores = n_cores
        partition_name = (nc.partition_id_tensor.name
                          if nc.partition_id_tensor else None)
        in_names, out_names, out_avals, zero_outs = [], [], [], []
        for alloc in nc.m.functions[0].allocations:
            if not isinstance(alloc, mybir.MemoryLocationSet):
                continue
            name = alloc.memorylocations[0].name
            if alloc.kind == "ExternalInput":
                if name != partition_name:
                    in_names.append(name)
            elif alloc.kind == "ExternalOutput":
                out_names.append(name)
                shape = tuple(alloc.tensor_shape)
                dtype = mybir.dt.np(alloc.dtype)
                out_avals.append(jax.core.ShapedArray(shape, dtype))
                zero_outs.append(np.zeros(shape, dtype))
        if nc.dbg_addr is not None:
            assert not nc.dbg_callbacks
        self.in_names = list(in_names)
        self.out_names = out_names
        self.out_avals = out_avals
        self.zero_outs = zero_outs
        n_params = len(in_names)
        n_outs = len(out_avals)
        all_in_names = list(in_names) + list(out_names)
        if partition_name is not None:
            all_in_names.append(partition_name)
        self.partition_name = partition_name

        def _body(*args):
            operands = list(args)
            if partition_name is not None:
                operands.append(partition_id_tensor())
            outs = _bass_exec_p.bind(
                *operands,
                out_avals=tuple(out_avals),
                in_names=tuple(all_in_names),
                out_names=tuple(out_names),
                lowering_input_output_aliases=(),
                sim_require_finite=True,
                sim_require_nnan=True,
                nc=nc,
            )
            return tuple(outs)

        devices = jax.devices()[:n_cores]
        mesh = Mesh(np.asarray(devices), ("core",))
        in_specs = (PartitionSpec("core"),) * (n_params + n_outs)
        out_specs = (PartitionSpec("core"),) * n_outs
        # no donation so the function is re-callable with the same buffers
        self.fn = jax.jit(shard_map(_body, mesh=mesh, in_specs=in_specs,
                                    out_specs=out_specs, check_rep=False))
        self.mesh = mesh

    def put_inputs(self, in_maps):
        dbg = {}
        if self.nc.dbg_addr is not None:
            dbg = {self.nc.dbg_addr.name: np.zeros((1, 2), np.uint32)}
        per_core = [[np.asarray({**m, **dbg}[n]) for n in self.in_names]
                    for m in in_maps]
        n_params = len(self.in_names)
        concat_in = [
            np.concatenate([per_core[c][i] for c in range(self.n_cores)],
                           axis=0) for i in range(n_params)]
        concat_zeros = [
            np.zeros((self.n_cores * z.shape[0], *z.shape[1:]), z.dtype)
            for z in self.zero_outs]
        sh = jax.sharding.NamedSharding(self.mesh, PartitionSpec("core"))
        self.dev_args = [jax.device_put(a, sh)
                         for a in (*concat_in, *concat_zeros)]

    def run(self):
        outs = jax.block_until_ready(self.fn(*self.dev_args))
        return outs

    def results(self, outs):
        return [
            {n: np.asarray(outs[i]).reshape(
                self.n_cores, *self.out_avals[i].shape)[c]
             for i, n in enumerate(self.out_names)}
            for c in range(self.n_cores)
        ]

    def bench(self, iters=5):
        self.run()
        best = float("inf")
        for _ in range(iters):
            t0 = time.perf_counter()
            self.run()
            best = min(best, time.perf_counter() - t0)
        return best


# ----------------------------------------------------------------- entry
_NC_CACHE = {}


def _get_nc(reps=1):
    if reps not in _NC_CACHE:
        _NC_CACHE[reps] = build(dbg=False, reps=reps)
    return _NC_CACHE[reps]


def kernel(**inputs) -> np.ndarray:
    """Full inputs -> full [2, 1024, 2048] fp32 output, computed on the
    8 TRN2 NeuronCores (DPxTP sharding, bf16 compute)."""
    from concourse.bass_utils import run_bass_kernel_spmd
    nc = _get_nc(1)
    maps = prep_inputs(inputs)
    res = run_bass_kernel_spmd(nc, maps, list(range(8)))
    return finish(res.results)


def benchmark_device_time(inputs, reps_hi=11, npipe=16, trials=16):
    """Per-layer device execution time: difference an on-device
    reps_hi-iteration NEFF against the single-iteration NEFF under
    pipelined launches (axon host dispatch is ~100ms and would otherwise
    swamp the sub-ms kernel). Samples are interleaved lo/hi/lo so slow
    host-side drift cancels; median over trials for jitter robustness."""
    import time as _time
    import statistics as _stats
    import jax as _jax
    maps = prep_inputs(inputs)
    trs = {}
    for reps in (1, reps_hi):
        tr = TimedRunner(_get_nc(reps), 8)
        tr.put_inputs(maps)
        tr.run()
        trs[reps] = tr

    def sample(tr):
        t0 = _time.perf_counter()
        outs = None
        for _ in range(npipe):
            outs = tr.fn(*tr.dev_args)
        _jax.block_until_ready(outs)
        return (_time.perf_counter() - t0) / npipe

    sample(trs[1])
    sample(trs[reps_hi])
    diffs = []
    for _ in range(trials):
        a = sample(trs[1])
        b = sample(trs[reps_hi])
        a2 = sample(trs[1])
        diffs.append((b - (a + a2) / 2) / (reps_hi - 1))
    return max(_stats.median(diffs), 1e-9)


# revision 43
# speedup vs baseline: 1.0611x; 1.0457x over previous
"""Bass/Tile kernel for nn_LlamaDecoderLayerDAT on 8 TRN2 cores.

Sharding: DP(batch=2) x TP(4) within batch groups [[0..3],[4..7]].
Core c: batch b=c//4, TP slot g=c%4 (heads 4g..4g+3, dff slice g*2048,
offset-net channel group g, output channel shard g*512..(g+1)*512).

All activations on device live in transposed [channel(part), token(free)]
layout, bf16 compute with fp32 PSUM accumulation.

Collective plan (all within the 4-core TP group):
  - sampT: AllGather (issued early, overlapped with q/k/v projections)
  - o-projection: per token half: ReduceScatter (each core gets its
    512-channel quarter of the o-sum) then AllGather back to full C;
    residual h2 = hTd + o_full assembled on the consumer side.
  - MLP down-projection: per token half: ReduceScatter only; each core
    emits outT shard = hTq + o_quarter + mlp_quarter; the host
    reassembles the 4 channel shards per batch.
Attention/o-proj/MLP are issued in token-half phases so no engine queue
ever blocks on a later collective (head-of-line) before earlier-phase
compute has been issued.
"""
import numpy as np
import ml_dtypes
from contextlib import ExitStack

import concourse.bass as bass
import concourse.bacc as bacc
import concourse.tile as tile
from concourse import mybir

BF = mybir.dt.bfloat16
F32 = mybir.dt.float32
F8 = mybir.dt.float8e4
I32 = mybir.dt.int32
AF = mybir.ActivationFunctionType
OP = mybir.AluOpType

P = 128
NQ, C, NH, HD = 1024, 2048, 16, 128
DFF = 8192
LR, HR, NIMG, NPAD = 24, 48, 576, 640
CA = C // P              # 16 K-tiles over channels
SCALE = float(1.0 / np.sqrt(HD))
GROUPS = [[0, 1, 2, 3], [4, 5, 6, 7]]
NEG = -1.0e30
bf16 = ml_dtypes.bfloat16
DACC_POOL = False


def _bf(x):
    return np.asarray(x, np.float32).astype(bf16)


# ----------------------------------------------------------------- host side
def _rope_tables():
    inv = 1.0 / (10000.0 ** (np.arange(0, HD, 2, dtype=np.float32) / HD))
    ang = np.arange(NQ, dtype=np.float32)[:, None] * inv[None, :]
    ang = np.concatenate([ang, ang], axis=-1)                 # [NQ, 128]
    sgn = np.ones((HD,), np.float32)
    sgn[: HD // 2] = -1.0
    return np.cos(ang).T.copy(), (np.sin(ang) * sgn[None, :]).T.copy()


def _grid640():
    ys = (np.linspace(0.5, LR - 0.5, LR, dtype=np.float32) / (LR - 1.0)) * 2 - 1
    gy, gx = np.meshgrid(ys, ys, indexing="ij")
    g = np.zeros((NPAD, 2), np.float32)
    g[:NIMG, 0] = gy.reshape(-1)
    g[:NIMG, 1] = gx.reshape(-1)
    return g


def prep_inputs(inputs):
    """Full problem inputs -> list of 8 per-core in_maps."""
    W = {k: np.asarray(v, np.float32) for k, v in inputs.items()}
    hid = W["hidden_states"]
    img = W["image_hd_features"]
    cosT, sinT = _rope_tables()
    kk = np.arange(P)
    maskd = np.where(kk[:, None] > kk[None, :], np.float32(NEG),
                     np.float32(0.0))
    swap = np.zeros((P, P), np.float32)
    swap[np.arange(P), (np.arange(P) + 64) % P] = 1.0
    shared = dict(
        cost=_bf(cosT), sint=_bf(sinT), grid=_grid640(),
        maskd=maskd, swapm=_bf(swap),
        idb=_bf(np.eye(P)), idf=np.eye(P, dtype=np.float32),
        onesb=_bf(np.ones((P, P))), onesf=np.ones((P, P), np.float32),
        convw=np.ascontiguousarray(W["conv_dw_w"].reshape(512, 9)),
        wlr=_bf(W["Wlrproj"]), wint=_bf(W["Wint"]), woff=_bf(W["Woff"]),
    )
    maps = []
    for c in range(8):
        b, g = c // 4, c % 4
        hT = np.ascontiguousarray(hid[b].T)                   # [C, NQ]
        s = 1.0 / np.sqrt((hid[b] ** 2).mean(-1) + 1e-5)      # [NQ]
        hTn = hT * s[None, :]
        img_g = np.ascontiguousarray(img[b][:, g * 512:(g + 1) * 512])
        flat = img_g.reshape(-1)
        st = flat.strides[0]
        imgp = np.zeros((HR * HR, 1024), np.float32)
        imgp[:HR * HR - 1] = np.lib.stride_tricks.as_strided(
            flat, (HR * HR - 1, 1024), (st * 512, st))
        imgp[HR * HR - 1, :512] = img_g[HR * HR - 1]
        hsl = slice(g * 512, (g + 1) * 512)
        fsl = slice(g * 2048, (g + 1) * 2048)
        m = dict(shared)
        m.update(
            hTn=_bf(hTn), hTd=_bf(hT), hTq=_bf(hT[hsl]),
            qidx=(g * 512 + np.arange(4, dtype=np.int32)[None, :] * 128
                  + np.arange(P, dtype=np.int32)[:, None]).copy(),
            lrin=_bf(hTn[hsl, :NIMG]),
            imgp=_bf(imgp),
            wq=_bf(W["Wq"][:, hsl]), wk=_bf(W["Wk"][:, hsl]),
            wv=_bf(W["Wv"][:, hsl]), wo=_bf(W["Wo"][hsl, :]),
            wkhd=_bf(W["Wk_hd"][:, hsl]), wvhd=_bf(W["Wv_hd"][:, hsl]),
            wgate=_bf(W["Wgate"][:, fsl]), wup=_bf(W["Wup"][:, fsl]),
            wdown=_bf(W["Wdown"][fsl, :]),
        )
        maps.append(m)
    return maps


def finish(results):
    out = np.empty((2, NQ, C), np.float32)
    for b in range(2):
        for g in range(4):
            sh = np.asarray(results[4 * b + g]["outT"]).astype(np.float32)
            out[b, :, g * 512:(g + 1) * 512] = sh.T
    return out


# --------------------------------------------------------------- device side
def build(dbg=False, reps=1, no_cc=False, phase="full"):
    nc = bacc.Bacc("TRN2", num_devices=8)
    D = {}

    def inp(name, shape, dt):
        D[name] = nc.dram_tensor(name, shape, dt, kind="ExternalInput")
        return D[name]

    for n in ("hTn", "hTd"):
        inp(n, [C, NQ], BF)
    inp("hTq", [512, NQ], BF)
    inp("lrin", [512, NIMG], BF)
    inp("imgp", [HR * HR, 1024], BF)
    for n in ("wq", "wk", "wv", "wkhd", "wvhd"):
        inp(n, [C, 512], BF)
    inp("wo", [512, C], BF)
    for n in ("wgate", "wup"):
        inp(n, [C, 2048], BF)
    inp("wdown", [2048, C], BF)
    inp("wlr", [512, 256], BF)
    inp("wint", [C, 256], BF)
    inp("woff", [512, 2], BF)
    inp("convw", [512, 9], F32)
    inp("cost", [P, NQ], BF)
    inp("sint", [P, NQ], BF)
    inp("grid", [NPAD, 2], F32)
    inp("qidx", [P, 4], I32)
    inp("maskd", [P, P], F32)
    for n in ("idb", "onesb", "swapm"):
        inp(n, [P, P], BF)
    for n in ("idf", "onesf"):
        inp(n, [P, P], F32)

    outT = nc.dram_tensor("outT", [512, NQ], BF, kind="ExternalOutput")
    dbg_t = {}
    if dbg:
        for n, shape, dt in (
            ("d_samp", [C, NIMG], BF), ("d_q", [512, NQ], BF),
            ("d_k", [512, NQ], BF), ("d_khd", [512, NIMG], BF),
            ("d_oT", [512, NQ], BF), ("d_h2", [C, NQ], BF),
            ("d_mT", [C, NQ], BF),
        ):
            dbg_t[n] = nc.dram_tensor(n, shape, dt, kind="ExternalOutput")

    with tile.TileContext(nc) as tc, ExitStack() as ctx:
        const = ctx.enter_context(tc.tile_pool(name="const", bufs=1))
        dram = ctx.enter_context(tc.tile_pool(name="dram", bufs=1,
                                              space="DRAM"))
        ps = ctx.enter_context(tc.tile_pool(name="ps", bufs=4, space="PSUM"))
        psd = ctx.enter_context(tc.tile_pool(name="psd", bufs=2, space="PSUM"))
        psm = ctx.enter_context(tc.tile_pool(name="psm", bufs=2, space="PSUM"))

        def psa():
            return ps.tile([P, 512], F32, tag="a", name="psa")

        # ---- persistent consts ----
        cn = {}
        for n, shape, dt in (
            ("idb", [P, P], BF), ("onesb", [P, P], BF), ("swapm", [P, P], BF),
            ("idf", [P, P], F32), ("onesf", [P, P], F32),
            ("maskd", [P, P], F32), ("cost", [P, NQ], BF),
            ("sint", [P, NQ], BF),
        ):
            cn[n] = const.tile(shape, dt, tag=n, name=n)
            nc.sync.dma_start(cn[n][:], D[n][:])
        qidx_sb = const.tile([P, 4], I32, tag="qidx", name="qidx_sb")
        nc.sync.dma_start(qidx_sb[:], D["qidx"][:])
        idb, onesb, swapm = cn["idb"], cn["onesb"], cn["swapm"]
        idf, onesf, maskd = cn["idf"], cn["onesf"], cn["maskd"]
        cost, sint = cn["cost"], cn["sint"]

        # DRAM bounce buffers for collectives
        ag_in = dram.tile([512, NIMG], F8)
        ag_out = dram.tile([C, NIMG], F8)
        ar1_in = [dram.tile([C, 512], F8, name=f"ar1i{i}") for i in range(2)]
        ar1_out = [dram.tile([C, 512], F8, name=f"ar1o{i}") for i in range(2)]
        ar2_in = [dram.tile([C, 512], BF, name=f"ar2i{i}") for i in range(2)]
        rs2_out = [dram.tile([512, 512], BF, name=f"rs2o{i}")
                   for i in range(2)]

        def cc(kind, op, ins, outs):
            if no_cc:
                # debug fallback: local copies standing in for the exchange
                n_in, n_out = ins[0].shape[0], outs[0].shape[0]
                if kind == "AllGather":
                    for i in range(n_out // n_in):
                        nc.sync.dma_start(
                            outs[0].tensor[i * n_in:(i + 1) * n_in, :],
                            ins[0].tensor[:, :])
                else:
                    nc.sync.dma_start(outs[0].tensor[0:n_out, :],
                                      ins[0].tensor[0:n_out, :])
            else:
                nc.gpsimd.collective_compute(
                    kind, op, replica_groups=GROUPS, ins=ins, outs=outs)

        def mlp_section(rep, with_attn=True):
            with ExitStack() as lctx:
                abig = lctx.enter_context(
                    tc.tile_pool(name=f"abig{rep}", bufs=1))
                wbig = lctx.enter_context(
                    tc.tile_pool(name=f"wbig{rep}", bufs=2))
                mwork = lctx.enter_context(
                    tc.tile_pool(name=f"mwork{rep}", bufs=2))
                mst = lctx.enter_context(
                    tc.tile_pool(name=f"mst{rep}", bufs=1))

                mT = abig.tile([P, CA, NQ], BF, tag="mT", name="mT")
                hTr = D["hTd"].rearrange("(a p) n -> p a n", p=P)

                def wchunk(src, j):
                    wt = wbig.tile([P, CA, 512], BF, tag="w", name="wt")
                    nc.sync.dma_start(
                        wt[:], src[:, j * 512:(j + 1) * 512]
                        .rearrange("(a p) m -> p a m", p=P))
                    return wt

                for ci in range(2):
                    lo_c, hi_c = ci * 512, (ci + 1) * 512
                    # --- assemble h2 (in place into osum) for this half ---
                    osum = mwork.tile([P, CA, 512], BF, tag="osum",
                                      name="osum")
                    if with_attn:
                        osum8 = mwork.tile([P, CA, 512], F8, tag="osum8",
                                           name="osum8")
                        # Activation-queue DMA: this read waits on the
                        # AllReduce; on the SP queue it would head-of-line
                        # block the MLP weight stream.
                        nc.scalar.dma_start(
                            osum8[:],
                            ar1_out[ci].rearrange("(a p) n -> p a n", p=P))
                        nc.vector.tensor_copy(osum[:], osum8[:])
                        hTh = mwork.tile([P, CA, 512], BF, tag="hTh",
                                         name="hTh", bufs=1)
                        nc.sync.dma_start(hTh[:], hTr[:, :, lo_c:hi_c])
                        for a in range(CA):
                            nc.vector.tensor_add(osum[:, a, :],
                                                 osum[:, a, :], hTh[:, a, :])
                    else:
                        nc.sync.dma_start(osum[:], hTr[:, :, lo_c:hi_c])
                    # --- rmsnorm stats ---
                    var_ps = psd.tile([1, 512], F32, tag="d", name="var")
                    for a in range(CA):
                        sq = mwork.tile([P, 512], F32, tag="sq",
                                        name="sq", bufs=3)
                        nc.scalar.activation(sq[:], osum[:, a, :], AF.Square)
                        nc.tensor.matmul(var_ps[:], onesf[:, 0:1], sq[:],
                                         start=(a == 0), stop=(a == CA - 1))
                    sd2 = mst.tile([1, 512], F32, tag="sd2", name="sd2",
                                   bufs=2)
                    s2b = mst.tile([1, 512], BF, tag="s2b", name="s2b",
                                   bufs=2)
                    nc.vector.tensor_scalar(
                        out=sd2[:], in0=var_ps[:], scalar1=1.0 / C,
                        scalar2=1e-5, op0=OP.mult, op1=OP.add)
                    nc.scalar.activation(sd2[:], sd2[:], AF.Sqrt)
                    nc.vector.reciprocal(sd2[:], sd2[:])
                    nc.vector.tensor_copy(s2b[:], sd2[:])
                    s2bb = mst.tile([P, 512], BF, tag="s2bb", name="s2bb",
                                    bufs=2)
                    sb_ps = psa()
                    nc.tensor.matmul(sb_ps[:], onesb[0:1, :], s2b[0:1, :],
                                     start=True, stop=True)
                    nc.scalar.copy(s2bb[:], sb_ps[:])
                    for a in range(CA):
                        nc.vector.tensor_mul(mT[:, a, lo_c:hi_c],
                                             osum[:, a, :], s2bb[:])
                    if dbg:
                        nc.sync.dma_start(
                            dbg_t["d_h2"][:, lo_c:hi_c]
                            .rearrange("(a p) n -> p a n", p=P), osum[:])
                        if ci == 1:
                            nc.sync.dma_start(
                                dbg_t["d_mT"].rearrange("(a p) n -> p a n",
                                                        p=P), mT[:])

                    # --- MLP for this half ---
                    gact = mwork.tile([P, CA, 512], BF, tag="gact",
                                      name="gact")
                    for j in range(4):
                        wg = wchunk(D["wgate"], j)
                        for mfl in range(4):
                            mf = j * 4 + mfl
                            pp = psa()
                            for a in range(CA):
                                nc.tensor.matmul(
                                    pp[:], wg[:, a, mfl * P:(mfl + 1) * P],
                                    mT[:, a, lo_c:hi_c],
                                    start=(a == 0), stop=(a == CA - 1))
                            sgm = mwork.tile([P, 512], BF, tag="sgm",
                                             name="sgm", bufs=3)
                            nc.scalar.activation(sgm[:], pp[:], AF.Sigmoid)
                            nc.vector.tensor_mul(gact[:, mf, :], pp[:],
                                                 sgm[:])
                    for j in range(4):
                        wu = wchunk(D["wup"], j)
                        for mfl in range(4):
                            mf = j * 4 + mfl
                            pp = psa()
                            for a in range(CA):
                                nc.tensor.matmul(
                                    pp[:], wu[:, a, mfl * P:(mfl + 1) * P],
                                    mT[:, a, lo_c:hi_c],
                                    start=(a == 0), stop=(a == CA - 1))
                            nc.vector.tensor_mul(gact[:, mf, :], pp[:],
                                                 gact[:, mf, :])
                    for j in range(4):
                        wd = wchunk(D["wdown"], j)
                        for mcl in range(4):
                            pp = psa()
                            for a in range(CA):
                                nc.tensor.matmul(
                                    pp[:], wd[:, a, mcl * P:(mcl + 1) * P],
                                    gact[:, a, :],
                                    start=(a == 0), stop=(a == CA - 1))
                            dev = mwork.tile([P, 512], BF, tag="dev",
                                             name="dev", bufs=3)
                            nc.scalar.copy(dev[:], pp[:])
                            nc.scalar.dma_start(
                                ar2_in[ci][(j * 4 + mcl) * P:
                                           (j * 4 + mcl + 1) * P, :],
                                dev[:])
                    cc("ReduceScatter", OP.add, [ar2_in[ci][:]],
                       [rs2_out[ci][:]])

                # --- final assembly: outT = hTq + o_q + mlp_q ---
                hqr = D["hTq"].rearrange("(a p) n -> p a n", p=P)
                for ci in range(2):
                    lo_c, hi_c = ci * 512, (ci + 1) * 512
                    hq = mst.tile([P, 4, 512], BF, tag="hq", name="hq",
                                  bufs=1)
                    nc.sync.dma_start(hq[:], hqr[:, :, lo_c:hi_c])
                    if with_attn:
                        r18 = mst.tile([P, 4, 512], F8, tag="r18",
                                       name="r18", bufs=1)
                        for a in range(4):
                            nc.gpsimd.indirect_dma_start(
                                out=r18[:, a, :], out_offset=None,
                                in_=ar1_out[ci][:],
                                in_offset=bass.IndirectOffsetOnAxis(
                                    ap=qidx_sb[:, a:a + 1], axis=0))
                        r1 = mst.tile([P, 4, 512], BF, tag="r1", name="r1",
                                      bufs=1)
                        nc.vector.tensor_copy(r1[:], r18[:])
                        nc.vector.tensor_add(hq[:], hq[:], r1[:])
                    r2 = mst.tile([P, 4, 512], BF, tag="r2", name="r2",
                                  bufs=1)
                    nc.sync.dma_start(
                        r2[:], rs2_out[ci].rearrange("(a p) n -> p a n", p=P))
                    nc.vector.tensor_add(hq[:], hq[:], r2[:])
                    nc.sync.dma_start(
                        outT[:, lo_c:hi_c].rearrange("(a p) n -> p a n", p=P),
                        hq[:])


        def layer(rep):
            if phase == "mlp":
                mlp_section(rep, with_attn=False)
                return
            actx = ExitStack()
            att = actx.enter_context(tc.tile_pool(name=f"att{rep}", bufs=1))

            # q/k/v/oT (read until the end of attention) sit at the
            # base of the pool; hTn (dead after the projections) goes
            # above them, so the MLP weight pool reuses hTn's region
            # and its prefetch DMAs don't wait for attention to finish.
            q_sb = att.tile([P, 4, NQ], BF, tag="q")
            k_sb = att.tile([P, 4, NQ], BF, tag="k")
            v_sb = att.tile([P, 8, 512], BF, tag="v")
            oT_sb = att.tile([P, 4, NQ], BF, tag="oT")
            hTn_sb = att.tile([P, CA, NQ], BF, tag="hTn")
            hTn_r = D["hTn"].rearrange("(a p) n -> p a n", p=P)
            for ch in range(4):
                nc.sync.dma_start(
                    hTn_sb[:, ch * 4:(ch + 1) * 4, :],
                    hTn_r[:, ch * 4:(ch + 1) * 4, :])

            # =========================================================
            # offset network + q/k/v projections, interleaved issue so
            # the DVE/Act-heavy offset net hides under qkv matmuls and
            # the sampT AllGather overlaps the tail of the projections.
            # pre/wk2 sit at the top of the SBUF stack and are released
            # before the hd-projection tiles (khdp) allocate, so the
            # causal-attention pool (hw) below never waits on them.
            # =========================================================
            wpr = actx.enter_context(tc.tile_pool(name=f"wpra{rep}", bufs=2))
            rtp = actx.enter_context(tc.tile_pool(name=f"rtpa{rep}", bufs=3))
            hw = actx.enter_context(tc.tile_pool(name=f"hw{rep}", bufs=1))
            pctx = ExitStack()
            pre = pctx.enter_context(tc.tile_pool(name=f"pre{rep}", bufs=1))
            wk2 = pctx.enter_context(tc.tile_pool(name=f"wk2{rep}", bufs=1))

            # ---- offset stage 1: small DMAs + padded lr input ----
            grid_sb = pre.tile([P, 5, 2], F32, tag="grid")
            nc.sync.dma_start(
                grid_sb[:], D["grid"].rearrange("(s p) c -> p s c", p=P))
            convw_sb = pre.tile([P, 4, 9], F32, tag="convw")
            nc.sync.dma_start(
                convw_sb[:], D["convw"].rearrange("(a p) k -> p a k", p=P))
            wlr_sb = pre.tile([P, 4, 256], BF, tag="wlr")
            nc.sync.dma_start(
                wlr_sb[:], D["wlr"].rearrange("(a p) m -> p a m", p=P))
            woff_sb = pre.tile([P, 4, 2], BF, tag="woff")
            nc.sync.dma_start(
                woff_sb[:], D["woff"].rearrange("(a p) m -> p a m", p=P))
            lrin_sb = pre.tile([P, 4, NIMG], BF, tag="lrin")
            nc.sync.dma_start(
                lrin_sb[:], D["lrin"].rearrange("(a p) n -> p a n", p=P))
            xpad = pre.tile([P, 4, 26 * 26], BF, tag="xpad")
            nc.vector.memset(xpad[:], 0.0)
            acc_sb = pre.tile([P, 4, NIMG], BF, tag="acc")
            for a in range(4):
                x3 = xpad[:, a, :].rearrange("p (y x) -> p y x", y=26)
                nc.vector.tensor_copy(
                    x3[:, 1:25, 1:25],
                    lrin_sb[:, a, :].rearrange("p (y x) -> p y x", y=24))

            def conv_group(a):
                # TensorScalarPtr is DVE-only (Pool rejects it in codegen)
                eng = nc.vector
                x3 = xpad[:, a, :].rearrange("p (y x) -> p y x", y=26)
                a3 = acc_sb[:, a, :].rearrange("p (y x) -> p y x", y=24)
                for ky in range(3):
                    for kx in range(3):
                        w_ap = convw_sb[:, a, ky * 3 + kx:ky * 3 + kx + 1]
                        win = x3[:, ky:ky + 24, kx:kx + 24]
                        if ky == 0 and kx == 0:
                            eng.tensor_scalar(
                                out=a3, in0=win, scalar1=w_ap,
                                scalar2=None, op0=OP.mult)
                        else:
                            eng.scalar_tensor_tensor(
                                out=a3, in0=win, scalar=w_ap, in1=a3,
                                op0=OP.mult, op1=OP.add)

            def pnorm_stats(src_sb, na, eps):
                """mean/var over na*128 partitions (PE ones-matmul sums)"""
                red = wk2.tile([1, NIMG], F32, tag="st", bufs=4, name="red")
                red2 = wk2.tile([1, NIMG], F32, tag="st", bufs=4, name="red2")
                sqs = [wk2.tile([P, NIMG], F32, tag="sq1", bufs=1,
                                name="sq1") for _ in range(1)]
                ones_l = onesf if src_sb.dtype == F32 else onesb
                for lo, hi in ((0, 512), (512, NIMG)):
                    rp = psd.tile([1, 512], F32, tag="d", name="rp")
                    for a in range(na):
                        nc.tensor.matmul(rp[:, :hi - lo], ones_l[:, 0:1],
                                         src_sb[:, a, lo:hi],
                                         start=(a == 0), stop=(a == na - 1))
                    nc.scalar.copy(red[0:1, lo:hi], rp[:, :hi - lo])
                rp2 = psd.tile([1, 512], F32, tag="d", name="rp2")
                rp3 = psd.tile([1, 512], F32, tag="d", name="rp3")
                for a in range(na):
                    sq = sqs[0]
                    nc.scalar.activation(sq[:], src_sb[:, a, :], AF.Square)
                    nc.tensor.matmul(rp2[:], onesf[:, 0:1], sq[:, 0:512],
                                     start=(a == 0), stop=(a == na - 1))
                    nc.tensor.matmul(rp3[:, :NIMG - 512], onesf[:, 0:1],
                                     sq[:, 512:NIMG],
                                     start=(a == 0), stop=(a == na - 1))
                nc.scalar.copy(red2[0:1, 0:512], rp2[:])
                nc.scalar.copy(red2[0:1, 512:NIMG], rp3[:, :NIMG - 512])
                nch = float(na * P)
                mu = wk2.tile([1, NIMG], F32, tag="st", bufs=4, name="mu")
                nc.scalar.mul(mu[:], red[:], 1.0 / nch)
                var = wk2.tile([1, NIMG], F32, tag="st", bufs=4, name="var")
                nc.vector.tensor_mul(var[:], mu[:], mu[:])
                nc.vector.scalar_tensor_tensor(
                    out=var[:], in0=red2[:], scalar=1.0 / nch,
                    in1=var[:], op0=OP.mult, op1=OP.subtract)
                nc.vector.tensor_scalar(out=var[:], in0=var[:],
                                        scalar1=eps, scalar2=None, op0=OP.add)
                nc.scalar.activation(var[:], var[:], AF.Sqrt)
                inv = wk2.tile([1, NIMG], F32, tag="inv", name="inv")
                nc.vector.reciprocal(inv[:], var[:])
                aoff = wk2.tile([1, NIMG], F32, tag="aoff", name="aoff")
                nc.vector.scalar_tensor_tensor(
                    out=aoff[:], in0=mu[:], scalar=-1.0, in1=inv[:],
                    op0=OP.mult, op1=OP.mult)
                return inv, aoff

            def pnorm_bcast(inv, aoff):
                invb = wk2.tile([1, NIMG], BF, tag="invb", name="invb")
                aofb = wk2.tile([1, NIMG], BF, tag="aofb", name="aofb")
                nc.scalar.copy(invb[:], inv[:])
                nc.scalar.copy(aofb[:], aoff[:])
                ib = wk2.tile([P, NIMG], BF, tag="ibb", name="ibb")
                ab = wk2.tile([P, NIMG], BF, tag="abb", name="abb")
                for lo, hi in ((0, 512), (512, NIMG)):
                    pi = psd.tile([P, 512], F32, tag="d", name="pi")
                    nc.tensor.matmul(pi[:, :hi - lo], onesb[0:1, :],
                                     invb[0:1, lo:hi], start=True, stop=True)
                    nc.scalar.copy(ib[:, lo:hi], pi[:, :hi - lo])
                    pa = psd.tile([P, 512], F32, tag="d", name="pa")
                    nc.tensor.matmul(pa[:, :hi - lo], onesb[0:1, :],
                                     aofb[0:1, lo:hi], start=True, stop=True)
                    nc.scalar.copy(ab[:, lo:hi], pa[:, :hi - lo])
                return ib, ab

            # ---- qkv projection helpers ----
            def rope_evict(dst, raw_sb, pos_lo, pos_hi):
                n = pos_hi - pos_lo
                rp = psa()
                nc.tensor.matmul(rp[:, :n], swapm[:], raw_sb[:, :n],
                                 start=True, stop=True)
                tmp1 = rtp.tile([P, 512], BF, tag="rt1", name="rt1", bufs=2)
                nc.vector.tensor_mul(tmp1[:, :n], raw_sb[:, :n],
                                     cost[:, pos_lo:pos_hi])
                tmp2 = rtp.tile([P, 512], BF, tag="rt2", name="rt2", bufs=2)
                nc.vector.tensor_mul(tmp2[:, :n], rp[:, :n],
                                     sint[:, pos_lo:pos_hi])
                nc.vector.tensor_add(dst[:, pos_lo:pos_hi], tmp1[:, :n],
                                     tmp2[:, :n])

            def qk_load(wname):
                wt = wpr.tile([P, CA, 512], BF, tag="wpr", name="wt")
                nc.sync.dma_start(
                    wt[:], D[wname].rearrange("(a p) m -> p a m", p=P))
                return wt

            def qk_heads(wt, dst, src_sb, heads, n_src):
                for h in heads:
                    for lo, hi in ((0, 512), (512, n_src)):
                        pp = psa()
                        for a in range(CA):
                            nc.tensor.matmul(pp[:, :hi - lo],
                                             wt[:, a, h * P:(h + 1) * P],
                                             src_sb[:, a, lo:hi],
                                             start=(a == 0),
                                             stop=(a == CA - 1))
                        raw = rtp.tile([P, 512], BF, tag="raw", name="raw")
                        nc.scalar.copy(raw[:, :hi - lo], pp[:, :hi - lo])
                        rope_evict(dst[:, h, :], raw, lo, hi)

            # ---- interleaved issue ----
            wt_q = qk_load("wq")
            conv_group(0)
            conv_group(2)
            qk_heads(wt_q, q_sb, hTn_sb, (0, 1), NQ)
            conv_group(1)
            conv_group(3)
            qk_heads(wt_q, q_sb, hTn_sb, (2, 3), NQ)

            inv1, aoff1 = pnorm_stats(acc_sb, 4, 1e-6)
            ib1, ab1 = pnorm_bcast(inv1, aoff1)
            xg_sb = pre.tile([P, 4, NIMG], BF, tag="xg")
            sgt = wk2.tile([P, NIMG], BF, tag="sgt", name="sgt")
            xh = wk2.tile([P, NIMG], F32, tag="xh", name="xh")
            for a in range(4):
                nc.vector.tensor_mul(xh[:], acc_sb[:, a, :], ib1[:])
                nc.vector.tensor_add(xh[:], xh[:], ab1[:])
                nc.scalar.activation(sgt[:], xh[:], AF.Sigmoid, scale=1.702)
                nc.vector.tensor_mul(xg_sb[:, a, :], xh[:], sgt[:])

            wt_k = qk_load("wk")
            qk_heads(wt_k, k_sb, hTn_sb, (0, 1), NQ)
            if dbg:
                nc.sync.dma_start(
                    dbg_t["d_q"].rearrange("(h p) n -> p h n", p=P), q_sb[:])

            # intent vector
            hmean = wk2.tile([P, CA], F32, tag="hmean", name="hmean")
            hmb = wk2.tile([P, CA], BF, tag="hmb", name="hmb")
            for a in range(CA):
                nc.vector.tensor_reduce(
                    hmean[:, a:a + 1], hTn_sb[:, a, :],
                    axis=mybir.AxisListType.X, op=OP.add)
            nc.vector.tensor_copy(hmb[:], hmean[:])
            intent = wk2.tile([P, 2], F32, tag="intent", name="intent")
            for m in range(4):
                wint_sb = wk2.tile([P, CA, 64], BF, tag="wint",
                                   name="wint_sb", bufs=1)
                nc.sync.dma_start(
                    wint_sb[:],
                    D["wint"][:, m * 64:(m + 1) * 64]
                    .rearrange("(a p) m -> p a m", p=P))
                ip = psm.tile([P, P], F32, tag="t", name="ip")
                prow = slice((m % 2) * 64, (m % 2) * 64 + 64)
                for a in range(CA):
                    nc.tensor.matmul(ip[prow, 0:1], wint_sb[:, a, :],
                                     hmb[:, a:a + 1], start=(a == 0),
                                     stop=(a == CA - 1))
                nc.scalar.mul(intent[prow, m // 2:m // 2 + 1],
                              ip[prow, 0:1], 1.0 / NQ)

            # cat = [xproj ; intent] -> ln2 (in place) -> off
            cat_sb = pre.tile([P, 4, NIMG], BF, tag="cat")
            for m in range(2):
                for lo, hi in ((0, 512), (512, NIMG)):
                    xp = psd.tile([P, 512], F32, tag="d", name="xp")
                    for a in range(4):
                        nc.tensor.matmul(xp[:, :hi - lo],
                                         wlr_sb[:, a, m * P:(m + 1) * P],
                                         xg_sb[:, a, lo:hi],
                                         start=(a == 0), stop=(a == 3))
                    nc.scalar.copy(cat_sb[:, m, lo:hi], xp[:, :hi - lo])
            for m in range(2):
                nc.vector.tensor_scalar(
                    out=cat_sb[:, 2 + m, :], in0=xg_sb[:, 0, :],
                    scalar1=0.0, scalar2=intent[:, m:m + 1], op0=OP.mult,
                    op1=OP.add)

            qk_heads(wt_k, k_sb, hTn_sb, (2, 3), NQ)
            if dbg:
                nc.sync.dma_start(
                    dbg_t["d_k"].rearrange("(h p) n -> p h n", p=P), k_sb[:])

            inv2, aoff2 = pnorm_stats(cat_sb, 4, 1e-6)
            ib2, ab2 = pnorm_bcast(inv2, aoff2)
            for a in range(4):
                nc.vector.tensor_mul(xh[:], cat_sb[:, a, :], ib2[:])
                nc.vector.tensor_add(cat_sb[:, a, :], xh[:], ab2[:])

            off_sb = wk2.tile([2, NPAD], F32, tag="off", name="off")
            nc.vector.memset(off_sb[:], 0.0)
            for lo, hi in ((0, 512), (512, NIMG)):
                op_ = psd.tile([2, 512], F32, tag="d", name="opp")
                for a in range(4):
                    nc.tensor.matmul(op_[:, :hi - lo], woff_sb[:, a, :],
                                     cat_sb[:, a, lo:hi], start=(a == 0),
                                     stop=(a == 3))
                nc.scalar.copy(off_sb[:, lo:hi], op_[:, :hi - lo])

            # bilinear coordinates, batched across all 5 s-tiles
            idx0 = wk2.tile([P, 5], I32, tag="idx0", name="idx0")
            idx1 = wk2.tile([P, 5], I32, tag="idx1", name="idx1")
            wcmb = wk2.tile([P, 5, 4], F32, tag="wcmb", name="wcmb")
            t2 = wk2.tile([P, 5, 2], F32, tag="t2", name="t2")
            fr = wk2.tile([P, 5, 2], F32, tag="fr", name="fr")
            f0 = wk2.tile([P, 5, 2], F32, tag="f0", name="f0")
            f1 = wk2.tile([P, 5, 2], F32, tag="f1", name="f1")
            w1m = wk2.tile([P, 5, 2], F32, tag="w1m", name="w1m")
            fi = wk2.tile([P, 5, 1], F32, tag="fi", name="fi")
            tps_c = psm.tile([P, 5, 2], F32, tag="t", name="tps_c")
            for st in range(5):
                nc.tensor.transpose(tps_c[:, st, :],
                                    off_sb[0:2, st * P:(st + 1) * P],
                                    idf[0:2, 0:2])
            nc.scalar.activation(t2[:], tps_c[:], AF.Tanh)
            nc.vector.scalar_tensor_tensor(
                out=t2[:], in0=t2[:], scalar=2.0 / LR,
                in1=grid_sb[:], op0=OP.mult, op1=OP.add)
            nc.vector.tensor_scalar(out=t2[:], in0=t2[:], scalar1=1.0,
                                    scalar2=-1.0, op0=OP.min, op1=OP.max)
            nc.vector.tensor_scalar(out=t2[:], in0=t2[:], scalar1=1.0,
                                    scalar2=(HR - 1) / 2.0,
                                    op0=OP.add, op1=OP.mult)
            ti = wk2.tile([P, 5, 2], I32, tag="ti", name="ti")
            nc.vector.tensor_copy(ti[:], t2[:])
            nc.vector.tensor_copy(f0[:], ti[:])
            nc.vector.tensor_tensor(out=fr[:], in0=f0[:], in1=t2[:],
                                    op=OP.is_gt)
            nc.vector.tensor_sub(f0[:], f0[:], fr[:])
            nc.vector.tensor_sub(fr[:], t2[:], f0[:])
            nc.vector.tensor_scalar(out=f1[:], in0=f0[:], scalar1=1.0,
                                    scalar2=float(HR - 1), op0=OP.add,
                                    op1=OP.min)
            nc.vector.scalar_tensor_tensor(
                out=fi[:], in0=f0[:, :, 0:1], scalar=float(HR),
                in1=f0[:, :, 1:2], op0=OP.mult, op1=OP.add)
            nc.vector.tensor_copy(idx0[:], fi[:, :, 0])
            nc.vector.scalar_tensor_tensor(
                out=fi[:], in0=f1[:, :, 0:1], scalar=float(HR),
                in1=f0[:, :, 1:2], op0=OP.mult, op1=OP.add)
            nc.vector.tensor_copy(idx1[:], fi[:, :, 0])
            nc.vector.tensor_scalar(out=w1m[:], in0=fr[:],
                                    scalar1=-1.0, scalar2=1.0,
                                    op0=OP.mult, op1=OP.add)
            nc.vector.tensor_mul(wcmb[:, :, 0:1], w1m[:, :, 0:1],
                                 w1m[:, :, 1:2])
            nc.vector.tensor_mul(wcmb[:, :, 1:2], w1m[:, :, 0:1],
                                 fr[:, :, 1:2])
            nc.vector.tensor_mul(wcmb[:, :, 2:3], fr[:, :, 0:1],
                                 w1m[:, :, 1:2])
            nc.vector.tensor_mul(wcmb[:, :, 3:4], fr[:, :, 0:1],
                                 fr[:, :, 1:2])

            wt_v = qk_load("wv")
            for m8 in range(4):
                pp = psa()
                for a in range(CA):
                    nc.tensor.matmul(pp[:],
                                     hTn_sb[:, a, m8 * P:(m8 + 1) * P],
                                     wt_v[:, a, :], start=(a == 0),
                                     stop=(a == CA - 1))
                nc.scalar.copy(v_sb[:, m8, :], pp[:])

            # gather + combine + transpose
            sampT_mine = pre.tile([P, 4, NPAD], F8, tag="sampT_mine")
            for st in range(5):
                p0 = wk2.tile([P, 1024], BF, tag="p0", bufs=1, name="p0")
                p1 = wk2.tile([P, 1024], BF, tag="p1", bufs=1, name="p1")
                nc.gpsimd.indirect_dma_start(
                    out=p0[:], out_offset=None, in_=D["imgp"][:],
                    in_offset=bass.IndirectOffsetOnAxis(
                        ap=idx0[:, st:st + 1], axis=0))
                nc.gpsimd.indirect_dma_start(
                    out=p1[:], out_offset=None, in_=D["imgp"][:],
                    in_offset=bass.IndirectOffsetOnAxis(
                        ap=idx1[:, st:st + 1], axis=0))
                smp = wk2.tile([P, 512], BF, tag="smp", bufs=2, name="smp")
                nc.vector.tensor_tensor(
                    out=smp[:], in0=p0[:, 0:512],
                    in1=wcmb[:, st, 0:1].to_broadcast([P, 512]), op=OP.mult)
                for pair, col in ((p0, 1), (p1, 2), (p1, 3)):
                    src = pair[:, 0:512] if col == 2 else pair[:, 512:1024]
                    nc.vector.scalar_tensor_tensor(
                        out=smp[:], in0=src,
                        scalar=wcmb[:, st, col:col + 1], in1=smp[:],
                        op0=OP.mult, op1=OP.add)
                for cm in range(4):
                    tp = psm.tile([P, P], BF, tag="t", name="tps")
                    nc.tensor.transpose(tp[:], smp[:, cm * P:(cm + 1) * P],
                                        idb[:])
                    nc.scalar.copy(
                        sampT_mine[:, cm, st * P:(st + 1) * P], tp[:])
            nc.sync.dma_start(ag_in.rearrange("(a p) n -> p a n", p=P),
                              sampT_mine[:, :, 0:NIMG])
            cc("AllGather", OP.bypass, [ag_in[:]], [ag_out[:]])

            # remaining v tiles while AllGather flies
            for m8 in range(4, 8):
                pp = psa()
                for a in range(CA):
                    nc.tensor.matmul(pp[:],
                                     hTn_sb[:, a, m8 * P:(m8 + 1) * P],
                                     wt_v[:, a, :], start=(a == 0),
                                     stop=(a == CA - 1))
                nc.scalar.copy(v_sb[:, m8, :], pp[:])

            # =========================================================
            # attention: causal tiles first (they only need q/k/v, so
            # they fill the AllGather window), head-pairs interleaved
            # for PE pipelining; image tiles + softmax finalize after
            # the hd projections land. Denominators accumulate on the
            # otherwise-idle Pool engine.
            # =========================================================
            daccs, oAs = {}, {}
            sampT_sb = khd_sb = vhd_sb = None

            def sc_exp_o(ci, pair, kind, kt, o_pss, start, stop,
                         dinit=False):
                lo_c, hi_c = ci * 512, (ci + 1) * 512
                if kind == "c":
                    qlo, kp = kt * P, P
                else:
                    qlo = 0
                    kp = P if kt < 4 else NIMG - 4 * P
                lo = max(qlo, lo_c)
                n = hi_c - lo
                o = lo - lo_c
                for h in pair:
                    dacc = daccs[(ci, h)]
                    sp = psa()
                    if kind == "c":
                        nc.tensor.matmul(sp[:, :n],
                                         k_sb[:, h, kt * P:(kt + 1) * P],
                                         q_sb[:, h, lo:hi_c],
                                         start=True, stop=True)
                        if lo == qlo:
                            nc.vector.tensor_add(sp[:, 0:P], sp[:, 0:P],
                                                 maskd[:])
                        lhs = v_sb[:, kt, h * P:(h + 1) * P]
                    else:
                        nc.tensor.matmul(sp[:kp, :n],
                                         khd_sb[:, h, kt * P:kt * P + kp],
                                         q_sb[:, h, lo:hi_c],
                                         start=True, stop=True)
                        lhs = vhd_sb[:kp, kt, h * P:(h + 1) * P]
                    ex = hw.tile([P, 512], BF, tag="ex", name="ex", bufs=3)
                    nc.scalar.activation(ex[:kp, o:], sp[:kp, :n], AF.Exp,
                                         scale=SCALE)
                    nc.tensor.matmul(o_pss[h][:, o:], lhs, ex[:kp, o:],
                                     start=start, stop=stop)
                    deng = nc.gpsimd if DACC_POOL else nc.vector
                    if dinit:
                        deng.tensor_copy(dacc[:kp, :], ex[:kp, :])
                    else:
                        deng.tensor_add(dacc[:kp, o:], dacc[:kp, o:],
                                        ex[:kp, o:])

            def causal_pass(ci, pair):
                ncaus = 4 * (ci + 1)
                o_pss = {h: psa() for h in pair}
                for h in pair:
                    daccs[(ci, h)] = hw.tile([P, 512], F32, tag="dacc",
                                             name=f"dc{ci}{h}", bufs=8)
                for kt in range(ncaus):
                    sc_exp_o(ci, pair, "c", kt, o_pss,
                             start=(kt == 0), stop=(kt == ncaus - 1),
                             dinit=(kt == 0))
                for h in pair:
                    oA = hw.tile([P, 512], BF, tag="oA",
                                 name=f"oA{ci}{h}", bufs=8)
                    nc.scalar.copy(oA[:], o_pss[h][:])
                    oAs[(ci, h)] = oA

            def img_pass(ci, pair):
                lo_c, hi_c = ci * 512, (ci + 1) * 512
                o_pss = {h: psa() for h in pair}
                for it in range(5):
                    sc_exp_o(ci, pair, "i", it, o_pss,
                             start=(it == 0), stop=(it == 4))
                for h in pair:
                    den = psd.tile([1, 512], F32, tag="d", name="den")
                    nc.tensor.matmul(den[:], onesf[:, 0:1],
                                     daccs[(ci, h)][:], start=True, stop=True)
                    rcf = hw.tile([1, 512], F32, tag="rcf", name="rcf",
                                  bufs=1)
                    rcb = hw.tile([1, 512], BF, tag="rcb", name="rcb",
                                  bufs=2)
                    nc.vector.reciprocal(rcf[:], den[:])
                    nc.vector.tensor_copy(rcb[:], rcf[:])
                    rb = psa()
                    nc.tensor.matmul(rb[:], onesb[0:1, :], rcb[0:1, :],
                                     start=True, stop=True)
                    rbs = hw.tile([P, 512], BF, tag="rbs", name="rbs",
                                  bufs=1)
                    nc.scalar.copy(rbs[:], rb[:])
                    otmp = hw.tile([P, 512], BF, tag="otmp", name="otmp",
                                   bufs=1)
                    nc.vector.tensor_add(otmp[:], oAs[(ci, h)][:],
                                         o_pss[h][:])
                    nc.vector.tensor_mul(oT_sb[:, h, lo_c:hi_c], otmp[:],
                                         rbs[:])

            # causal part of attention (during the AllGather flight)
            for ci in range(2):
                for pair in ((0, 1), (2, 3)):
                    causal_pass(ci, pair)

            pctx.close()

            # hd-tile pool reuses the released pre/wk2 region; its
            # writes only depend on the AllGather anyway.
            khdp = actx.enter_context(tc.tile_pool(name=f"khdp{rep}",
                                                   bufs=1))
            sampT_sb = khdp.tile([P, CA, NIMG], BF, tag="sampT")
            sampT8_sb = khdp.tile([P, CA, NIMG], F8, tag="sampT8")
            khd_sb = khdp.tile([P, 4, NIMG], BF, tag="khd")
            vhd_sb = khdp.tile([P, 5, 512], BF, tag="vhd")
            wo_sb = khdp.tile([P, 4, C], BF, tag="wo")
            wt_khd = qk_load("wkhd")
            wt_vhd = qk_load("wvhd")
            nc.sync.dma_start(
                wo_sb[:], D["wo"].rearrange("(a p) m -> p a m", p=P))
            # Activation-queue DMA: waits on the AllGather; on SP it would
            # block the o-proj eviction stream and MLP weight prefetch.
            nc.scalar.dma_start(
                sampT8_sb[:], ag_out.rearrange("(a p) n -> p a n", p=P))
            # fp8 -> bf16 in column chunks so khd starts during the cast
            nc.vector.tensor_copy(sampT_sb[:, :, 0:512],
                                  sampT8_sb[:, :, 0:512])
            nc.vector.tensor_copy(sampT_sb[:, :, 512:NIMG],
                                  sampT8_sb[:, :, 512:NIMG])
            if dbg:
                nc.sync.dma_start(
                    dbg_t["d_samp"].rearrange("(a p) n -> p a n", p=P),
                    sampT_sb[:])

            # ---- hd-token projections (need the AllGather result) ----
            for h in range(4):
                for lo, hi in ((0, 512), (512, NIMG)):
                    pp = psa()
                    for a in range(CA):
                        nc.tensor.matmul(pp[:, :hi - lo],
                                         wt_khd[:, a, h * P:(h + 1) * P],
                                         sampT_sb[:, a, lo:hi],
                                         start=(a == 0), stop=(a == CA - 1))
                    raw = rtp.tile([P, 512], BF, tag="raw", name="raw")
                    nc.scalar.copy(raw[:, :hi - lo], pp[:, :hi - lo])
                    rope_evict(khd_sb[:, h, :], raw, lo, hi)
            if dbg:
                nc.sync.dma_start(
                    dbg_t["d_khd"].rearrange("(h p) n -> p h n", p=P),
                    khd_sb[:])

            for st in range(5):
                kp = P if st < 4 else NIMG - 4 * P
                pp = psa()
                for a in range(CA):
                    nc.tensor.matmul(pp[:kp, :],
                                     sampT_sb[:, a, st * P:st * P + kp],
                                     wt_vhd[:, a, :], start=(a == 0),
                                     stop=(a == CA - 1))
                nc.scalar.copy(vhd_sb[:kp, st, :], pp[:kp, :])

            # ---- image attention + o-projection per token half ----
            for ci in range(2):
                lo_c, hi_c = ci * 512, (ci + 1) * 512
                img_pass(ci, (0, 1))
                img_pass(ci, (2, 3))
                if dbg and ci == 1:
                    nc.sync.dma_start(
                        dbg_t["d_oT"].rearrange("(h p) n -> p h n", p=P),
                        oT_sb[:])

                # o-projection for this token half -> ReduceScatter -> AG
                for m in range(CA):
                    pp = psa()
                    for h in range(4):
                        nc.tensor.matmul(pp[:], wo_sb[:, h, m * P:(m + 1) * P],
                                         oT_sb[:, h, lo_c:hi_c],
                                         start=(h == 0), stop=(h == 3))
                    oev = khdp.tile([P, 512], F8, tag="oev", bufs=3,
                                    name="oev")
                    nc.scalar.copy(oev[:], pp[:])
                    # Act-queue DMA: an SP-queue write here would stall SP
                    # on o-proj completion and block MLP weight prefetch.
                    nc.scalar.dma_start(ar1_in[ci][m * P:(m + 1) * P, :],
                                        oev[:])
                cc("AllReduce", OP.add, [ar1_in[ci][:]], [ar1_out[ci][:]])

            actx.close()

            if phase == "attn":
                with ExitStack() as lctx:
                    mstx = lctx.enter_context(
                        tc.tile_pool(name=f"mstx{rep}", bufs=2))
                    hqr = D["hTq"].rearrange("(a p) n -> p a n", p=P)
                    for ci in range(2):
                        lo_c, hi_c = ci * 512, (ci + 1) * 512
                        hq = mstx.tile([P, 4, 512], BF, tag="hq", name="hq")
                        nc.sync.dma_start(hq[:], hqr[:, :, lo_c:hi_c])
                        r1 = mstx.tile([P, 4, 512], BF, tag="r1", name="r1")
                        for a in range(4):
                            nc.gpsimd.indirect_dma_start(
                                out=r1[:, a, :], out_offset=None,
                                in_=ar1_out[ci][:],
                                in_offset=bass.IndirectOffsetOnAxis(
                                    ap=qidx_sb[:, a:a + 1], axis=0))
                        nc.vector.tensor_add(hq[:], hq[:], r1[:])
                        nc.sync.dma_start(
                            outT[:, lo_c:hi_c]
                            .rearrange("(a p) n -> p a n", p=P), hq[:])
                return

            mlp_section(rep, with_attn=True)

        for rep in range(reps):
            layer(rep)

    nc.compile()
    return nc


import time
import jax
from jax.sharding import Mesh, PartitionSpec
from jax.experimental.shard_map import shard_map
from concourse import bass2jax
from concourse.bass2jax import _bass_exec_p, install_neuronx_cc_hook, \
    partition_id_tensor


class TimedRunner:
    def __init__(self, nc, n_cores=8):
        install_neuronx_cc_hook()
        self.nc = nc
        self.n_cores = n_cores
        partition_name = (nc.partition_id_tensor.name
                          if nc.partition_id_tensor else None)
        in_names, out_names, out_avals, zero_outs = [], [], [], []
        for alloc in nc.m.functions[0].allocations:
            if not isinstance(alloc, mybir.MemoryLocationSet):
                continue
            name = alloc.memorylocations[0].name
            if alloc.kind == "ExternalInput":
                if name != partition_name:
                    in_names.append(name)
            elif alloc.kind == "ExternalOutput":
                out_names.append(name)
                shape = tuple(alloc.tensor_shape)
                dtype = mybir.dt.np(alloc.dtype)
                out_avals.append(jax.core.ShapedArray(shape, dtype))
                zero_outs.append(np.zeros(shape, dtype))
        if nc.dbg_addr is not None:
            assert not nc.dbg_callbacks
        self.in_names = list(in_names)
        self.out_names = out_names
        self.out_avals = out_avals
        self.zero_outs = zero_outs
        n_params = len(in_names)
        n_outs = len(out_avals)
        all_in_names = list(in_names) + list(out_names)
        if partition_name is not None:
            all_in_names.append(partition_name)
        self.partition_name = partition_name

        def _body(*args):
            operands = list(args)
            if partition_name is not None:
                operands.append(partition_id_tensor())
            outs = _bass_exec_p.bind(
                *operands,
                out_avals=tuple(out_avals),
                in_names=tuple(all_in_names),
                out_names=tuple(out_names),
                lowering_input_output_aliases=(),
                sim_require_finite=True,
                sim_require_nnan=True,
                nc=nc,
            )
            return tuple(outs)

        devices = jax.devices()[:n_cores]
        mesh = Mesh(np.asarray(devices), ("core",))
        in_specs = (PartitionSpec("core"),) * (n_params + n_outs)
        out_specs = (PartitionSpec("core"),) * n_outs
        # no donation so the function is re-callable with the same buffers
        self.fn = jax.jit(shard_map(_body, mesh=mesh, in_specs=in_specs,
                                    out_specs=out_specs, check_rep=False))
        self.mesh = mesh

    def put_inputs(self, in_maps):
        dbg = {}
        if self.nc.dbg_addr is not None:
            dbg = {self.nc.dbg_addr.name: np.zeros((1, 2), np.uint32)}
        per_core = [[np.asarray({**m, **dbg}[n]) for n in self.in_names]
                    for m in in_maps]
        n_params = len(self.in_names)
        concat_in = [
            np.concatenate([per_core[c][i] for c in range(self.n_cores)],
                           axis=0) for i in range(n_params)]
        concat_zeros = [
            np.zeros((self.n_cores * z.shape[0], *z.shape[1:]), z.dtype)
            for z in self.zero_outs]
        sh = jax.sharding.NamedSharding(self.mesh, PartitionSpec("core"))
        self.dev_args = [jax.device_put(a, sh)
                         for a in (*concat_in, *concat_zeros)]

    def run(self):
        outs = jax.block_until_ready(self.fn(*self.dev_args))
        return outs

    def results(self, outs):
        return [
            {n: np.asarray(outs[i]).reshape(
                self.n_cores, *self.out_avals[i].shape)[c]
             for i, n in enumerate(self.out_names)}
            for c in range(self.n_cores)
        ]

    def bench(self, iters=5):
        self.run()
        best = float("inf")
        for _ in range(iters):
            t0 = time.perf_counter()
            self.run()
            best = min(best, time.perf_counter() - t0)
        return best


# ----------------------------------------------------------------- entry
_NC_CACHE = {}


def _get_nc(reps=1):
    if reps not in _NC_CACHE:
        _NC_CACHE[reps] = build(dbg=False, reps=reps)
    return _NC_CACHE[reps]


def kernel(**inputs) -> np.ndarray:
    """Full inputs -> full [2, 1024, 2048] fp32 output, computed on the
    8 TRN2 NeuronCores (DPxTP sharding, bf16 compute)."""
    from concourse.bass_utils import run_bass_kernel_spmd
    nc = _get_nc(1)
    maps = prep_inputs(inputs)
    res = run_bass_kernel_spmd(nc, maps, list(range(8)))
    return finish(res.results)


def benchmark_device_time(inputs, reps_hi=11, npipe=16, trials=16):
    """Per-layer device execution time: difference an on-device
    reps_hi-iteration NEFF against the single-iteration NEFF under
    pipelined launches (axon host dispatch is ~100ms and would otherwise
    swamp the sub-ms kernel). Samples are interleaved lo/hi/lo so slow
    host-side drift cancels; median over trials for jitter robustness."""
    import time as _time
    import statistics as _stats
    import jax as _jax
    maps = prep_inputs(inputs)
    trs = {}
    for reps in (1, reps_hi):
        tr = TimedRunner(_get_nc(reps), 8)
        tr.put_inputs(maps)
        tr.run()
        trs[reps] = tr

    def sample(tr):
        t0 = _time.perf_counter()
        outs = None
        for _ in range(npipe):
            outs = tr.fn(*tr.dev_args)
        _jax.block_until_ready(outs)
        return (_time.perf_counter() - t0) / npipe

    sample(trs[1])
    sample(trs[reps_hi])
    diffs = []
    for _ in range(trials):
        a = sample(trs[1])
        b = sample(trs[reps_hi])
        a2 = sample(trs[1])
        diffs.append((b - (a + a2) / 2) / (reps_hi - 1))
    return max(_stats.median(diffs), 1e-9)
